# revision 1
# baseline (speedup 1.0000x reference)
"""Trainium2 Bass kernel: BiLSTM dependency-parser edge scorer.

Self-contained. Accepts FULL inputs (as produced by setup_inputs()), returns
the FULL [65025, 1] float32 score tensor.

Algorithm mapping (per NeuronCore, SPMD over 8 cores):
  - embeddings gathered on device via indirect DMA (replicated on all cores)
  - 2-layer BiLSTM replicated on every core; recurrent matvec runs on the
    tensor engine with h as the (tiny) stationary operand and Whh^T streamed,
    4-way column-tiled so the 4 PE column groups stream concurrently.
    Precomputed input projections xg[t] are injected into the same PSUM
    accumulation group as a rank-1 (K=1) matmul row.
    All gate nonlinearities use a single sigmoid table:
    tanh(x) = 2*sigmoid(2x) - 1 (the g-gate rows are pre-scaled by 2 on host).
  - Edge MLP is factored: scores[h,m] = w2 . tanh(A[h] + B[m] + b1) + b2 with
    A = h1 @ Uh^T, B = h1 @ Um^T (Uh/Um = halves of fc1_W). Each core computes
    a [32, 256] slice of the full score grid (rows selected by a per-core
    one-hot matrix input); the host assembles and compacts to edge order.
"""

import os
import sys

sys.path.insert(0, "/opt/trn_rl_repo")

import numpy as np

import concourse.bass as bass
import concourse.mybir as mybir
from concourse import bacc
from concourse.bass import IndirectOffsetOnAxis
from concourse.masks import make_identity
from concourse.tile import TileContext

N = 256          # sequence length
H = 400          # hidden size per direction
G = 1600         # 4*H gate rows
NC = 8           # cores
F32 = mybir.dt.float32
BF16 = mybir.dt.float16
F32R = mybir.dt.float32r
I32 = mybir.dt.int32
AF = mybir.ActivationFunctionType
OP = mybir.AluOpType

# number of recurrence steps actually emitted (256 for real runs; smaller for
# simulator bring-up via env var)
STEPS = int(os.environ.get("DP_STEPS", str(N)))


# ---------------------------------------------------------------------------
# host-side weight layout prep
# ---------------------------------------------------------------------------

_P = np.arange(128)


def _bf(a):
    return np.ascontiguousarray(np.asarray(a).astype(np.float16))


def _gate_perm():
    """perm[new] = old gate-row index.

    New order: n = 400*g + 100*gate + j  where g = unit//100 (PE col group),
    j = unit%100; original r = 400*gate + unit, gate order (i, f, g, o).
    """
    perm = np.empty(G, dtype=np.int64)
    for g in range(4):
        for gt in range(4):
            for j in range(100):
                unit = 100 * g + j
                perm[400 * g + 100 * gt + j] = 400 * gt + unit
    return perm


_PERM = _gate_perm()


def _scale_rows(W):
    """Scale the g-gate rows (original rows 800:1200) by 2 for the
    tanh-via-sigmoid trick. W: [1600, ...] or [1600]."""
    Ws = np.array(W, dtype=np.float64)
    Ws[800:1200] *= 2.0
    return Ws


def _kmap_block(D):
    """Block K-chunk maps for a D-dim hidden vector (D = 400 or 800).

    Chunk kc = 4*half + b; unit(p, kc) = 400*half + 100*(p//32) + 32*b + (p%32)
    valid iff 32*b + p%32 < 100. Matches the DVE 32x32 block-transpose layout
    of h tiles (data rows {0,32,64,96}, cols 0:100).
    Returns (U [nkc,128] int, V [nkc,128] float 0/1).
    """
    Us, Vs = [], []
    for half in range(D // 400):
        for b in range(4):
            u = 400 * half + 100 * (_P // 32) + 32 * b + (_P % 32)
            v = (32 * b + (_P % 32)) < 100
            Us.append(np.where(v, u, 0))
            Vs.append(v.astype(np.float64))
    return np.stack(Us), np.stack(Vs)


_U4, _V4 = _kmap_block(400)
_U8, _V8 = _kmap_block(800)


def _expand_block(WT, U, V):
    """WT: [D, M] K-major. Returns [nkc, 128, M] with zero rows for invalid."""
    return (WT[U] * V[:, :, None]).astype(np.float32)


def _prep_inputs(word_idx, pos_idx, word_emb, pos_emb,
                 Wih0, Whh0, bih0, bhh0, Wih1, Whh1, bih1, bhh1,
                 fc1_W, fc1_b, fc2_W, fc2_b):
    arr = {}
    arr["widx"] = np.ascontiguousarray(
        np.asarray(word_idx).reshape(N, 1).astype(np.int32))
    arr["pidx"] = np.ascontiguousarray(
        np.asarray(pos_idx).reshape(N, 1).astype(np.int32))
    arr["wemb"] = np.ascontiguousarray(np.asarray(word_emb, dtype=np.float32))
    arr["pemb"] = np.ascontiguousarray(np.asarray(pos_emb, dtype=np.float32))

    Wih = [np.asarray(Wih0, np.float64), np.asarray(Wih1, np.float64)]
    Whh = [np.asarray(Whh0, np.float64), np.asarray(Whh1, np.float64)]
    bih = [np.asarray(bih0, np.float64), np.asarray(bih1, np.float64)]
    bhh = [np.asarray(bhh0, np.float64), np.asarray(bhh1, np.float64)]

    # whhT [4, 128, 6400]: dl = 2*l + d; free = kc*1600 + n (n permuted)
    whhT = np.zeros((4, 128, 4 * G), np.float32)
    bias = np.zeros((1, 4 * G), np.float32)
    for l in range(2):
        for d in range(2):
            dl = 2 * l + d
            Wp = _scale_rows(Whh[l][d])[_PERM]          # [1600, 400]
            ch = _expand_block(Wp.T, _U4, _V4)          # [4, 128, 1600]
            whhT[dl] = ch.transpose(1, 0, 2).reshape(128, 4 * G)
            bias[0, G * dl: G * (dl + 1)] = \
                _scale_rows(bih[l][d] + bhh[l][d])[_PERM].astype(np.float32)
    arr["whhT"] = _bf(whhT)
    arr["bias"] = _bf(bias)

    # wih0T [2, 4, 128, 1600]: straight K-chunks of x's 400 dims
    wih0T = np.zeros((2, 4, 128, G), np.float32)
    for d in range(2):
        Wp = _scale_rows(Wih[0][d])[_PERM]              # [1600, 400]
        WT = np.zeros((512, G))
        WT[:400] = Wp.T
        for kc in range(4):
            wih0T[d, kc] = WT[128 * kc:128 * (kc + 1)].astype(np.float32)
    arr["wih0T"] = _bf(wih0T)

    # wih1T [2, 8, 128, 1600]: block K-chunks over h0cat's 800 dims
    wih1T = np.zeros((2, 8, 128, G), np.float32)
    for d in range(2):
        Wp = _scale_rows(Wih[1][d])[_PERM]              # [1600, 800]
        wih1T[d] = _expand_block(Wp.T, _U8, _V8)
    arr["wih1T"] = _bf(wih1T)

    # edge MLP weights
    f1 = np.asarray(fc1_W, np.float64)                  # [100, 1600]
    Uh = f1[:, :800].T                                  # [800, 100]
    Um = f1[:, 800:].T
    arr["uhT"] = _bf(
        _expand_block(Uh, _U8, _V8).transpose(1, 0, 2).reshape(128, 800))
    arr["umT"] = _bf(
        _expand_block(Um, _U8, _V8).transpose(1, 0, 2).reshape(128, 800))
    w2e = np.zeros((101, 1), np.float32)
    w2e[:100, 0] = np.asarray(fc2_W, np.float32)[0]
    w2e[100, 0] = 1.0
    arr["w2e"] = _bf(w2e)
    arr["b1"] = np.ascontiguousarray(
        np.asarray(fc1_b, np.float32).reshape(100, 1))
    arr["b2"] = np.ascontiguousarray(
        np.full((128, 1), np.float32(np.asarray(fc2_b).reshape(())),
                dtype=np.float32))
    # one-hot selector: oh32[p, j] = 1 iff p % 32 == j  (rank-1 row injection)
    oh = np.zeros((128, 32), np.float32)
    oh[_P, _P % 32] = 1.0
    arr["oh32"] = _bf(oh)
    return arr


def _make_selT(core):
    s = np.zeros((2, 128, 32), np.float32)
    for r in range(32):
        t = 32 * core + r
        s[t // 128, t % 128, r] = 1.0
    return _bf(s)


# ---------------------------------------------------------------------------
# device kernel build
# ---------------------------------------------------------------------------


def _emit_xg(nc, tc, ctx, l, wih_dram, bias_sb, ones_sb, lhs_tile, xg_dram,
             wih_pool, ps_pool, stage_pool):
    """Compute xg[t] = x @ Wih^T + b for both directions of layer l and store
    to xg_dram[dl]. lhs_tile: xT [128, 4*256] (l=0) or H0T [128, 8*256] (l=1).
    """
    nkc = 4 if l == 0 else 8
    kwidths = [128, 128, 128, 16] if l == 0 else [128] * 8
    for d in range(2):
        dl = 2 * l + d
        for m in range(2):
            pts = [ps_pool.tile([128, 400], F32, name=f"pxg{n}", tag=f"pxg{n}") for n in range(4)]
            for kc in range(nkc):
                wt = wih_pool.tile([128, G], BF16, name="wih", tag="wih")
                nc.sync.dma_start(out=wt[:, :], in_=wih_dram[d, kc])
                K = kwidths[kc]
                lhsT = lhs_tile[0:K, kc * 256 + 128 * m: kc * 256 + 128 * m + 128]
                for n in range(4):
                    nc.tensor.matmul(
                        pts[n][0:128, 0:400],
                        lhsT=lhsT,
                        rhs=wt[0:K, 400 * n: 400 * n + 400],
                        start=(kc == 0), stop=False)
            # bias row: xg += 1 x bias[dl]  (K=1 rank-1, bias on partition 0)
            for n in range(4):
                nc.tensor.matmul(
                    pts[n][0:128, 0:400],
                    lhsT=ones_sb[0:1, 0:128],
                    rhs=bias_sb[0:1, G * dl + 400 * n: G * dl + 400 * n + 400],
                    start=False, stop=True)
            st = stage_pool.tile([128, G], BF16, name="xgstage", tag="xgstage")
            for n in range(4):
                nc.vector.tensor_copy(
                    out=st[0:128, 400 * n: 400 * n + 400],
                    in_=pts[n][0:128, 0:400])
            nc.sync.dma_start(
                out=xg_dram[dl, 128 * m: 128 * m + 128, :], in_=st[:, :])


def _emit_recurrence(nc, tc, ctx, l, whh_sb, xgs_tiles, oh32_sb, HT_out,
                     state, pools):
    """Emit STEPS wall-steps for layer l (both directions interleaved)."""
    sg_pool, tmp_pool, ps_pool = pools
    for t in range(STEPS):
        for d in range(2):
            S = state[d]
            tdx = t if d == 0 else (STEPS - 1 - t)
            mblk, row = divmod(tdx, 96)
            htr, hsb, c = S["htr"], S["hsb"], S["c"]
            xgs = xgs_tiles[d][mblk]
            # one full PSUM bank per partition so the partition stride (2048B)
            # matches the simulator's per-bank zero-region bookkeeping
            ps = ps_pool.tile([128, 512], F32, name=f"ps{d}", tag=f"ps{d}")
            # --- gates = Whh @ h  (4 block-K rounds x 4 col groups; the
            # stationary h column is broadcast to M=32 so the matmul fills
            # all 32 partitions of each column group) ---
            for kc in range(4):
                for g in range(4):
                    nc.tensor.matmul(
                        ps[32 * g: 32 * g + 32, 0:400],
                        lhsT=htr[0:128, 32 * kc: 32 * kc + 1].to_broadcast([128, 32]),
                        rhs=whh_sb[2 * l + d][0:128,
                                                kc * G + 400 * g: kc * G + 400 * g + 400],
                        start=(kc == 0), stop=False,
                        skip_group_check=True,
                        tile_position=(0, 32 * g))
            # --- gates += xg[tdx]  (K=32 one-hot row selection) ---
            bb, rr = divmod(row, 32)
            for g in range(4):
                nc.tensor.matmul(
                    ps[32 * g: 32 * g + 32, 0:400],
                    lhsT=oh32_sb[32 * bb: 32 * bb + 32, rr:rr + 1].to_broadcast([32, 32]),
                    rhs=xgs[32 * bb: 32 * bb + 32, 400 * g: 400 * g + 400],
                    start=False, stop=True,
                    skip_group_check=True,
                    tile_position=(32 * bb, 32 * g))
            # --- sigmoid over all gates (g rows pre-scaled by 2) ---
            sg = sg_pool.tile([128, 400], F32, name=f"sg{d}", tag=f"sg{d}")
            nc.scalar.activation(sg[0:128, 0:400], ps[0:128, 0:400], AF.Sigmoid)
            # --- c = sig(f)*c + sig(i)*(2*sig(2g) - 1) ---
            tg = tmp_pool.tile([128, 100], F32, name=f"tg{d}", tag=f"tg{d}")
            t1 = tmp_pool.tile([128, 100], F32, name=f"t1{d}", tag=f"t1{d}")
            nc.gpsimd.tensor_scalar(
                out=tg[0:128, 0:100], in0=sg[0:128, 200:300],
                scalar1=2.0, scalar2=-1.0, op0=OP.mult, op1=OP.add)
            nc.gpsimd.tensor_tensor(
                out=t1[0:128, 0:100], in0=sg[0:128, 0:100],
                in1=tg[0:128, 0:100], op=OP.mult)
            nc.vector.tensor_tensor(
                out=c[0:128, 0:100], in0=sg[0:128, 100:200],
                in1=c[0:128, 0:100], op=OP.mult)
            nc.vector.tensor_tensor(
                out=c[0:128, 0:100], in0=c[0:128, 0:100],
                in1=t1[0:128, 0:100], op=OP.add)
            # --- h = sig(o) * tanh(c)  (Tanh shares the sigmoid table set) ---
            th = tmp_pool.tile([128, 100], F32, name=f"th{d}", tag=f"th{d}")
            nc.scalar.activation(th[0:128, 0:100], c[0:128, 0:100], AF.Tanh)
            nc.vector.tensor_tensor(
                out=hsb[0:128, 0:100], in0=sg[0:128, 300:400],
                in1=th[0:128, 0:100], op=OP.mult)
            # --- relayout h for next step's lhsT (32x32 block transpose) ---
            nc.vector.transpose(out=htr[0:128, 0:128], in_=hsb[0:128, 0:128])
            # --- store h into HT (block-chunk cols b at (4d+b)*256 + tdx) ---
            nc.gpsimd.tensor_copy(
                out=HT_out[0:128, 4 * d * 256 + tdx: (4 * d + 4) * 256: 256],
                in_=htr[0:128, 0:128:32])


def build_nc():
    nc = bacc.Bacc("TRN2", target_bir_lowering=False, debug=False,
                   num_devices=NC)
    # ---- DRAM parameters ----
    wemb = nc.dram_tensor("wemb", [50000, 300], F32, kind="ExternalInput").ap()
    pemb = nc.dram_tensor("pemb", [50, 100], F32, kind="ExternalInput").ap()
    widx = nc.dram_tensor("widx", [N, 1], I32, kind="ExternalInput").ap()
    pidx = nc.dram_tensor("pidx", [N, 1], I32, kind="ExternalInput").ap()
    wih0T = nc.dram_tensor("wih0T", [2, 4, 128, G], BF16, kind="ExternalInput").ap()
    whhT = nc.dram_tensor("whhT", [4, 128, 4 * G], BF16, kind="ExternalInput").ap()
    wih1T = nc.dram_tensor("wih1T", [2, 8, 128, G], BF16, kind="ExternalInput").ap()
    biasd = nc.dram_tensor("bias", [1, 4 * G], BF16, kind="ExternalInput").ap()
    oh32d = nc.dram_tensor("oh32", [128, 32], BF16, kind="ExternalInput").ap()
    uhTd = nc.dram_tensor("uhT", [128, 800], BF16, kind="ExternalInput").ap()
    umTd = nc.dram_tensor("umT", [128, 800], BF16, kind="ExternalInput").ap()
    w2ed = nc.dram_tensor("w2e", [101, 1], BF16, kind="ExternalInput").ap()
    b1d = nc.dram_tensor("b1", [100, 1], F32, kind="ExternalInput").ap()
    b2d = nc.dram_tensor("b2", [128, 1], F32, kind="ExternalInput").ap()
    selTd = nc.dram_tensor("selT", [2, 128, 32], BF16, kind="ExternalInput").ap()
    xg_dram = nc.dram_tensor("xg", [4, N, G], BF16).ap()
    grid = nc.dram_tensor("grid", [32, N], F32, kind="ExternalOutput").ap()

    from contextlib import ExitStack
    with TileContext(nc) as tc, ExitStack() as ctx:
        top = ctx.enter_context(tc.tile_pool(name="top", bufs=1))
        # ---- persistent tiles ----
        whh_sb = [top.tile([128, 4 * G], BF16, name=f"whh{dl}", tag=f"whh{dl}") for dl in range(4)]
        for dl in range(4):
            nc.sync.dma_start(out=whh_sb[dl][:, :], in_=whhT[dl])
        bias_sb = top.tile([1, 4 * G], BF16, name="bias", tag="bias")
        oh32_sb = top.tile([128, 32], BF16, name="oh32", tag="oh32")
        nc.sync.dma_start(out=oh32_sb[:, :], in_=oh32d[:, :])
        nc.sync.dma_start(out=bias_sb[:, :], in_=biasd[:, :])
        ones_sb = top.tile([1, 128], BF16, name="ones", tag="ones")
        nc.gpsimd.memset(ones_sb[:, :], 1.0)
        idn = top.tile([128, 128], F32, name="idn", tag="idn")
        make_identity(nc, idn[:, :])
        H0T = top.tile([128, 8 * 256], BF16, name="H0T", tag="H0T")
        H1T = top.tile([128, 8 * 256], BF16, name="H1T", tag="H1T")
        if STEPS < N:
            nc.gpsimd.memset(H0T[:, :], 0.0)
            nc.gpsimd.memset(H1T[:, :], 0.0)

        # =========== embedding gather + transpose ===========
        with tc.tile_pool(name="embed", bufs=1) as epool, \
             tc.tile_pool(name="embps", bufs=2, space="PSUM") as eps:
            idx_sb = epool.tile([128, 4], I32, name="idx", tag="idx")
            nc.sync.dma_start(out=idx_sb[0:128, 0:1], in_=widx[0:128, 0:1])
            nc.sync.dma_start(out=idx_sb[0:128, 1:2], in_=widx[128:256, 0:1])
            nc.sync.dma_start(out=idx_sb[0:128, 2:3], in_=pidx[0:128, 0:1])
            nc.sync.dma_start(out=idx_sb[0:128, 3:4], in_=pidx[128:256, 0:1])
            x_sb = epool.tile([128, 800], F32, name="xsb", tag="xsb")
            for cch in range(2):
                nc.gpsimd.indirect_dma_start(
                    out=x_sb[0:128, 400 * cch: 400 * cch + 300],
                    out_offset=None,
                    in_=wemb[:, :],
                    in_offset=IndirectOffsetOnAxis(
                        ap=idx_sb[0:128, cch:cch + 1], axis=0))
                nc.gpsimd.indirect_dma_start(
                    out=x_sb[0:128, 400 * cch + 300: 400 * cch + 400],
                    out_offset=None,
                    in_=pemb[:, :],
                    in_offset=IndirectOffsetOnAxis(
                        ap=idx_sb[0:128, 2 + cch:3 + cch], axis=0))
            xT = epool.tile([128, 4 * 256], BF16, name="xT", tag="xT")
            nc.gpsimd.memset(xT[:, :], 0.0)
            for cch in range(2):
                for kc in range(4):
                    w = 128 if kc < 3 else 16
                    ptr = eps.tile([128, 128], F32, name="ptr", tag="ptr")
                    nc.tensor.transpose(
                        out=ptr[0:w, 0:128],
                        in_=x_sb[0:128, 400 * cch + 128 * kc: 400 * cch + 128 * kc + w],
                        identity=idn[:, :])
                    nc.vector.tensor_copy(
                        out=xT[0:w, kc * 256 + 128 * cch: kc * 256 + 128 * cch + 128],
                        in_=ptr[0:w, 0:128])

            # =========== xg for layer 0 ===========
            with tc.tile_pool(name="wih", bufs=3) as wih_pool, \
                 tc.tile_pool(name="xgps", bufs=1, space="PSUM") as xg_ps, \
                 tc.tile_pool(name="xgstage", bufs=2) as stage_pool:
                _emit_xg(nc, tc, ctx, 0, wih0T, bias_sb, ones_sb, xT, xg_dram,
                         wih_pool, xg_ps, stage_pool)

        # =========== recurrence helper state ===========
        def make_state(rpool, rps):
            state = []
            for d in range(2):
                htr = rpool.tile([128, 128], BF16, name=f"htr{d}", tag=f"htr{d}")
                nc.gpsimd.memset(htr[:, :], 0.0)
                hsb = rpool.tile([128, 128], BF16, name=f"hsb{d}", tag=f"hsb{d}")
                nc.gpsimd.memset(hsb[:, :], 0.0)
                c = rpool.tile([128, 100], F32, name=f"c{d}", tag=f"c{d}")
                nc.gpsimd.memset(c[:, :], 0.0)
                state.append(dict(htr=htr, hsb=hsb, c=c))
            return state

        nmb = (STEPS + 95) // 96

        # =========== layer 0 recurrence ===========
        with tc.tile_pool(name="rec0", bufs=1) as rpool, \
             tc.tile_pool(name="rec0ps", bufs=2, space="PSUM") as rps, \
             tc.tile_pool(name="sg0", bufs=2) as sg_pool, \
             tc.tile_pool(name="tmp0", bufs=2) as tmp_pool:
            xgs_tiles = []
            for d in range(2):
                tiles = []
                for m in range(nmb):
                    nr = min(96, STEPS - 96 * m)
                    xt = rpool.tile([96, G], BF16, name=f"xgs{d}{m}", tag=f"xgs{d}{m}")
                    if nr < 96:
                        nc.gpsimd.memset(xt[:, :], 0.0)
                    nc.sync.dma_start(
                        out=xt[0:nr, :],
                        in_=xg_dram[2 * 0 + d, 96 * m: 96 * m + nr, :])
                    tiles.append(xt)
                xgs_tiles.append(tiles)
            st0 = make_state(rpool, rps)
            _emit_recurrence(nc, tc, ctx, 0, whh_sb, xgs_tiles, oh32_sb, H0T,
                             st0, (sg_pool, tmp_pool, rps))

        # =========== xg for layer 1 (from H0T) ===========
        with tc.tile_pool(name="wih1", bufs=3) as wih_pool, \
             tc.tile_pool(name="xg1ps", bufs=1, space="PSUM") as xg_ps, \
             tc.tile_pool(name="xg1stage", bufs=2) as stage_pool:
            _emit_xg(nc, tc, ctx, 1, wih1T, bias_sb, ones_sb, H0T, xg_dram,
                     wih_pool, xg_ps, stage_pool)

        # =========== layer 1 recurrence ===========
        with tc.tile_pool(name="rec1", bufs=1) as rpool, \
             tc.tile_pool(name="rec1ps", bufs=2, space="PSUM") as rps, \
             tc.tile_pool(name="sg1", bufs=2) as sg_pool, \
             tc.tile_pool(name="tmp1", bufs=2) as tmp_pool:
            xgs_tiles = []
            for d in range(2):
                tiles = []
                for m in range(nmb):
                    nr = min(96, STEPS - 96 * m)
                    xt = rpool.tile([96, G], BF16, name=f"xgs{d}{m}", tag=f"xgs{d}{m}")
                    if nr < 96:
                        nc.gpsimd.memset(xt[:, :], 0.0)
                    nc.sync.dma_start(
                        out=xt[0:nr, :],
                        in_=xg_dram[2 * 1 + d, 96 * m: 96 * m + nr, :])
                    tiles.append(xt)
                xgs_tiles.append(tiles)
            st1 = make_state(rpool, rps)
            _emit_recurrence(nc, tc, ctx, 1, whh_sb, xgs_tiles, oh32_sb, H1T,
                             st1, (sg_pool, tmp_pool, rps))

        # =========== edge scorer ===========
        with tc.tile_pool(name="edge", bufs=1) as ep, \
             tc.tile_pool(name="edgeth", bufs=3) as thp, \
             tc.tile_pool(name="edgeps", bufs=1, space="PSUM") as epps, \
             tc.tile_pool(name="edgepsS", bufs=1, space="PSUM") as spps:
            uhT_sb = ep.tile([128, 800], BF16, name="uhT", tag="uhT")
            nc.sync.dma_start(out=uhT_sb[:, :], in_=uhTd[:, :])
            umT_sb = ep.tile([128, 800], BF16, name="umT", tag="umT")
            nc.sync.dma_start(out=umT_sb[:, :], in_=umTd[:, :])
            w2e_sb = ep.tile([101, 1], BF16, name="w2e", tag="w2e")
            nc.sync.dma_start(out=w2e_sb[:, :], in_=w2ed[:, :])
            b1_sb = ep.tile([100, 1], F32, name="b1", tag="b1")
            nc.sync.dma_start(out=b1_sb[:, :], in_=b1d[:, :])
            b2_sb = ep.tile([128, 1], F32, name="b2", tag="b2")
            nc.sync.dma_start(out=b2_sb[:, :], in_=b2d[:, :])
            selT_sb = ep.tile([128, 64], BF16, name="selT", tag="selT")
            nc.sync.dma_start(out=selT_sb[0:128, 0:32], in_=selTd[0])
            nc.sync.dma_start(out=selT_sb[0:128, 32:64], in_=selTd[1])

            # A in t-major layout: [128, 2*100]
            A_sb = ep.tile([128, 200], BF16, name="A", tag="A")
            for m in range(2):
                pA = epps.tile([128, 100], F32, name="pA", tag="pA")
                for kc in range(8):
                    nc.tensor.matmul(
                        pA[0:128, 0:100],
                        lhsT=H1T[0:128, kc * 256 + 128 * m: kc * 256 + 128 * m + 128],
                        rhs=uhT_sb[0:128, kc * 100: kc * 100 + 100],
                        start=(kc == 0), stop=(kc == 7))
                nc.vector.tensor_copy(out=A_sb[0:128, 100 * m: 100 * m + 100],
                                      in_=pA[0:128, 0:100])
            # B^T [100, 256] with b1 folded in
            B_sb = ep.tile([128, 256], F32, name="B", tag="B")
            pB = epps.tile([128, 256], F32, name="pB", tag="pB")
            for kc in range(8):
                nc.tensor.matmul(
                    pB[0:100, 0:256],
                    lhsT=umT_sb[0:128, kc * 100: kc * 100 + 100],
                    rhs=H1T[0:128, kc * 256: kc * 256 + 256],
                    start=(kc == 0), stop=(kc == 7))
            nc.vector.tensor_scalar(
                out=B_sb[0:100, 0:256], in0=pB[0:100, 0:256],
                scalar1=b1_sb[0:100, 0:1], scalar2=None, op0=OP.add)
            # Asel = selT^T @ A  -> [32, 100], then transpose -> [100, 32]
            AselS = ep.tile([128, 128], F32, name="AselS", tag="AselS")
            nc.gpsimd.memset(AselS[:, :], 0.0)
            pS = epps.tile([128, 100], F32, name="pS", tag="pS")
            for m in range(2):
                nc.tensor.matmul(
                    pS[0:32, 0:100],
                    lhsT=selT_sb[0:128, 32 * m: 32 * m + 32],
                    rhs=A_sb[0:128, 100 * m: 100 * m + 100],
                    start=(m == 0), stop=(m == 1))
            nc.vector.tensor_copy(out=AselS[0:32, 0:100], in_=pS[0:32, 0:100])
            pAT = epps.tile([128, 128], F32, name="pAT", tag="pAT")
            nc.tensor.transpose(out=pAT[0:128, 0:128], in_=AselS[0:128, 0:128],
                                identity=idn[:, :])
            AT_sb = ep.tile([128, 32], F32, name="AT", tag="AT")
            nc.vector.tensor_copy(out=AT_sb[0:128, 0:32], in_=pAT[0:128, 0:32])

            # per-row tanh + w2 dot
            psS_tiles = [spps.tile([128, 512], F32, name=f"psS{q}", tag=f"psS{q}")
                         for q in range(4)]
            for q in range(4):
                nc.vector.memset(psS_tiles[q][:, :], 0.0)
            gsb_tiles = [ep.tile([128, 512], F32, name=f"gsb{q}", tag=f"gsb{q}")
                         for q in range(4)]
            for r in range(32):
                th_t = thp.tile([128, 256], BF16, name="th", tag="th")
                nc.scalar.activation(
                    th_t[0:100, 0:256], B_sb[0:100, 0:256], AF.Tanh,
                    bias=AT_sb[0:100, r:r + 1], scale=1.0)
                q, half = divmod(r // 4, 2)
                nc.tensor.matmul(
                    psS_tiles[q][32 * (r % 4): 32 * (r % 4) + 1,
                                 256 * half: 256 * half + 256],
                    lhsT=w2e_sb[0:100, 0:1],
                    rhs=th_t[0:100, 0:256],
                    start=True, stop=True,
                    skip_group_check=True,
                    tile_position=(0, 32 * (r % 4)))
            for q in range(4):
                nc.vector.tensor_scalar(
                    out=gsb_tiles[q][0:128, 0:512],
                    in0=psS_tiles[q][0:128, 0:512],
                    scalar1=b2_sb[0:128, 0:1], scalar2=None, op0=OP.add)
                for half in range(2):
                    rb = 4 * (2 * q + half)
                    nc.sync.dma_start(
                        out=grid[rb:rb + 4, 0:256],
                        in_=gsb_tiles[q][0:128:32, 256 * half: 256 * half + 256])

    nc.compile()
    return nc


_NC_CACHE = None


def _get_nc():
    global _NC_CACHE
    if _NC_CACHE is None:
        _NC_CACHE = build_nc()
    return _NC_CACHE


def kernel(**inputs) -> np.ndarray:
    from concourse.bass_utils import run_bass_kernel_spmd

    arr = _prep_inputs(**inputs)
    nc = _get_nc()
    in_maps = []
    for k in range(NC):
        m = dict(arr)
        m["selT"] = _make_selT(k)
        in_maps.append(m)
    res = run_bass_kernel_spmd(nc, in_maps, core_ids=list(range(NC)))
    grid = np.concatenate([res.results[k]["grid"] for k in range(NC)], axis=0)
    mask = np.ones((N, N), dtype=bool)
    np.fill_diagonal(mask, False)
    mask[:, 0] = False
    return grid[mask].reshape(-1, 1).astype(np.float32)



# revision 4
# speedup vs baseline: 2.8657x; 2.8657x over previous
"""Trainium2 Bass kernel: BiLSTM dependency-parser edge scorer (v2).

Self-contained. Accepts FULL inputs (as produced by setup_inputs()), returns
the FULL [65280, 1] float32 score tensor.

Key idea vs v1: all recurrence matmuls are WEIGHTS-STATIONARY (weights in
lhsT, the tiny h vector streams as rhs), so each step's 64 gate matmuls have
output free-size 1 instead of streaming 6400 PSUM rows.

Layouts (per direction d, layer l):
  gates PSUM tile [100, 16]: partition p, col n = 4*j + g where the LSTM
    unit is u = 100*j + p (j in 0..4) and g in {0:i, 1:f, 2:g, 3:o}.
  h storage H[l][d] [100, 4*256] bf16: h_t for unit (j, p) at col 4*t + j.
    Column 4*t+j is directly the rhs [100, 1] for K-chunk j of the next
    step's matmul -- no transpose inside the loop.
  c state [100, 4] f32.
  xg_sb[d] [100, 16*256] bf16: precomputed input projections + bias,
    injected into the PSUM accumulation via an identity-weight matmul.
g-gate rows are pre-scaled by 2 on host: tanh(x) = 2*sigmoid(2x) - 1.
"""

import os
import sys

sys.path.insert(0, "/opt/trn_rl_repo")

import numpy as np

import concourse.bass as bass
import concourse.mybir as mybir
from concourse import bacc
from concourse.bass import IndirectOffsetOnAxis
from concourse.masks import make_identity
from concourse.tile import TileContext

N = 256          # sequence length
HID = 400        # hidden per direction
NC = 8           # cores
P = 100          # partitions used for unit math
NG = 16          # gate cols per step
F32 = mybir.dt.float32
BF16 = mybir.dt.float16
I32 = mybir.dt.int32
AF = mybir.ActivationFunctionType
OP = mybir.AluOpType

STEPS = int(os.environ.get("DP_STEPS", str(N)))


# ---------------------------------------------------------------------------
# host-side weight layout prep
# ---------------------------------------------------------------------------

def _bf(a):
    return np.ascontiguousarray(np.asarray(a).astype(np.float16))


# R[p, n] = original torch gate-row for (partition p, col n)
_PP, _NN = np.meshgrid(np.arange(P), np.arange(NG), indexing="ij")
_R = 400 * (_NN % 4) + 100 * (_NN // 4) + _PP      # [100, 16]


def _scale_g(W):
    """Scale g-gate rows (orig rows 800:1200) by 2."""
    Ws = np.array(W, dtype=np.float64)
    Ws[800:1200] *= 2.0
    return Ws


def _wblocks(W, nuc):
    """W: [1600, U] scaled gate-major weights, U = 100*nuc.
    Returns [100, 16*nuc*100]: block (n, uc) at cols (n*nuc+uc)*100 holds
    lhsT[k, m] = W[R[m, n], 100*uc + k]."""
    arr = W[_R]                                    # [100p, 16n, U]
    A4 = arr.reshape(P, NG, nuc, 100)              # [p, n, uc, k]
    return A4.transpose(3, 1, 2, 0).reshape(100, NG * nuc * 100)


def _prep_inputs(word_idx, pos_idx, word_emb, pos_emb,
                 Wih0, Whh0, bih0, bhh0, Wih1, Whh1, bih1, bhh1,
                 fc1_W, fc1_b, fc2_W, fc2_b):
    arr = {}
    arr["widx"] = np.ascontiguousarray(
        np.asarray(word_idx).reshape(N, 1).astype(np.int32))
    arr["pidx"] = np.ascontiguousarray(
        np.asarray(pos_idx).reshape(N, 1).astype(np.int32))
    arr["wemb"] = np.ascontiguousarray(np.asarray(word_emb, dtype=np.float32))
    arr["pemb"] = np.ascontiguousarray(np.asarray(pos_emb, dtype=np.float32))

    Wih = [np.asarray(Wih0, np.float64), np.asarray(Wih1, np.float64)]
    Whh = [np.asarray(Whh0, np.float64), np.asarray(Whh1, np.float64)]
    bih = [np.asarray(bih0, np.float64), np.asarray(bih1, np.float64)]
    bhh = [np.asarray(bhh0, np.float64), np.asarray(bhh1, np.float64)]

    whhT = np.zeros((4, 100, NG * 4 * 100), np.float32)
    biasT = np.zeros((4, 100, 512), np.float32)
    wih0T = np.zeros((2, 100, NG * 4 * 100), np.float32)
    wih1T = np.zeros((2, 100, NG * 8 * 100), np.float32)
    for l in range(2):
        for d in range(2):
            dl = 2 * l + d
            whhT[dl] = _wblocks(_scale_g(Whh[l][d]), 4)
            b = _scale_g(bih[l][d] + bhh[l][d])[_R]          # [100, 16]
            biasT[dl] = np.tile(b, (1, 32)).astype(np.float32)
    for d in range(2):
        wih0T[d] = _wblocks(_scale_g(Wih[0][d]), 4)
        wih1T[d] = _wblocks(_scale_g(Wih[1][d]), 8)
    arr["whhT"] = _bf(whhT)
    arr["biasT"] = np.ascontiguousarray(biasT)
    arr["wih0T"] = _bf(wih0T)
    arr["wih1T"] = _bf(wih1T)

    # identity for the xg injection matmul
    arr["id100"] = _bf(np.eye(P, dtype=np.float32))

    # edge MLP: uhT/umT [100, 800]: block uc at cols 100*uc holds
    # lhsT[k, a] = fc1_W[a, 100*uc + k]
    f1 = np.asarray(fc1_W, np.float64)               # [100, 1600]
    arr["uhT"] = _bf(np.concatenate(
        [f1[:, 100 * u:100 * u + 100].T for u in range(8)], axis=1))
    arr["umT"] = _bf(np.concatenate(
        [f1[:, 800 + 100 * u:800 + 100 * u + 100].T for u in range(8)],
        axis=1))
    arr["b1row"] = np.ascontiguousarray(
        np.asarray(fc1_b, np.float32).reshape(1, 100))
    arr["w2"] = _bf(np.asarray(fc2_W, np.float32).reshape(100, 1))
    arr["b2"] = np.ascontiguousarray(
        np.full((128, 1), np.float32(np.asarray(fc2_b).reshape(())),
                dtype=np.float32))
    return arr


def _make_selT(core):
    s = np.zeros((2, 128, 32), np.float32)
    for r in range(32):
        t = 32 * core + r
        s[t // 128, t % 128, r] = 1.0
    return np.ascontiguousarray(s)


# ---------------------------------------------------------------------------
# device kernel build
# ---------------------------------------------------------------------------

def _emit_xg(nc, l, wih_sb, rhs_chunk, xg_sbs, bias_sbs, ps_pool):
    """xg[d][p, 16*t + n] = sum_u Wih[r(p,n), u] * in[t, u]  (+ bias)."""
    nuc = 4 if l == 0 else 8
    for d in range(2):
        for tc in range(8):
            ps = ps_pool.tile([128, 512], F32, name="xgps", tag="xgps")
            for n in range(NG):
                for uc in range(nuc):
                    nc.tensor.matmul(
                        ps[0:P, n:512:16],
                        lhsT=wih_sb[d][0:P, (n * nuc + uc) * 100:
                                       (n * nuc + uc) * 100 + 100],
                        rhs=rhs_chunk(d, uc, tc),
                        start=(uc == 0), stop=(uc == nuc - 1),
                        skip_group_check=True)
            nc.vector.tensor_tensor(
                out=xg_sbs[d][0:P, 512 * tc: 512 * tc + 512],
                in0=ps[0:P, 0:512],
                in1=bias_sbs[d][0:P, 0:512], op=OP.add)


def _emit_rec(nc, l, whh_sb, xg_sbs, id_sb, H_out, pools):
    """STEPS wall-steps, both directions interleaved."""
    state_pool, sg_pool, tmp_pool, ps_pool = pools
    cs = []
    for d in range(2):
        c = state_pool.tile([P, 4], F32, name=f"c{d}", tag=f"c{d}")
        nc.gpsimd.memset(c[:, :], 0.0)
        cs.append(c)

    for t in range(STEPS):
        ps_t, sg_t, tg_t, t1_t, th_t = [], [], [], [], []
        # --- PE: injection + 64 weight matmuls per direction ---
        for d in range(2):
            tdx = t if d == 0 else (STEPS - 1 - t)
            ps = ps_pool.tile([128, 512], F32, name=f"ps{d}", tag=f"ps{d}")
            ps_t.append(ps)
            first = (t == 0)
            nc.tensor.matmul(
                ps[0:P, 0:NG],
                lhsT=id_sb[0:P, 0:P],
                rhs=xg_sbs[d][0:P, NG * tdx: NG * tdx + NG],
                start=True, stop=first, skip_group_check=True)
            if not first:
                pdx = tdx - 1 if d == 0 else tdx + 1
                for n in range(NG):
                    for uc in range(4):
                        nc.tensor.matmul(
                            ps[0:P, n:n + 1],
                            lhsT=whh_sb[d][0:P, (4 * n + uc) * 100:
                                           (4 * n + uc) * 100 + 100],
                            rhs=H_out[d][0:P, 4 * pdx + uc: 4 * pdx + uc + 1],
                            start=False, stop=(uc == 3),
                            skip_group_check=True)
        # --- Act: sigmoid over all gates (g pre-scaled by 2) ---
        for d in range(2):
            sg = sg_pool.tile([P, NG], F32, name=f"sg{d}", tag=f"sg{d}")
            sg_t.append(sg)
            nc.scalar.activation(sg[0:P, 0:NG], ps_t[d][0:P, 0:NG], AF.Sigmoid)
        # --- c update: c = sig(f)*c + sig(i)*(2*sig(2g) - 1) ---
        for d in range(2):
            sg, c = sg_t[d], cs[d]
            tg = tmp_pool.tile([P, 4], F32, name=f"tg{d}", tag=f"tg{d}")
            t1 = tmp_pool.tile([P, 4], F32, name=f"t1{d}", tag=f"t1{d}")
            cf = tmp_pool.tile([P, 4], F32, name=f"cf{d}", tag=f"cf{d}")
            tg_t.append(tg)
            t1_t.append(t1)
            nc.vector.tensor_scalar(
                out=tg[0:P, 0:4], in0=sg[0:P, 2:NG:4],
                scalar1=2.0, scalar2=-1.0, op0=OP.mult, op1=OP.add)
            nc.gpsimd.tensor_tensor(
                out=cf[0:P, 0:4], in0=sg[0:P, 1:NG:4],
                in1=c[0:P, 0:4], op=OP.mult)
            nc.vector.tensor_tensor(
                out=t1[0:P, 0:4], in0=sg[0:P, 0:NG:4],
                in1=tg[0:P, 0:4], op=OP.mult)
            nc.vector.tensor_tensor(
                out=c[0:P, 0:4], in0=cf[0:P, 0:4],
                in1=t1[0:P, 0:4], op=OP.add)
        # --- Act: tanh(c);  DVE: h = sig(o)*tanh(c) into H ---
        for d in range(2):
            th = tmp_pool.tile([P, 4], F32, name=f"th{d}", tag=f"th{d}")
            th_t.append(th)
            nc.scalar.activation(th[0:P, 0:4], cs[d][0:P, 0:4], AF.Tanh)
        for d in range(2):
            tdx = t if d == 0 else (STEPS - 1 - t)
            nc.vector.tensor_tensor(
                out=H_out[d][0:P, 4 * tdx: 4 * tdx + 4],
                in0=sg_t[d][0:P, 3:NG:4], in1=th_t[d][0:P, 0:4], op=OP.mult)


def build_nc():
    nc = bacc.Bacc("TRN2", target_bir_lowering=False, debug=False,
                   num_devices=NC)
    wemb = nc.dram_tensor("wemb", [50000, 300], F32, kind="ExternalInput").ap()
    pemb = nc.dram_tensor("pemb", [50, 100], F32, kind="ExternalInput").ap()
    widx = nc.dram_tensor("widx", [N, 1], I32, kind="ExternalInput").ap()
    pidx = nc.dram_tensor("pidx", [N, 1], I32, kind="ExternalInput").ap()
    whhTd = nc.dram_tensor("whhT", [4, 100, 6400], BF16, kind="ExternalInput").ap()
    wih0Td = nc.dram_tensor("wih0T", [2, 100, 6400], BF16, kind="ExternalInput").ap()
    wih1Td = nc.dram_tensor("wih1T", [2, 100, 12800], BF16, kind="ExternalInput").ap()
    biasTd = nc.dram_tensor("biasT", [4, 100, 512], F32, kind="ExternalInput").ap()
    id100d = nc.dram_tensor("id100", [100, 100], BF16, kind="ExternalInput").ap()
    uhTd = nc.dram_tensor("uhT", [100, 800], BF16, kind="ExternalInput").ap()
    umTd = nc.dram_tensor("umT", [100, 800], BF16, kind="ExternalInput").ap()
    b1rowd = nc.dram_tensor("b1row", [1, 100], F32, kind="ExternalInput").ap()
    w2d = nc.dram_tensor("w2", [100, 1], BF16, kind="ExternalInput").ap()
    b2d = nc.dram_tensor("b2", [128, 1], F32, kind="ExternalInput").ap()
    selTd = nc.dram_tensor("selT", [2, 128, 32], F32, kind="ExternalInput").ap()
    grid = nc.dram_tensor("grid", [32, N], F32, kind="ExternalOutput").ap()

    from contextlib import ExitStack
    with TileContext(nc) as tc, ExitStack() as ctx:
        top = ctx.enter_context(tc.tile_pool(name="top", bufs=1))
        # ---- persistent SBUF tiles ----
        wih0_sb = [top.tile([100, 6400], BF16, name=f"wih0{d}", tag=f"wih0{d}")
                   for d in range(2)]
        for d in range(2):
            nc.sync.dma_start(out=wih0_sb[d][:, :], in_=wih0Td[d])
        whh_sb = [top.tile([100, 6400], BF16, name=f"whh{dl}", tag=f"whh{dl}")
                  for dl in range(4)]
        for dl in range(4):
            nc.sync.dma_start(out=whh_sb[dl][:, :], in_=whhTd[dl])
        wih1_sb = [top.tile([100, 12800], BF16, name=f"wih1{d}", tag=f"wih1{d}")
                   for d in range(2)]
        for d in range(2):
            nc.sync.dma_start(out=wih1_sb[d][:, :], in_=wih1Td[d])
        bias_sb = [top.tile([100, 512], F32, name=f"bias{dl}", tag=f"bias{dl}")
                   for dl in range(4)]
        for dl in range(4):
            nc.sync.dma_start(out=bias_sb[dl][:, :], in_=biasTd[dl])
        id_sb = top.tile([100, 100], BF16, name="id100", tag="id100")
        nc.sync.dma_start(out=id_sb[:, :], in_=id100d[:, :])
        idn = top.tile([128, 128], F32, name="idn", tag="idn")
        make_identity(nc, idn[:, :])
        ones_sb = top.tile([1, 256], F32, name="ones", tag="ones")
        nc.gpsimd.memset(ones_sb[:, :], 1.0)
        xg_sbs = [top.tile([100, 4096], BF16, name=f"xg{d}", tag=f"xg{d}")
                  for d in range(2)]
        H = [[top.tile([100, 4 * N], BF16, name=f"H{l}{d}", tag=f"H{l}{d}")
              for d in range(2)] for l in range(2)]
        if STEPS < N:
            for l in range(2):
                for d in range(2):
                    nc.gpsimd.memset(H[l][d][:, :], 0.0)

        # =========== embedding gather + transpose to xT ===========
        xT = top.tile([100, 4 * N], BF16, name="xT", tag="xT")
        with tc.tile_pool(name="embed", bufs=1) as epool, \
             tc.tile_pool(name="embps", bufs=2, space="PSUM") as eps:
            idx_sb = epool.tile([128, 4], I32, name="idx", tag="idx")
            nc.sync.dma_start(out=idx_sb[0:128, 0:1], in_=widx[0:128, 0:1])
            nc.sync.dma_start(out=idx_sb[0:128, 1:2], in_=widx[128:256, 0:1])
            nc.sync.dma_start(out=idx_sb[0:128, 2:3], in_=pidx[0:128, 0:1])
            nc.sync.dma_start(out=idx_sb[0:128, 3:4], in_=pidx[128:256, 0:1])
            x_sb = epool.tile([128, 800], F32, name="xsb", tag="xsb")
            for cch in range(2):
                nc.gpsimd.indirect_dma_start(
                    out=x_sb[0:128, 400 * cch: 400 * cch + 300],
                    out_offset=None,
                    in_=wemb[:, :],
                    in_offset=IndirectOffsetOnAxis(
                        ap=idx_sb[0:128, cch:cch + 1], axis=0))
                nc.gpsimd.indirect_dma_start(
                    out=x_sb[0:128, 400 * cch + 300: 400 * cch + 400],
                    out_offset=None,
                    in_=pemb[:, :],
                    in_offset=IndirectOffsetOnAxis(
                        ap=idx_sb[0:128, 2 + cch:3 + cch], axis=0))
            for cch in range(2):
                for uc in range(4):
                    ptr = eps.tile([128, 128], F32, name="ptr", tag="ptr")
                    nc.tensor.transpose(
                        out=ptr[0:100, 0:128],
                        in_=x_sb[0:128, 400 * cch + 100 * uc:
                                 400 * cch + 100 * uc + 100],
                        identity=idn[:, :])
                    nc.vector.tensor_copy(
                        out=xT[0:100, 256 * uc + 128 * cch:
                               256 * uc + 128 * cch + 128],
                        in_=ptr[0:100, 0:128])

        # =========== layer 0: xg + recurrence ===========
        def rhs_l0(d, uc, tc):
            return xT[0:P, 256 * uc + 32 * tc: 256 * uc + 32 * tc + 32]

        with tc.tile_pool(name="xg0ps", bufs=2, space="PSUM") as xg_ps:
            _emit_xg(nc, 0, wih0_sb, rhs_l0, xg_sbs, bias_sb[0:2], xg_ps)

        with tc.tile_pool(name="rst0", bufs=1) as state_pool, \
             tc.tile_pool(name="sg0", bufs=2) as sg_pool, \
             tc.tile_pool(name="tmp0", bufs=2) as tmp_pool, \
             tc.tile_pool(name="rec0ps", bufs=2, space="PSUM") as rec_ps:
            _emit_rec(nc, 0, whh_sb[0:2], xg_sbs, id_sb, H[0],
                      (state_pool, sg_pool, tmp_pool, rec_ps))

        # =========== layer 1: xg + recurrence ===========
        def rhs_l1(d, uc, tc):
            src = H[0][uc // 4]
            j = uc % 4
            return src[0:P, 128 * tc + j: 128 * tc + 128: 4]

        with tc.tile_pool(name="xg1ps", bufs=2, space="PSUM") as xg_ps:
            _emit_xg(nc, 1, wih1_sb, rhs_l1, xg_sbs, bias_sb[2:4], xg_ps)

        with tc.tile_pool(name="rst1", bufs=1) as state_pool, \
             tc.tile_pool(name="sg1", bufs=2) as sg_pool, \
             tc.tile_pool(name="tmp1", bufs=2) as tmp_pool, \
             tc.tile_pool(name="rec1ps", bufs=2, space="PSUM") as rec_ps:
            _emit_rec(nc, 1, whh_sb[2:4], xg_sbs, id_sb, H[1],
                      (state_pool, sg_pool, tmp_pool, rec_ps))

        # =========== edge scorer ===========
        with tc.tile_pool(name="edge", bufs=1) as ep, \
             tc.tile_pool(name="edgeth", bufs=3) as thp, \
             tc.tile_pool(name="edgeps", bufs=1, space="PSUM") as epps, \
             tc.tile_pool(name="edgepsS", bufs=1, space="PSUM") as spps:
            uh_sb = ep.tile([100, 800], BF16, name="uhT", tag="uhT")
            nc.sync.dma_start(out=uh_sb[:, :], in_=uhTd[:, :])
            um_sb = ep.tile([100, 800], BF16, name="umT", tag="umT")
            nc.sync.dma_start(out=um_sb[:, :], in_=umTd[:, :])
            b1_sb = ep.tile([1, 100], F32, name="b1row", tag="b1row")
            nc.sync.dma_start(out=b1_sb[:, :], in_=b1rowd[:, :])
            w2_sb = ep.tile([100, 1], BF16, name="w2", tag="w2")
            nc.sync.dma_start(out=w2_sb[:, :], in_=w2d[:, :])
            b2_sb = ep.tile([128, 1], F32, name="b2", tag="b2")
            nc.sync.dma_start(out=b2_sb[:, :], in_=b2d[:, :])
            selT_sb = ep.tile([128, 64], F32, name="selT", tag="selT")
            nc.sync.dma_start(out=selT_sb[0:128, 0:32], in_=selTd[0])
            nc.sync.dma_start(out=selT_sb[0:128, 32:64], in_=selTd[1])

            def h1_rhs(uc):
                return H[1][uc // 4][0:P, uc % 4: 4 * N: 4]

            # A^T [100, 256] (head half of fc1)
            pA = epps.tile([128, 512], F32, name="e1", tag="e1")
            for uc in range(8):
                nc.tensor.matmul(
                    pA[0:P, 0:256],
                    lhsT=uh_sb[0:P, 100 * uc: 100 * uc + 100],
                    rhs=h1_rhs(uc),
                    start=(uc == 0), stop=(uc == 7), skip_group_check=True)
            A_sb = ep.tile([100, 256], F32, name="A", tag="A")
            nc.vector.tensor_copy(out=A_sb[0:P, 0:256], in_=pA[0:P, 0:256])
            # B^T [100, 256] + b1 (modifier half)
            pB = epps.tile([128, 512], F32, name="e3", tag="e3")
            for uc in range(8):
                nc.tensor.matmul(
                    pB[0:P, 0:256],
                    lhsT=um_sb[0:P, 100 * uc: 100 * uc + 100],
                    rhs=h1_rhs(uc),
                    start=(uc == 0), stop=False, skip_group_check=True)
            nc.tensor.matmul(
                pB[0:P, 0:256],
                lhsT=b1_sb[0:1, 0:100],
                rhs=ones_sb[0:1, 0:256],
                start=False, stop=True, skip_group_check=True)
            # A -> token-major via transpose, then per-core 32-head select
            A_tok = ep.tile([128, 256], F32, name="Atok", tag="Atok")
            for m in range(2):
                pT = epps.tile([128, 512], F32, name="e2", tag="e2")
                nc.tensor.transpose(
                    out=pT[0:128, 0:100],
                    in_=A_sb[0:100, 128 * m: 128 * m + 128],
                    identity=idn[0:100, 0:100])
                nc.vector.tensor_copy(
                    out=A_tok[0:128, 128 * m: 128 * m + 100],
                    in_=pT[0:128, 0:100])
            pS = epps.tile([128, 512], F32, name="e1", tag="e1")
            for m in range(2):
                nc.tensor.matmul(
                    pS[0:32, 0:100],
                    lhsT=selT_sb[0:128, 32 * m: 32 * m + 32],
                    rhs=A_tok[0:128, 128 * m: 128 * m + 100],
                    start=(m == 0), stop=(m == 1), skip_group_check=True)
            AselS = ep.tile([128, 128], F32, name="AselS", tag="AselS")
            nc.gpsimd.memset(AselS[:, :], 0.0)
            nc.vector.tensor_copy(out=AselS[0:32, 0:100], in_=pS[0:32, 0:100])
            pAT = epps.tile([128, 512], F32, name="e2", tag="e2")
            nc.tensor.transpose(out=pAT[0:128, 0:128],
                                in_=AselS[0:128, 0:128], identity=idn[:, :])
            AT_sb = ep.tile([128, 32], F32, name="AT", tag="AT")
            nc.vector.tensor_copy(out=AT_sb[0:128, 0:32], in_=pAT[0:128, 0:32])

            # per-head tanh + w2 dot
            psS_tiles = [spps.tile([128, 512], F32, name=f"psS{q}", tag=f"psS{q}")
                         for q in range(4)]
            for q in range(4):
                nc.vector.memset(psS_tiles[q][:, :], 0.0)
            gsb_tiles = [ep.tile([128, 512], F32, name=f"gsb{q}", tag=f"gsb{q}")
                         for q in range(4)]
            for r in range(32):
                th_t = thp.tile([100, 256], BF16, name="th", tag="th")
                nc.scalar.activation(
                    th_t[0:100, 0:256], pB[0:100, 0:256], AF.Tanh,
                    bias=AT_sb[0:100, r:r + 1], scale=1.0)
                q, half = divmod(r // 4, 2)
                nc.tensor.matmul(
                    psS_tiles[q][32 * (r % 4): 32 * (r % 4) + 1,
                                 256 * half: 256 * half + 256],
                    lhsT=w2_sb[0:100, 0:1],
                    rhs=th_t[0:100, 0:256],
                    start=True, stop=True,
                    skip_group_check=True,
                    tile_position=(0, 32 * (r % 4)))
            for q in range(4):
                nc.vector.tensor_scalar(
                    out=gsb_tiles[q][0:128, 0:512],
                    in0=psS_tiles[q][0:128, 0:512],
                    scalar1=b2_sb[0:128, 0:1], scalar2=None, op0=OP.add)
                for half in range(2):
                    rb = 4 * (2 * q + half)
                    nc.sync.dma_start(
                        out=grid[rb:rb + 4, 0:256],
                        in_=gsb_tiles[q][0:128:32, 256 * half: 256 * half + 256])

    nc.compile()
    return nc


_NC_CACHE = None


def _get_nc():
    global _NC_CACHE
    if _NC_CACHE is None:
        _NC_CACHE = build_nc()
    return _NC_CACHE


def kernel(**inputs) -> np.ndarray:
    from concourse.bass_utils import run_bass_kernel_spmd

    arr = _prep_inputs(**inputs)
    nc = _get_nc()
    in_maps = []
    for k in range(NC):
        m = dict(arr)
        m["selT"] = _make_selT(k)
        in_maps.append(m)
    res = run_bass_kernel_spmd(nc, in_maps, core_ids=list(range(NC)))
    grid = np.concatenate([res.results[k]["grid"] for k in range(NC)], axis=0)
    mask = np.ones((N, N), dtype=bool)
    np.fill_diagonal(mask, False)
    mask[:, 0] = False
    return grid[mask].reshape(-1, 1).astype(np.float32)


# revision 18
# speedup vs baseline: 3.0970x; 1.0807x over previous
"""Trainium2 Bass kernel: BiLSTM dependency-parser edge scorer (v2).

Self-contained. Accepts FULL inputs (as produced by setup_inputs()), returns
the FULL [65280, 1] float32 score tensor.

Key idea vs v1: all recurrence matmuls are WEIGHTS-STATIONARY (weights in
lhsT, the tiny h vector streams as rhs), so each step's 64 gate matmuls have
output free-size 1 instead of streaming 6400 PSUM rows.

Layouts (per direction d, layer l):
  gates PSUM tile [100, 16]: partition p, col n = 4*j + g where the LSTM
    unit is u = 100*j + p (j in 0..4) and g in {0:i, 1:f, 2:g, 3:o}.
  h storage H[l][d] [100, 4*256] bf16: h_t for unit (j, p) at col 4*t + j.
    Column 4*t+j is directly the rhs [100, 1] for K-chunk j of the next
    step's matmul -- no transpose inside the loop.
  c state [100, 4] f32.
  xg_sb[d] [100, 16*256] bf16: precomputed input projections + bias,
    injected into the PSUM accumulation via an identity-weight matmul.
g-gate rows are pre-scaled by 2 on host: tanh(x) = 2*sigmoid(2x) - 1.
"""

import os
import sys

sys.path.insert(0, "/opt/trn_rl_repo")

import numpy as np

import concourse.bass as bass
import concourse.mybir as mybir
from concourse import bacc
from concourse.bass import IndirectOffsetOnAxis
from concourse.masks import make_identity
from concourse.tile import TileContext

N = 256          # sequence length
HID = 400        # hidden per direction
NC = 8           # cores
P = 100          # partitions used for unit math
NG = 16          # gate cols per step
F32 = mybir.dt.float32
BF16 = mybir.dt.float16
I32 = mybir.dt.int32
AF = mybir.ActivationFunctionType
OP = mybir.AluOpType

STEPS = int(os.environ.get("DP_STEPS", str(N)))


# ---------------------------------------------------------------------------
# host-side weight layout prep
# ---------------------------------------------------------------------------

def _bf(a):
    return np.ascontiguousarray(np.asarray(a).astype(np.float16))


# R[p, n] = original torch gate-row for (partition p, col n)
_PP, _NN = np.meshgrid(np.arange(P), np.arange(NG), indexing="ij")
_R = 400 * (_NN % 4) + 100 * (_NN // 4) + _PP      # [100, 16]


def _scale_g(W):
    """Scale g-gate rows (orig rows 800:1200) by 2."""
    Ws = np.array(W, dtype=np.float64)
    Ws[800:1200] *= 2.0
    return Ws


def _wblocks(W, nuc):
    """W: [1600, U] scaled gate-major weights, U = 100*nuc.
    Returns [100, 16*nuc*100]: block (n, uc) at cols (n*nuc+uc)*100 holds
    lhsT[k, m] = W[R[m, n], 100*uc + k]."""
    arr = W[_R]                                    # [100p, 16n, U]
    A4 = arr.reshape(P, NG, nuc, 100)              # [p, n, uc, k]
    return A4.transpose(3, 1, 2, 0).reshape(100, NG * nuc * 100)


def _prep_inputs(word_idx, pos_idx, word_emb, pos_emb,
                 Wih0, Whh0, bih0, bhh0, Wih1, Whh1, bih1, bhh1,
                 fc1_W, fc1_b, fc2_W, fc2_b):
    arr = {}
    arr["widx"] = np.ascontiguousarray(
        np.asarray(word_idx).reshape(N, 1).astype(np.int32))
    arr["pidx"] = np.ascontiguousarray(
        np.asarray(pos_idx).reshape(N, 1).astype(np.int32))
    arr["wemb"] = np.ascontiguousarray(np.asarray(word_emb, dtype=np.float32))
    arr["pemb"] = np.ascontiguousarray(np.asarray(pos_emb, dtype=np.float32))

    Wih = [np.asarray(Wih0, np.float64), np.asarray(Wih1, np.float64)]
    Whh = [np.asarray(Whh0, np.float64), np.asarray(Whh1, np.float64)]
    bih = [np.asarray(bih0, np.float64), np.asarray(bih1, np.float64)]
    bhh = [np.asarray(bhh0, np.float64), np.asarray(bhh1, np.float64)]

    whhT = np.zeros((4, 100, NG * 4 * 100), np.float32)
    biasT = np.zeros((4, 100, 512), np.float32)
    wih0T = np.zeros((2, 100, NG * 4 * 100), np.float32)
    wih1T = np.zeros((2, 100, NG * 8 * 100), np.float32)
    for l in range(2):
        for d in range(2):
            dl = 2 * l + d
            whhT[dl] = _wblocks(_scale_g(Whh[l][d]), 4)
            b = _scale_g(bih[l][d] + bhh[l][d])[_R]          # [100, 16]
            biasT[dl] = np.tile(b, (1, 32)).astype(np.float32)
    for d in range(2):
        wih0T[d] = _wblocks(_scale_g(Wih[0][d]), 4)
        wih1T[d] = _wblocks(_scale_g(Wih[1][d]), 8)
    arr["whhT"] = _bf(whhT)
    arr["biasT"] = np.ascontiguousarray(biasT)
    arr["wih0T"] = _bf(wih0T)
    arr["wih1T"] = _bf(wih1T)

    # identity for the xg injection matmul
    arr["id100"] = _bf(np.eye(P, dtype=np.float32))

    # edge MLP: uhT/umT [100, 800]: block uc at cols 100*uc holds
    # lhsT[k, a] = fc1_W[a, 100*uc + k]
    f1 = np.asarray(fc1_W, np.float64)               # [100, 1600]
    arr["uhT"] = _bf(np.concatenate(
        [f1[:, 100 * u:100 * u + 100].T for u in range(8)], axis=1))
    arr["umT"] = _bf(np.concatenate(
        [f1[:, 800 + 100 * u:800 + 100 * u + 100].T for u in range(8)],
        axis=1))
    arr["b1row"] = np.ascontiguousarray(
        np.asarray(fc1_b, np.float32).reshape(1, 100))
    arr["w2"] = _bf(np.asarray(fc2_W, np.float32).reshape(100, 1))
    arr["b2"] = np.ascontiguousarray(
        np.full((128, 1), np.float32(np.asarray(fc2_b).reshape(())),
                dtype=np.float32))
    return arr


def _make_selT(core):
    s = np.zeros((2, 128, 32), np.float32)
    for r in range(32):
        t = 32 * core + r
        s[t // 128, t % 128, r] = 1.0
    return np.ascontiguousarray(s)


# ---------------------------------------------------------------------------
# device kernel build
# ---------------------------------------------------------------------------

def _emit_xg_group(nc, nuc, ucs, wih_sb_d, rhs_chunk, d, tc, dst_sb,
                   bias_sb, ps_pool, tag):
    """One t-chunk (32 tokens) of an input-projection GEMM: 16*len(ucs)
    weights-stationary matmuls accumulating into a PSUM bank, then one
    PSUM->SBUF copy (adding bias if given)."""
    ps = ps_pool.tile([128, 512], F32, name=tag, tag=tag)
    for n in range(NG):
        for i, uc in enumerate(ucs):
            nc.tensor.matmul(
                ps[0:P, n:512:16],
                lhsT=wih_sb_d[0:P, (n * nuc + uc) * 100:
                              (n * nuc + uc) * 100 + 100],
                rhs=rhs_chunk(d, uc, tc),
                start=(i == 0), stop=(i == len(ucs) - 1),
                skip_group_check=True)
    if bias_sb is not None:
        nc.vector.tensor_tensor(
            out=dst_sb[0:P, 512 * tc: 512 * tc + 512],
            in0=ps[0:P, 0:512], in1=bias_sb[0:P, 0:512], op=OP.add)
    else:
        nc.vector.tensor_copy(
            out=dst_sb[0:P, 512 * tc: 512 * tc + 512],
            in_=ps[0:P, 0:512])


def _emit_xg(nc, l, wih_sb, rhs_chunk, xg_sbs, bias_sbs, ps_pool):
    """Full xg for layer l: xg[d][p, 16*t + n] = sum_u W[r(p,n), u]*in[t,u]+b."""
    nuc = 4 if l == 0 else 8
    for d in range(2):
        for tc in range(8):
            _emit_xg_group(nc, nuc, list(range(nuc)), wih_sb[d], rhs_chunk,
                           d, tc, xg_sbs[d], bias_sbs[d], ps_pool, "xgps")


def _emit_rec(nc, l, whh_sb, xg_sbs, id_sb, H_out, pools, xg2_sbs=None,
              extra=None):
    """STEPS wall-steps, both directions interleaved. xg2_sbs: optional
    second injection source (bwd-half input projections for layer 1).
    extra(t): called after each wall-step to emit overlapped work."""
    state_pool, sg_pool, tmp_pool, ps_pool = pools
    cs = []
    for d in range(2):
        c = state_pool.tile([P, 4], F32, name=f"c{d}", tag=f"c{d}")
        nc.gpsimd.memset(c[:, :], 0.0)
        cs.append(c)

    for t in range(STEPS):
        ps_t, sg_t, th_t = [], [], []
        # --- PE: injection + 64 weight matmuls per direction ---
        for d in range(2):
            tdx = t if d == 0 else (STEPS - 1 - t)
            ps = ps_pool.tile([128, 512], F32, name=f"ps{d}", tag=f"ps{d}")
            ps_t.append(ps)
            first = (t == 0)
            nc.tensor.matmul(
                ps[0:P, 0:NG],
                lhsT=id_sb[0:P, 0:P],
                rhs=xg_sbs[d][0:P, NG * tdx: NG * tdx + NG],
                start=True, stop=(first and xg2_sbs is None),
                skip_group_check=True)
            if xg2_sbs is not None:
                nc.tensor.matmul(
                    ps[0:P, 0:NG],
                    lhsT=id_sb[0:P, 0:P],
                    rhs=xg2_sbs[d][0:P, NG * tdx: NG * tdx + NG],
                    start=False, stop=first, skip_group_check=True)
            if not first:
                pdx = tdx - 1 if d == 0 else tdx + 1
                # even cols (i, g gates) first: they gate the c-update chain
                for n in list(range(0, NG, 2)) + list(range(1, NG, 2)):
                    for uc in range(4):
                        nc.tensor.matmul(
                            ps[0:P, n:n + 1],
                            lhsT=whh_sb[d][0:P, (4 * n + uc) * 100:
                                           (4 * n + uc) * 100 + 100],
                            rhs=H_out[d][0:P, 4 * pdx + uc: 4 * pdx + uc + 1],
                            start=False, stop=(uc == 3),
                            skip_group_check=True)
        # --- Act: sigmoid over all gates (g pre-scaled by 2) ---
        for d in range(2):
            sg = sg_pool.tile([P, NG], F32, name=f"sg{d}", tag=f"sg{d}")
            sg_t.append(sg)
            nc.scalar.activation(sg[0:P, 0:NG], ps_t[d][0:P, 0:NG], AF.Sigmoid)
        # --- DVE per direction: c = sig(f)*c + sig(i)*(2*sig(2g) - 1),
        #     then tanh(c) via Pade [3/2] (|c| < 0.5 here, err < 1e-6):
        #     tanh(c) ~= c*(15 + c^2) / (15 + 6*c^2);  h = sig(o)*tanh ---
        for d in range(2):
            tdx = t if d == 0 else (STEPS - 1 - t)
            sg, c = sg_t[d], cs[d]
            tg = tmp_pool.tile([P, 4], F32, name=f"tg{d}", tag=f"tg{d}")
            t1 = tmp_pool.tile([P, 4], F32, name=f"t1{d}", tag=f"t1{d}")
            cf = tmp_pool.tile([P, 4], F32, name=f"cf{d}", tag=f"cf{d}")
            nc.vector.tensor_scalar(
                out=tg[0:P, 0:4], in0=sg[0:P, 2:NG:4],
                scalar1=2.0, scalar2=-1.0, op0=OP.mult, op1=OP.add)
            nc.vector.tensor_tensor(
                out=cf[0:P, 0:4], in0=sg[0:P, 1:NG:4],
                in1=c[0:P, 0:4], op=OP.mult)
            nc.vector.tensor_tensor(
                out=t1[0:P, 0:4], in0=sg[0:P, 0:NG:4],
                in1=tg[0:P, 0:4], op=OP.mult)
            nc.vector.tensor_tensor(
                out=c[0:P, 0:4], in0=cf[0:P, 0:4],
                in1=t1[0:P, 0:4], op=OP.add)
            th = tmp_pool.tile([P, 4], F32, name=f"th{d}", tag=f"th{d}")
            th_t.append(th)
            nc.scalar.activation(th[0:P, 0:4], c[0:P, 0:4], AF.Tanh)
        for d in range(2):
            tdx = t if d == 0 else (STEPS - 1 - t)
            nc.vector.tensor_tensor(
                out=H_out[d][0:P, 4 * tdx: 4 * tdx + 4],
                in0=sg_t[d][0:P, 3:NG:4], in1=th_t[d][0:P, 0:4], op=OP.mult)
        if extra is not None:
            extra(t)


def build_nc():
    nc = bacc.Bacc("TRN2", target_bir_lowering=False, debug=False,
                   num_devices=NC)
    wemb = nc.dram_tensor("wemb", [50000, 300], F32, kind="ExternalInput").ap()
    pemb = nc.dram_tensor("pemb", [50, 100], F32, kind="ExternalInput").ap()
    widx = nc.dram_tensor("widx", [N, 1], I32, kind="ExternalInput").ap()
    pidx = nc.dram_tensor("pidx", [N, 1], I32, kind="ExternalInput").ap()
    whhTd = nc.dram_tensor("whhT", [4, 100, 6400], BF16, kind="ExternalInput").ap()
    wih0Td = nc.dram_tensor("wih0T", [2, 100, 6400], BF16, kind="ExternalInput").ap()
    wih1Td = nc.dram_tensor("wih1T", [2, 100, 12800], BF16, kind="ExternalInput").ap()
    biasTd = nc.dram_tensor("biasT", [4, 100, 512], F32, kind="ExternalInput").ap()
    id100d = nc.dram_tensor("id100", [100, 100], BF16, kind="ExternalInput").ap()
    uhTd = nc.dram_tensor("uhT", [100, 800], BF16, kind="ExternalInput").ap()
    umTd = nc.dram_tensor("umT", [100, 800], BF16, kind="ExternalInput").ap()
    b1rowd = nc.dram_tensor("b1row", [1, 100], F32, kind="ExternalInput").ap()
    w2d = nc.dram_tensor("w2", [100, 1], BF16, kind="ExternalInput").ap()
    b2d = nc.dram_tensor("b2", [128, 1], F32, kind="ExternalInput").ap()
    selTd = nc.dram_tensor("selT", [2, 128, 32], F32, kind="ExternalInput").ap()
    grid = nc.dram_tensor("grid", [32, N], F32, kind="ExternalOutput").ap()

    from contextlib import ExitStack
    with TileContext(nc) as tc, ExitStack() as ctx:
        top = ctx.enter_context(tc.tile_pool(name="top", bufs=1))
        # ---- persistent SBUF tiles (DMAs emitted in priority order) ----
        idn = top.tile([128, 128], F32, name="idn", tag="idn")
        make_identity(nc, idn[:, :])
        ones_sb = top.tile([1, 256], F32, name="ones", tag="ones")
        nc.gpsimd.memset(ones_sb[:, :], 1.0)
        whh_sb = [top.tile([100, 6400], BF16, name=f"whh{dl}", tag=f"whh{dl}")
                  for dl in range(4)]
        bias_sb = [top.tile([100, 512], F32, name=f"bias{dl}", tag=f"bias{dl}")
                   for dl in range(4)]
        id_sb = top.tile([100, 100], BF16, name="id100", tag="id100")
        wih1_sb = [top.tile([100, 12800], BF16, name=f"wih1{d}", tag=f"wih1{d}")
                   for d in range(2)]
        xg_sbs = [top.tile([100, 4096], BF16, name=f"xg{d}", tag=f"xg{d}")
                  for d in range(2)]
        H = [[top.tile([100, 4 * N], BF16, name=f"H{l}{d}", tag=f"H{l}{d}")
              for d in range(2)] for l in range(2)]
        xT = top.tile([100, 4 * N], BF16, name="xT", tag="xT")
        if STEPS < N:
            for l in range(2):
                for d in range(2):
                    nc.gpsimd.memset(H[l][d][:, :], 0.0)

        # ========= embedding gather (first DMAs in the queue) =========
        with tc.tile_pool(name="wih0p", bufs=1) as w0p, \
             tc.tile_pool(name="embps", bufs=2, space="PSUM") as eps:
            idx_sb = w0p.tile([128, 4], I32, name="idx", tag="idx")
            nc.sync.dma_start(out=idx_sb[0:128, 0:1], in_=widx[0:128, 0:1])
            nc.sync.dma_start(out=idx_sb[0:128, 1:2], in_=widx[128:256, 0:1])
            nc.sync.dma_start(out=idx_sb[0:128, 2:3], in_=pidx[0:128, 0:1])
            nc.sync.dma_start(out=idx_sb[0:128, 3:4], in_=pidx[128:256, 0:1])
            x_sb = w0p.tile([128, 800], F32, name="xsb", tag="xsb")
            for cch in range(2):
                nc.gpsimd.indirect_dma_start(
                    out=x_sb[0:128, 400 * cch: 400 * cch + 300],
                    out_offset=None,
                    in_=wemb[:, :],
                    in_offset=IndirectOffsetOnAxis(
                        ap=idx_sb[0:128, cch:cch + 1], axis=0))
                nc.gpsimd.indirect_dma_start(
                    out=x_sb[0:128, 400 * cch + 300: 400 * cch + 400],
                    out_offset=None,
                    in_=pemb[:, :],
                    in_offset=IndirectOffsetOnAxis(
                        ap=idx_sb[0:128, 2 + cch:3 + cch], axis=0))
            # layer-0 weights + rec0 needs, in DMA-queue priority order
            wih0_sb = [w0p.tile([100, 6400], BF16, name=f"wih0{d}",
                                tag=f"wih0{d}") for d in range(2)]
            for d in range(2):
                nc.sync.dma_start(out=wih0_sb[d][:, :], in_=wih0Td[d])
            for dl in range(2):
                nc.sync.dma_start(out=bias_sb[dl][:, :], in_=biasTd[dl])
            nc.sync.dma_start(out=id_sb[:, :], in_=id100d[:, :])
            for dl in range(2):
                nc.sync.dma_start(out=whh_sb[dl][:, :], in_=whhTd[dl])

            # x -> xT transpose
            for cch in range(2):
                for uc in range(4):
                    ptr = eps.tile([128, 128], F32, name="ptr", tag="ptr")
                    nc.tensor.transpose(
                        out=ptr[0:100, 0:128],
                        in_=x_sb[0:128, 400 * cch + 100 * uc:
                                 400 * cch + 100 * uc + 100],
                        identity=idn[:, :])
                    nc.vector.tensor_copy(
                        out=xT[0:100, 256 * uc + 128 * cch:
                               256 * uc + 128 * cch + 128],
                        in_=ptr[0:100, 0:128])

            # ========= layer 0 xg (serial, before rec0) =========
            def rhs_l0(d, uc, tc):
                return xT[0:P, 256 * uc + 32 * tc: 256 * uc + 32 * tc + 32]

            with tc.tile_pool(name="xg0ps", bufs=2, space="PSUM") as xg_ps:
                _emit_xg(nc, 0, wih0_sb, rhs_l0, xg_sbs, bias_sb[0:2], xg_ps)

        # remaining big DMAs: execute during rec0
        for d in range(2):
            nc.sync.dma_start(out=wih1_sb[d][:, :], in_=wih1Td[d])
        for dl in range(2, 4):
            nc.sync.dma_start(out=whh_sb[dl][:, :], in_=whhTd[dl])
            nc.sync.dma_start(out=bias_sb[dl][:, :], in_=biasTd[dl])

        # ========= rec0 with layer-1 xg interleaved =========
        def rhs_l1(d, uc, tc):
            src = H[0][uc // 4]
            j = uc % 4
            return src[0:P, 128 * tc + j: 128 * tc + 128: 4]

        with tc.tile_pool(name="xg1buf", bufs=1) as xgbuf, \
             tc.tile_pool(name="xg1ps", bufs=2, space="PSUM") as xg1_ps:
            xgf_sbs = [xgbuf.tile([100, 4096], BF16, name=f"xgf{d}",
                                  tag=f"xgf{d}") for d in range(2)]
            xgb_sbs = [xgbuf.tile([100, 4096], BF16, name=f"xgb{d}",
                                  tag=f"xgb{d}") for d in range(2)]

            def mk_group(d, tc_, half):
                def emit():
                    _emit_xg_group(
                        nc, 8, list(range(4 * half, 4 * half + 4)),
                        wih1_sb[d], rhs_l1, d, tc_,
                        xgf_sbs[d] if half == 0 else xgb_sbs[d],
                        bias_sb[2 + d] if half == 0 else None,
                        xg1_ps, "xg1ps")
                return emit

            pend = []
            for d in range(2):
                for tc_ in range(8):
                    pend.append((32 * tc_ + 32, mk_group(d, tc_, 0)))
                    pend.append((N - 32 * tc_, mk_group(d, tc_, 1)))
            pend.sort(key=lambda x: x[0])
            st = {"i": 0, "last": -10}

            def extra(t):
                if (st["i"] < len(pend) and pend[st["i"]][0] <= t
                        and t - st["last"] >= 2):
                    pend[st["i"]][1]()
                    st["i"] += 1
                    st["last"] = t

            with tc.tile_pool(name="rst0", bufs=1) as state_pool, \
                 tc.tile_pool(name="sg0", bufs=3) as sg_pool, \
                 tc.tile_pool(name="tmp0", bufs=3) as tmp_pool, \
                 tc.tile_pool(name="rec0ps", bufs=3, space="PSUM") as rec_ps:
                _emit_rec(nc, 0, whh_sb[0:2], xg_sbs, id_sb, H[0],
                          (state_pool, sg_pool, tmp_pool, rec_ps),
                          extra=extra)
            # leftover xg1 groups (ends of both directions)
            while st["i"] < len(pend):
                pend[st["i"]][1]()
                st["i"] += 1

            # ========= rec1 (dual injection: fwd + bwd halves) =========
            with tc.tile_pool(name="rst1", bufs=1) as state_pool, \
                 tc.tile_pool(name="sg1", bufs=3) as sg_pool, \
                 tc.tile_pool(name="tmp1", bufs=3) as tmp_pool, \
                 tc.tile_pool(name="rec1ps", bufs=3, space="PSUM") as rec_ps:
                _emit_rec(nc, 1, whh_sb[2:4], xgf_sbs, id_sb, H[1],
                          (state_pool, sg_pool, tmp_pool, rec_ps),
                          xg2_sbs=xgb_sbs)

        # ========= edge scorer =========
        with tc.tile_pool(name="edge", bufs=1) as ep, \
             tc.tile_pool(name="edgeth", bufs=16) as thp, \
             tc.tile_pool(name="edgeps", bufs=1, space="PSUM") as epps, \
             tc.tile_pool(name="edgepsS", bufs=1, space="PSUM") as spps:
            uh_sb = ep.tile([100, 800], BF16, name="uhT", tag="uhT")
            nc.sync.dma_start(out=uh_sb[:, :], in_=uhTd[:, :])
            um_sb = ep.tile([100, 800], BF16, name="umT", tag="umT")
            nc.sync.dma_start(out=um_sb[:, :], in_=umTd[:, :])
            b1_sb = ep.tile([1, 100], F32, name="b1row", tag="b1row")
            nc.sync.dma_start(out=b1_sb[:, :], in_=b1rowd[:, :])
            w2_sb = ep.tile([100, 1], BF16, name="w2", tag="w2")
            nc.sync.dma_start(out=w2_sb[:, :], in_=w2d[:, :])
            b2_sb = ep.tile([128, 1], F32, name="b2", tag="b2")
            nc.sync.dma_start(out=b2_sb[:, :], in_=b2d[:, :])
            selT_sb = ep.tile([128, 64], F32, name="selT", tag="selT")
            nc.sync.dma_start(out=selT_sb[0:128, 0:32], in_=selTd[0])
            nc.sync.dma_start(out=selT_sb[0:128, 32:64], in_=selTd[1])

            def h1_rhs(uc):
                return H[1][uc // 4][0:P, uc % 4: 4 * N: 4]

            # A^T [100, 256] (head half of fc1)
            pA = epps.tile([128, 512], F32, name="e1", tag="e1")
            for uc in range(8):
                nc.tensor.matmul(
                    pA[0:P, 0:256],
                    lhsT=uh_sb[0:P, 100 * uc: 100 * uc + 100],
                    rhs=h1_rhs(uc),
                    start=(uc == 0), stop=(uc == 7), skip_group_check=True)
            A_sb = ep.tile([100, 256], F32, name="A", tag="A")
            nc.vector.tensor_copy(out=A_sb[0:P, 0:256], in_=pA[0:P, 0:256])
            # B^T [100, 256] + b1 (modifier half)
            pB = epps.tile([128, 512], F32, name="e3", tag="e3")
            for uc in range(8):
                nc.tensor.matmul(
                    pB[0:P, 0:256],
                    lhsT=um_sb[0:P, 100 * uc: 100 * uc + 100],
                    rhs=h1_rhs(uc),
                    start=(uc == 0), stop=False, skip_group_check=True)
            nc.tensor.matmul(
                pB[0:P, 0:256],
                lhsT=b1_sb[0:1, 0:100],
                rhs=ones_sb[0:1, 0:256],
                start=False, stop=True, skip_group_check=True)
            # A -> token-major via transpose, then per-core 32-head select
            A_tok = ep.tile([128, 256], F32, name="Atok", tag="Atok")
            for m in range(2):
                pT = epps.tile([128, 512], F32, name="e2", tag="e2")
                nc.tensor.transpose(
                    out=pT[0:128, 0:100],
                    in_=A_sb[0:100, 128 * m: 128 * m + 128],
                    identity=idn[0:100, 0:100])
                nc.vector.tensor_copy(
                    out=A_tok[0:128, 128 * m: 128 * m + 100],
                    in_=pT[0:128, 0:100])
            pS = epps.tile([128, 512], F32, name="e1", tag="e1")
            for m in range(2):
                nc.tensor.matmul(
                    pS[0:32, 0:100],
                    lhsT=selT_sb[0:128, 32 * m: 32 * m + 32],
                    rhs=A_tok[0:128, 128 * m: 128 * m + 100],
                    start=(m == 0), stop=(m == 1), skip_group_check=True)
            AselS = ep.tile([128, 128], F32, name="AselS", tag="AselS")
            nc.gpsimd.memset(AselS[:, :], 0.0)
            nc.vector.tensor_copy(out=AselS[0:32, 0:100], in_=pS[0:32, 0:100])
            pAT = epps.tile([128, 512], F32, name="e2", tag="e2")
            nc.tensor.transpose(out=pAT[0:128, 0:128],
                                in_=AselS[0:128, 0:128], identity=idn[:, :])
            AT_sb = ep.tile([128, 32], F32, name="AT", tag="AT")
            nc.vector.tensor_copy(out=AT_sb[0:128, 0:32], in_=pAT[0:128, 0:32])

            # per-head tanh + w2 dot
            psS_tiles = [spps.tile([128, 512], F32, name=f"psS{q}", tag=f"psS{q}")
                         for q in range(4)]
            for q in range(4):
                nc.vector.memset(psS_tiles[q][:, :], 0.0)
            gsb_tiles = [ep.tile([128, 512], F32, name=f"gsb{q}", tag=f"gsb{q}")
                         for q in range(4)]
            for r in range(32):
                th_t = thp.tile([100, 256], BF16, name="th", tag="th")
                nc.scalar.activation(
                    th_t[0:100, 0:256], pB[0:100, 0:256], AF.Tanh,
                    bias=AT_sb[0:100, r:r + 1], scale=1.0)
                q, half = divmod(r // 4, 2)
                nc.tensor.matmul(
                    psS_tiles[q][32 * (r % 4): 32 * (r % 4) + 1,
                                 256 * half: 256 * half + 256],
                    lhsT=w2_sb[0:100, 0:1],
                    rhs=th_t[0:100, 0:256],
                    start=True, stop=True,
                    skip_group_check=True,
                    tile_position=(0, 32 * (r % 4)))
            for q in range(4):
                nc.vector.tensor_scalar(
                    out=gsb_tiles[q][0:128, 0:512],
                    in0=psS_tiles[q][0:128, 0:512],
                    scalar1=b2_sb[0:128, 0:1], scalar2=None, op0=OP.add)
                for half in range(2):
                    rb = 4 * (2 * q + half)
                    nc.sync.dma_start(
                        out=grid[rb:rb + 4, 0:256],
                        in_=gsb_tiles[q][0:128:32, 256 * half: 256 * half + 256])

    nc.compile()
    return nc


_NC_CACHE = None


def _get_nc():
    global _NC_CACHE
    if _NC_CACHE is None:
        _NC_CACHE = build_nc()
    return _NC_CACHE


def kernel(**inputs) -> np.ndarray:
    from concourse.bass_utils import run_bass_kernel_spmd

    arr = _prep_inputs(**inputs)
    nc = _get_nc()
    in_maps = []
    for k in range(NC):
        m = dict(arr)
        m["selT"] = _make_selT(k)
        in_maps.append(m)
    res = run_bass_kernel_spmd(nc, in_maps, core_ids=list(range(NC)))
    grid = np.concatenate([res.results[k]["grid"] for k in range(NC)], axis=0)
    mask = np.ones((N, N), dtype=bool)
    np.fill_diagonal(mask, False)
    mask[:, 0] = False
    return grid[mask].reshape(-1, 1).astype(np.float32)


# revision 27
# speedup vs baseline: 3.1338x; 1.0119x over previous
"""Trainium2 Bass kernel: BiLSTM dependency-parser edge scorer (v2).

Self-contained. Accepts FULL inputs (as produced by setup_inputs()), returns
the FULL [65280, 1] float32 score tensor.

Key idea vs v1: all recurrence matmuls are WEIGHTS-STATIONARY (weights in
lhsT, the tiny h vector streams as rhs), so each step's 64 gate matmuls have
output free-size 1 instead of streaming 6400 PSUM rows.

Layouts (per direction d, layer l):
  gates PSUM tile [100, 16]: partition p, col n = 4*j + g where the LSTM
    unit is u = 100*j + p (j in 0..4) and g in {0:i, 1:f, 2:g, 3:o}.
  h storage H[l][d] [100, 4*256] bf16: h_t for unit (j, p) at col 4*t + j.
    Column 4*t+j is directly the rhs [100, 1] for K-chunk j of the next
    step's matmul -- no transpose inside the loop.
  c state [100, 4] f32.
  xg_sb[d] [100, 16*256] bf16: precomputed input projections + bias,
    injected into the PSUM accumulation via an identity-weight matmul.
g-gate rows are pre-scaled by 2 on host: tanh(x) = 2*sigmoid(2x) - 1.
"""

import os
import sys

sys.path.insert(0, "/opt/trn_rl_repo")

import numpy as np

import concourse.bass as bass
import concourse.mybir as mybir
from concourse import bacc
from concourse.bass import IndirectOffsetOnAxis
from concourse.masks import make_identity
from concourse.tile import TileContext

N = 256          # sequence length
HID = 400        # hidden per direction
NC = 8           # cores
P = 100          # partitions used for unit math
NG = 16          # gate cols per step
F32 = mybir.dt.float32
BF16 = mybir.dt.float16
I32 = mybir.dt.int32
AF = mybir.ActivationFunctionType
OP = mybir.AluOpType

STEPS = int(os.environ.get("DP_STEPS", str(N)))


# ---------------------------------------------------------------------------
# host-side weight layout prep
# ---------------------------------------------------------------------------

def _bf(a):
    return np.ascontiguousarray(np.asarray(a).astype(np.float16))


# R[p, n] = original torch gate-row for (partition p, col n)
_PP, _NN = np.meshgrid(np.arange(P), np.arange(NG), indexing="ij")
_R = 400 * (_NN % 4) + 100 * (_NN // 4) + _PP      # [100, 16]


def _scale_g(W):
    """Scale g-gate rows (orig rows 800:1200) by 2."""
    Ws = np.array(W, dtype=np.float64)
    Ws[800:1200] *= 2.0
    return Ws


def _wblocks(W, nuc):
    """W: [1600, U] scaled gate-major weights, U = 100*nuc.
    Returns [100, 16*nuc*100]: block (n, uc) at cols (n*nuc+uc)*100 holds
    lhsT[k, m] = W[R[m, n], 100*uc + k]."""
    arr = W[_R]                                    # [100p, 16n, U]
    A4 = arr.reshape(P, NG, nuc, 100)              # [p, n, uc, k]
    return A4.transpose(3, 1, 2, 0).reshape(100, NG * nuc * 100)


def _prep_inputs(word_idx, pos_idx, word_emb, pos_emb,
                 Wih0, Whh0, bih0, bhh0, Wih1, Whh1, bih1, bhh1,
                 fc1_W, fc1_b, fc2_W, fc2_b):
    arr = {}
    arr["widx"] = np.ascontiguousarray(
        np.asarray(word_idx).reshape(N, 1).astype(np.int32))
    arr["pidx"] = np.ascontiguousarray(
        np.asarray(pos_idx).reshape(N, 1).astype(np.int32))
    arr["wemb"] = np.ascontiguousarray(np.asarray(word_emb, dtype=np.float32))
    arr["pemb"] = np.ascontiguousarray(np.asarray(pos_emb, dtype=np.float32))

    Wih = [np.asarray(Wih0, np.float64), np.asarray(Wih1, np.float64)]
    Whh = [np.asarray(Whh0, np.float64), np.asarray(Whh1, np.float64)]
    bih = [np.asarray(bih0, np.float64), np.asarray(bih1, np.float64)]
    bhh = [np.asarray(bhh0, np.float64), np.asarray(bhh1, np.float64)]

    whhT = np.zeros((4, 100, NG * 4 * 100), np.float32)
    biasT = np.zeros((4, 100, 512), np.float32)
    wih0T = np.zeros((2, 100, NG * 4 * 100), np.float32)
    wih1T = np.zeros((2, 100, NG * 8 * 100), np.float32)
    for l in range(2):
        for d in range(2):
            dl = 2 * l + d
            whhT[dl] = _wblocks(_scale_g(Whh[l][d]), 4)
            b = _scale_g(bih[l][d] + bhh[l][d])[_R]          # [100, 16]
            biasT[dl] = np.tile(b, (1, 32)).astype(np.float32)
    for d in range(2):
        wih0T[d] = _wblocks(_scale_g(Wih[0][d]), 4)
        wih1T[d] = _wblocks(_scale_g(Wih[1][d]), 8)
    arr["whhT"] = _bf(whhT)
    arr["biasT"] = np.ascontiguousarray(biasT)
    arr["wih0T"] = _bf(wih0T)
    arr["wih1T"] = _bf(wih1T)

    # identity for the xg injection matmul
    arr["id100"] = _bf(np.eye(P, dtype=np.float32))

    # edge MLP: uhT/umT [100, 800]: block uc at cols 100*uc holds
    # lhsT[k, a] = fc1_W[a, 100*uc + k]
    f1 = np.asarray(fc1_W, np.float64)               # [100, 1600]
    arr["uhT"] = _bf(np.concatenate(
        [f1[:, 100 * u:100 * u + 100].T for u in range(8)], axis=1))
    arr["umT"] = _bf(np.concatenate(
        [f1[:, 800 + 100 * u:800 + 100 * u + 100].T for u in range(8)],
        axis=1))
    arr["b1row"] = np.ascontiguousarray(
        np.asarray(fc1_b, np.float32).reshape(1, 100))
    arr["w2"] = _bf(np.asarray(fc2_W, np.float32).reshape(100, 1))
    arr["b2"] = np.ascontiguousarray(
        np.full((128, 1), np.float32(np.asarray(fc2_b).reshape(())),
                dtype=np.float32))
    return arr


def _make_selT(core):
    s = np.zeros((2, 128, 32), np.float32)
    for r in range(32):
        t = 32 * core + r
        s[t // 128, t % 128, r] = 1.0
    return np.ascontiguousarray(s)


# ---------------------------------------------------------------------------
# device kernel build
# ---------------------------------------------------------------------------

def _emit_xg_group(nc, nuc, ucs, wih_sb_d, rhs_chunk, d, tc, dst_sb,
                   bias_sb, ps_pool, tag):
    """One t-chunk (32 tokens) of an input-projection GEMM: 16*len(ucs)
    weights-stationary matmuls accumulating into a PSUM bank, then one
    PSUM->SBUF copy (adding bias if given)."""
    ps = ps_pool.tile([128, 512], F32, name=tag, tag=tag)
    for n in range(NG):
        for i, uc in enumerate(ucs):
            nc.tensor.matmul(
                ps[0:P, n:512:16],
                lhsT=wih_sb_d[0:P, (n * nuc + uc) * 100:
                              (n * nuc + uc) * 100 + 100],
                rhs=rhs_chunk(d, uc, tc),
                start=(i == 0), stop=(i == len(ucs) - 1),
                skip_group_check=True)
    if bias_sb is not None:
        nc.vector.tensor_tensor(
            out=dst_sb[0:P, 512 * tc: 512 * tc + 512],
            in0=ps[0:P, 0:512], in1=bias_sb[0:P, 0:512], op=OP.add)
    else:
        nc.vector.tensor_copy(
            out=dst_sb[0:P, 512 * tc: 512 * tc + 512],
            in_=ps[0:P, 0:512])


def _emit_xg(nc, l, wih_sb, rhs_chunk, xg_sbs, bias_sbs, ps_pool):
    """Full xg for layer l: xg[d][p, 16*t + n] = sum_u W[r(p,n), u]*in[t,u]+b."""
    nuc = 4 if l == 0 else 8
    for d in range(2):
        for tc in range(8):
            _emit_xg_group(nc, nuc, list(range(nuc)), wih_sb[d], rhs_chunk,
                           d, tc, xg_sbs[d], bias_sbs[d], ps_pool, "xgps")


def _emit_rec(nc, l, whh_sb, xg_sbs, id_sb, H_out, pools, xg2_sbs=None,
              extra=None):
    """STEPS wall-steps, both directions interleaved. xg2_sbs: optional
    second injection source (bwd-half input projections for layer 1).
    extra(t): called after each wall-step to emit overlapped work."""
    state_pool, sg_pool, tmp_pool, ps_pool = pools
    cs = []
    for d in range(2):
        c = state_pool.tile([P, 4], F32, name=f"c{d}", tag=f"c{d}")
        nc.gpsimd.memset(c[:, :], 0.0)
        cs.append(c)

    for t in range(STEPS):
        ps_t, sg_t, th_t = {}, {}, {}
        dorder = (0, 1)
        # --- PE: injection + 64 weight matmuls per direction ---
        for d in dorder:
            tdx = t if d == 0 else (STEPS - 1 - t)
            ps = ps_pool.tile([128, 512], F32, name=f"ps{d}", tag=f"ps{d}")
            ps_t[d] = ps
            first = (t == 0)
            nc.tensor.matmul(
                ps[0:P, 0:NG],
                lhsT=id_sb[0:P, 0:P],
                rhs=xg_sbs[d][0:P, NG * tdx: NG * tdx + NG],
                start=True, stop=(first and xg2_sbs is None),
                skip_group_check=True)
            if xg2_sbs is not None:
                nc.tensor.matmul(
                    ps[0:P, 0:NG],
                    lhsT=id_sb[0:P, 0:P],
                    rhs=xg2_sbs[d][0:P, NG * tdx: NG * tdx + NG],
                    start=False, stop=first, skip_group_check=True)
            if not first:
                pdx = tdx - 1 if d == 0 else tdx + 1
                # even cols (i, g gates) first: they gate the c-update chain
                for n in list(range(0, NG, 2)) + list(range(1, NG, 2)):
                    for uc in range(4):
                        nc.tensor.matmul(
                            ps[0:P, n:n + 1],
                            lhsT=whh_sb[d][0:P, (4 * n + uc) * 100:
                                           (4 * n + uc) * 100 + 100],
                            rhs=H_out[d][0:P, 4 * pdx + uc: 4 * pdx + uc + 1],
                            start=False, stop=(uc == 3),
                            skip_group_check=True)
        # --- Act: sigmoid over all gates (g pre-scaled by 2) ---
        for d in dorder:
            sg = sg_pool.tile([P, NG], F32, name=f"sg{d}", tag=f"sg{d}")
            sg_t[d] = sg
            nc.scalar.activation(sg[0:P, 0:NG], ps_t[d][0:P, 0:NG], AF.Sigmoid)
        # --- DVE per direction: c = sig(f)*c + sig(i)*(2*sig(2g) - 1),
        #     then tanh(c) via Pade [3/2] (|c| < 0.5 here, err < 1e-6):
        #     tanh(c) ~= c*(15 + c^2) / (15 + 6*c^2);  h = sig(o)*tanh ---
        for d in dorder:
            tdx = t if d == 0 else (STEPS - 1 - t)
            sg, c = sg_t[d], cs[d]
            tg = tmp_pool.tile([P, 4], F32, name=f"tg{d}", tag=f"tg{d}")
            t1 = tmp_pool.tile([P, 4], F32, name=f"t1{d}", tag=f"t1{d}")
            cf = tmp_pool.tile([P, 4], F32, name=f"cf{d}", tag=f"cf{d}")
            nc.vector.tensor_scalar(
                out=tg[0:P, 0:4], in0=sg[0:P, 2:NG:4],
                scalar1=2.0, scalar2=-1.0, op0=OP.mult, op1=OP.add)
            nc.vector.tensor_tensor(
                out=cf[0:P, 0:4], in0=sg[0:P, 1:NG:4],
                in1=c[0:P, 0:4], op=OP.mult)
            nc.vector.tensor_tensor(
                out=t1[0:P, 0:4], in0=sg[0:P, 0:NG:4],
                in1=tg[0:P, 0:4], op=OP.mult)
            nc.vector.tensor_tensor(
                out=c[0:P, 0:4], in0=cf[0:P, 0:4],
                in1=t1[0:P, 0:4], op=OP.add)
            th = tmp_pool.tile([P, 4], F32, name=f"th{d}", tag=f"th{d}")
            th_t[d] = th
            nc.scalar.activation(th[0:P, 0:4], c[0:P, 0:4], AF.Tanh)
        for d in dorder:
            tdx = t if d == 0 else (STEPS - 1 - t)
            nc.vector.tensor_tensor(
                out=H_out[d][0:P, 4 * tdx: 4 * tdx + 4],
                in0=sg_t[d][0:P, 3:NG:4], in1=th_t[d][0:P, 0:4], op=OP.mult)
        if extra is not None:
            extra(t)


def build_nc():
    nc = bacc.Bacc("TRN2", target_bir_lowering=False, debug=False,
                   num_devices=NC)
    wemb = nc.dram_tensor("wemb", [50000, 300], F32, kind="ExternalInput").ap()
    pemb = nc.dram_tensor("pemb", [50, 100], F32, kind="ExternalInput").ap()
    widx = nc.dram_tensor("widx", [N, 1], I32, kind="ExternalInput").ap()
    pidx = nc.dram_tensor("pidx", [N, 1], I32, kind="ExternalInput").ap()
    whhTd = nc.dram_tensor("whhT", [4, 100, 6400], BF16, kind="ExternalInput").ap()
    wih0Td = nc.dram_tensor("wih0T", [2, 100, 6400], BF16, kind="ExternalInput").ap()
    wih1Td = nc.dram_tensor("wih1T", [2, 100, 12800], BF16, kind="ExternalInput").ap()
    biasTd = nc.dram_tensor("biasT", [4, 100, 512], F32, kind="ExternalInput").ap()
    id100d = nc.dram_tensor("id100", [100, 100], BF16, kind="ExternalInput").ap()
    uhTd = nc.dram_tensor("uhT", [100, 800], BF16, kind="ExternalInput").ap()
    umTd = nc.dram_tensor("umT", [100, 800], BF16, kind="ExternalInput").ap()
    b1rowd = nc.dram_tensor("b1row", [1, 100], F32, kind="ExternalInput").ap()
    w2d = nc.dram_tensor("w2", [100, 1], BF16, kind="ExternalInput").ap()
    b2d = nc.dram_tensor("b2", [128, 1], F32, kind="ExternalInput").ap()
    selTd = nc.dram_tensor("selT", [2, 128, 32], F32, kind="ExternalInput").ap()
    grid = nc.dram_tensor("grid", [32, N], F32, kind="ExternalOutput").ap()

    from contextlib import ExitStack
    with TileContext(nc) as tc, ExitStack() as ctx:
        top = ctx.enter_context(tc.tile_pool(name="top", bufs=1))
        # ---- persistent SBUF tiles (DMAs emitted in priority order) ----
        idn = top.tile([128, 128], F32, name="idn", tag="idn")
        make_identity(nc, idn[:, :])
        ones_sb = top.tile([1, 256], F32, name="ones", tag="ones")
        nc.gpsimd.memset(ones_sb[:, :], 1.0)
        whh_sb = [top.tile([100, 6400], BF16, name=f"whh{dl}", tag=f"whh{dl}")
                  for dl in range(4)]
        bias_sb = [top.tile([100, 512], F32, name=f"bias{dl}", tag=f"bias{dl}")
                   for dl in range(4)]
        id_sb = top.tile([100, 100], BF16, name="id100", tag="id100")
        wih1_sb = [top.tile([100, 12800], BF16, name=f"wih1{d}", tag=f"wih1{d}")
                   for d in range(2)]
        xg_sbs = [top.tile([100, 4096], BF16, name=f"xg{d}", tag=f"xg{d}")
                  for d in range(2)]
        H = [[top.tile([100, 4 * N], BF16, name=f"H{l}{d}", tag=f"H{l}{d}")
              for d in range(2)] for l in range(2)]
        xT = top.tile([100, 4 * N], BF16, name="xT", tag="xT")
        wih0_sb = [top.tile([100, 6400], BF16, name=f"wih0{d}",
                            tag=f"wih0{d}") for d in range(2)]
        if STEPS < N:
            for l in range(2):
                for d in range(2):
                    nc.gpsimd.memset(H[l][d][:, :], 0.0)

        # ========= embedding gather (first DMAs in the queue) =========
        with tc.tile_pool(name="wih0p", bufs=1) as w0p, \
             tc.tile_pool(name="embps", bufs=2, space="PSUM") as eps:
            idx_sb = w0p.tile([128, 4], I32, name="idx", tag="idx")
            nc.sync.dma_start(out=idx_sb[0:128, 0:1], in_=widx[0:128, 0:1])
            nc.sync.dma_start(out=idx_sb[0:128, 1:2], in_=widx[128:256, 0:1])
            nc.sync.dma_start(out=idx_sb[0:128, 2:3], in_=pidx[0:128, 0:1])
            nc.sync.dma_start(out=idx_sb[0:128, 3:4], in_=pidx[128:256, 0:1])
            x_sb = w0p.tile([128, 800], F32, name="xsb", tag="xsb")
            for cch in range(2):
                nc.gpsimd.indirect_dma_start(
                    out=x_sb[0:128, 400 * cch: 400 * cch + 300],
                    out_offset=None,
                    in_=wemb[:, :],
                    in_offset=IndirectOffsetOnAxis(
                        ap=idx_sb[0:128, cch:cch + 1], axis=0))
                nc.gpsimd.indirect_dma_start(
                    out=x_sb[0:128, 400 * cch + 300: 400 * cch + 400],
                    out_offset=None,
                    in_=pemb[:, :],
                    in_offset=IndirectOffsetOnAxis(
                        ap=idx_sb[0:128, 2 + cch:3 + cch], axis=0))
            # layer-0 weights + rec0 needs, in DMA-queue priority order
            for d in range(2):
                nc.sync.dma_start(out=wih0_sb[d][:, :], in_=wih0Td[d])
            for dl in range(2):
                nc.sync.dma_start(out=bias_sb[dl][:, :], in_=biasTd[dl])
            nc.sync.dma_start(out=id_sb[:, :], in_=id100d[:, :])
            for dl in range(2):
                nc.sync.dma_start(out=whh_sb[dl][:, :], in_=whhTd[dl])

            # x -> xT transpose
            for cch in range(2):
                for uc in range(4):
                    ptr = eps.tile([128, 128], F32, name="ptr", tag="ptr")
                    nc.tensor.transpose(
                        out=ptr[0:100, 0:128],
                        in_=x_sb[0:128, 400 * cch + 100 * uc:
                                 400 * cch + 100 * uc + 100],
                        identity=idn[:, :])
                    nc.vector.tensor_copy(
                        out=xT[0:100, 256 * uc + 128 * cch:
                               256 * uc + 128 * cch + 128],
                        in_=ptr[0:100, 0:128])

            # ========= layer 0 xg: only the chunks needed at rec0 start
            # (rest are interleaved into rec0's idle PE time) =========
            def rhs_l0(d, uc, tc):
                return xT[0:P, 256 * uc + 32 * tc: 256 * uc + 32 * tc + 32]

            with tc.tile_pool(name="xg0ps", bufs=2, space="PSUM") as xg_ps:
                _emit_xg_group(nc, 4, [0, 1, 2, 3], wih0_sb[0], rhs_l0,
                               0, 0, xg_sbs[0], bias_sb[0], xg_ps, "xgps")
                _emit_xg_group(nc, 4, [0, 1, 2, 3], wih0_sb[1], rhs_l0,
                               1, 7, xg_sbs[1], bias_sb[1], xg_ps, "xgps")

        # remaining big DMAs: execute during rec0
        for d in range(2):
            nc.sync.dma_start(out=wih1_sb[d][:, :], in_=wih1Td[d])
        for dl in range(2, 4):
            nc.sync.dma_start(out=whh_sb[dl][:, :], in_=whhTd[dl])
            nc.sync.dma_start(out=bias_sb[dl][:, :], in_=biasTd[dl])

        # ========= rec0 with layer-1 xg interleaved =========
        def rhs_l1(d, uc, tc):
            src = H[0][uc // 4]
            j = uc % 4
            return src[0:P, 128 * tc + j: 128 * tc + 128: 4]

        with tc.tile_pool(name="xg1buf", bufs=1) as xgbuf, \
             tc.tile_pool(name="xg1ps", bufs=2, space="PSUM") as xg1_ps:
            xgf_sbs = [xgbuf.tile([100, 4096], BF16, name=f"xgf{d}",
                                  tag=f"xgf{d}") for d in range(2)]
            xgb_sbs = [xgbuf.tile([100, 4096], BF16, name=f"xgb{d}",
                                  tag=f"xgb{d}") for d in range(2)]

            def mk_group(d, tc_, half):
                def emit():
                    _emit_xg_group(
                        nc, 8, list(range(4 * half, 4 * half + 4)),
                        wih1_sb[d], rhs_l1, d, tc_,
                        xgf_sbs[d] if half == 0 else xgb_sbs[d],
                        bias_sb[2 + d] if half == 0 else None,
                        xg1_ps, "xg1ps")
                return emit

            def mk_group0(d, tc_):
                def emit():
                    _emit_xg_group(nc, 4, [0, 1, 2, 3], wih0_sb[d], rhs_l0,
                                   d, tc_, xg_sbs[d], bias_sb[d], xg1_ps,
                                   "xg1ps")
                return emit

            # xg0 leftovers first (avail immediately), ordered by deadline:
            # fwd chunk tc needed by wall step 32*tc, bwd chunk by 224-32*tc
            pend = []
            for tc_ in range(1, 8):
                pend.append((0, mk_group0(0, tc_)))        # deadline 32*tc_
                pend.append((0, mk_group0(1, 7 - tc_)))    # same deadline
            for d in range(2):
                for tc_ in range(8):
                    pend.append((32 * tc_ + 32, mk_group(d, tc_, 0)))
                    pend.append((N - 32 * tc_, mk_group(d, tc_, 1)))
            pend.sort(key=lambda x: x[0])
            # end-gated groups: rec1 needs xgb[0]c0 / xgf[1]c7 at its step 0,
            # but xgf[0]c7 / xgb[1]c0 only by step ~224 -- emit those two
            # inside rec1's idle time instead
            pend = [e for e in pend if e[0] < N]
            urgent = [mk_group(1, 7, 0), mk_group(0, 0, 1)]
            late = [mk_group(0, 7, 0), mk_group(1, 0, 1)]
            st = {"i": 0, "last": -10}

            def extra(t):
                gap = 1 if st["i"] < 14 else 2
                if (st["i"] < len(pend) and pend[st["i"]][0] <= t
                        and t - st["last"] >= gap):
                    pend[st["i"]][1]()
                    st["i"] += 1
                    st["last"] = t

            with tc.tile_pool(name="rst0", bufs=1) as state_pool, \
                 tc.tile_pool(name="sg0", bufs=3) as sg_pool, \
                 tc.tile_pool(name="tmp0", bufs=3) as tmp_pool, \
                 tc.tile_pool(name="rec0ps", bufs=3, space="PSUM") as rec_ps:
                _emit_rec(nc, 0, whh_sb[0:2], xg_sbs, id_sb, H[0],
                          (state_pool, sg_pool, tmp_pool, rec_ps),
                          extra=extra)
            # leftover in-rec0 groups, then the two urgently needed ones
            while st["i"] < len(pend):
                pend[st["i"]][1]()
                st["i"] += 1
            for fn in urgent:
                fn()

            st1 = {"i": 0, "last": -10}

            def extra1(t):
                if st1["i"] < len(late) and t - st1["last"] >= 2:
                    late[st1["i"]]()
                    st1["i"] += 1
                    st1["last"] = t

            # ========= rec1 (dual injection: fwd + bwd halves) =========
            with tc.tile_pool(name="rst1", bufs=1) as state_pool, \
                 tc.tile_pool(name="sg1", bufs=3) as sg_pool, \
                 tc.tile_pool(name="tmp1", bufs=3) as tmp_pool, \
                 tc.tile_pool(name="rec1ps", bufs=3, space="PSUM") as rec_ps:
                _emit_rec(nc, 1, whh_sb[2:4], xgf_sbs, id_sb, H[1],
                          (state_pool, sg_pool, tmp_pool, rec_ps),
                          xg2_sbs=xgb_sbs, extra=extra1)
            while st1["i"] < len(late):
                late[st1["i"]]()
                st1["i"] += 1

        # ========= edge scorer =========
        with tc.tile_pool(name="edge", bufs=1) as ep, \
             tc.tile_pool(name="edgeth", bufs=4) as thp, \
             tc.tile_pool(name="edgeps", bufs=1, space="PSUM") as epps, \
             tc.tile_pool(name="edgepsS", bufs=1, space="PSUM") as spps:
            uh_sb = ep.tile([100, 800], BF16, name="uhT", tag="uhT")
            nc.sync.dma_start(out=uh_sb[:, :], in_=uhTd[:, :])
            um_sb = ep.tile([100, 800], BF16, name="umT", tag="umT")
            nc.sync.dma_start(out=um_sb[:, :], in_=umTd[:, :])
            b1_sb = ep.tile([1, 100], F32, name="b1row", tag="b1row")
            nc.sync.dma_start(out=b1_sb[:, :], in_=b1rowd[:, :])
            w2_sb = ep.tile([100, 1], BF16, name="w2", tag="w2")
            nc.sync.dma_start(out=w2_sb[:, :], in_=w2d[:, :])
            b2_sb = ep.tile([128, 1], F32, name="b2", tag="b2")
            nc.sync.dma_start(out=b2_sb[:, :], in_=b2d[:, :])
            selT_sb = ep.tile([128, 64], F32, name="selT", tag="selT")
            nc.sync.dma_start(out=selT_sb[0:128, 0:32], in_=selTd[0])
            nc.sync.dma_start(out=selT_sb[0:128, 32:64], in_=selTd[1])

            def h1_rhs(uc):
                return H[1][uc // 4][0:P, uc % 4: 4 * N: 4]

            # A^T [100, 256] (head half of fc1)
            pA = epps.tile([128, 512], F32, name="e1", tag="e1")
            for uc in range(8):
                nc.tensor.matmul(
                    pA[0:P, 0:256],
                    lhsT=uh_sb[0:P, 100 * uc: 100 * uc + 100],
                    rhs=h1_rhs(uc),
                    start=(uc == 0), stop=(uc == 7), skip_group_check=True)
            A_sb = ep.tile([100, 256], F32, name="A", tag="A")
            nc.vector.tensor_copy(out=A_sb[0:P, 0:256], in_=pA[0:P, 0:256])
            # B^T [100, 256] + b1 (modifier half)
            pB = epps.tile([128, 512], F32, name="e3", tag="e3")
            for uc in range(8):
                nc.tensor.matmul(
                    pB[0:P, 0:256],
                    lhsT=um_sb[0:P, 100 * uc: 100 * uc + 100],
                    rhs=h1_rhs(uc),
                    start=(uc == 0), stop=False, skip_group_check=True)
            nc.tensor.matmul(
                pB[0:P, 0:256],
                lhsT=b1_sb[0:1, 0:100],
                rhs=ones_sb[0:1, 0:256],
                start=False, stop=True, skip_group_check=True)
            B_sb = ep.tile([100, 256], BF16, name="Bsb", tag="Bsb")
            nc.vector.tensor_copy(out=B_sb[0:P, 0:256], in_=pB[0:P, 0:256])
            # A -> token-major via transpose, then per-core 32-head select
            A_tok = ep.tile([128, 256], F32, name="Atok", tag="Atok")
            for m in range(2):
                pT = epps.tile([128, 512], F32, name="e2", tag="e2")
                nc.tensor.transpose(
                    out=pT[0:128, 0:100],
                    in_=A_sb[0:100, 128 * m: 128 * m + 128],
                    identity=idn[0:100, 0:100])
                nc.vector.tensor_copy(
                    out=A_tok[0:128, 128 * m: 128 * m + 100],
                    in_=pT[0:128, 0:100])
            pS = epps.tile([128, 512], F32, name="e1", tag="e1")
            for m in range(2):
                nc.tensor.matmul(
                    pS[0:32, 0:100],
                    lhsT=selT_sb[0:128, 32 * m: 32 * m + 32],
                    rhs=A_tok[0:128, 128 * m: 128 * m + 100],
                    start=(m == 0), stop=(m == 1), skip_group_check=True)
            AselS = ep.tile([128, 128], F32, name="AselS", tag="AselS")
            nc.gpsimd.memset(AselS[:, :], 0.0)
            nc.vector.tensor_copy(out=AselS[0:32, 0:100], in_=pS[0:32, 0:100])
            pAT = epps.tile([128, 512], F32, name="e2", tag="e2")
            nc.tensor.transpose(out=pAT[0:128, 0:128],
                                in_=AselS[0:128, 0:128], identity=idn[:, :])
            AT_sb = ep.tile([128, 32], F32, name="AT", tag="AT")
            nc.vector.tensor_copy(out=AT_sb[0:128, 0:32], in_=pAT[0:128, 0:32])

            # per-head tanh + w2 dot
            psS_tiles = [spps.tile([128, 512], F32, name=f"psS{q}", tag=f"psS{q}")
                         for q in range(4)]
            for q in range(4):
                nc.vector.memset(psS_tiles[q][:, :], 0.0)
            gsb_tiles = [ep.tile([128, 512], F32, name=f"gsb{q}", tag=f"gsb{q}")
                         for q in range(4)]
            for r in range(32):
                th_t = thp.tile([100, 256], BF16, name=f"th{r % 4}",
                                tag=f"th{r % 4}")
                nc.scalar.activation(
                    th_t[0:100, 0:256], B_sb[0:100, 0:256], AF.Tanh,
                    bias=AT_sb[0:100, r:r + 1], scale=1.0)
                q, half = divmod(r // 4, 2)
                nc.tensor.matmul(
                    psS_tiles[q][32 * (r % 4): 32 * (r % 4) + 1,
                                 256 * half: 256 * half + 256],
                    lhsT=w2_sb[0:100, 0:1],
                    rhs=th_t[0:100, 0:256],
                    start=True, stop=True,
                    skip_group_check=True,
                    tile_position=(0, 32 * (r % 4)))
            for q in range(4):
                nc.vector.tensor_scalar(
                    out=gsb_tiles[q][0:128, 0:512],
                    in0=psS_tiles[q][0:128, 0:512],
                    scalar1=b2_sb[0:128, 0:1], scalar2=None, op0=OP.add)
                for half in range(2):
                    rb = 4 * (2 * q + half)
                    nc.sync.dma_start(
                        out=grid[rb:rb + 4, 0:256],
                        in_=gsb_tiles[q][0:128:32, 256 * half: 256 * half + 256])

    nc.compile()
    return nc


_NC_CACHE = None


def _get_nc():
    global _NC_CACHE
    if _NC_CACHE is None:
        _NC_CACHE = build_nc()
    return _NC_CACHE


def kernel(**inputs) -> np.ndarray:
    from concourse.bass_utils import run_bass_kernel_spmd

    arr = _prep_inputs(**inputs)
    nc = _get_nc()
    in_maps = []
    for k in range(NC):
        m = dict(arr)
        m["selT"] = _make_selT(k)
        in_maps.append(m)
    res = run_bass_kernel_spmd(nc, in_maps, core_ids=list(range(NC)))
    grid = np.concatenate([res.results[k]["grid"] for k in range(NC)], axis=0)
    mask = np.ones((N, N), dtype=bool)
    np.fill_diagonal(mask, False)
    mask[:, 0] = False
    return grid[mask].reshape(-1, 1).astype(np.float32)


# revision 31
# speedup vs baseline: 3.1544x; 1.0066x over previous
"""Trainium2 Bass kernel: BiLSTM dependency-parser edge scorer (v2).

Self-contained. Accepts FULL inputs (as produced by setup_inputs()), returns
the FULL [65280, 1] float32 score tensor.

Key idea vs v1: all recurrence matmuls are WEIGHTS-STATIONARY (weights in
lhsT, the tiny h vector streams as rhs), so each step's 64 gate matmuls have
output free-size 1 instead of streaming 6400 PSUM rows.

Layouts (per direction d, layer l):
  gates PSUM tile [100, 16]: partition p, col n = 4*j + g where the LSTM
    unit is u = 100*j + p (j in 0..4) and g in {0:i, 1:f, 2:g, 3:o}.
  h storage H[l][d] [100, 4*256] bf16: h_t for unit (j, p) at col 4*t + j.
    Column 4*t+j is directly the rhs [100, 1] for K-chunk j of the next
    step's matmul -- no transpose inside the loop.
  c state [100, 4] f32.
  xg_sb[d] [100, 16*256] bf16: precomputed input projections + bias,
    injected into the PSUM accumulation via an identity-weight matmul.
g-gate rows are pre-scaled by 2 on host: tanh(x) = 2*sigmoid(2x) - 1.
"""

import os
import sys

sys.path.insert(0, "/opt/trn_rl_repo")

import numpy as np

import concourse.bass as bass
import concourse.mybir as mybir
from concourse import bacc
from concourse.bass import IndirectOffsetOnAxis
from concourse.masks import make_identity
from concourse.tile import TileContext

N = 256          # sequence length
HID = 400        # hidden per direction
NC = 8           # cores
P = 100          # partitions used for unit math
NG = 16          # gate cols per step
F32 = mybir.dt.float32
BF16 = mybir.dt.float16
I32 = mybir.dt.int32
AF = mybir.ActivationFunctionType
OP = mybir.AluOpType

STEPS = int(os.environ.get("DP_STEPS", str(N)))


# ---------------------------------------------------------------------------
# host-side weight layout prep
# ---------------------------------------------------------------------------

def _bf(a):
    return np.ascontiguousarray(np.asarray(a).astype(np.float16))


# R[p, n] = original torch gate-row for (partition p, col n)
_PP, _NN = np.meshgrid(np.arange(P), np.arange(NG), indexing="ij")
_R = 400 * (_NN % 4) + 100 * (_NN // 4) + _PP      # [100, 16]


def _scale_g(W):
    """Scale g-gate rows (orig rows 800:1200) by 2."""
    Ws = np.array(W, dtype=np.float64)
    Ws[800:1200] *= 2.0
    return Ws


def _wblocks(W, nuc):
    """W: [1600, U] scaled gate-major weights, U = 100*nuc.
    Returns [100, 16*nuc*100]: block (n, uc) at cols (n*nuc+uc)*100 holds
    lhsT[k, m] = W[R[m, n], 100*uc + k]."""
    arr = W[_R]                                    # [100p, 16n, U]
    A4 = arr.reshape(P, NG, nuc, 100)              # [p, n, uc, k]
    return A4.transpose(3, 1, 2, 0).reshape(100, NG * nuc * 100)


def _prep_inputs(word_idx, pos_idx, word_emb, pos_emb,
                 Wih0, Whh0, bih0, bhh0, Wih1, Whh1, bih1, bhh1,
                 fc1_W, fc1_b, fc2_W, fc2_b):
    arr = {}
    arr["widx"] = np.ascontiguousarray(
        np.asarray(word_idx).reshape(N, 1).astype(np.int32))
    arr["pidx"] = np.ascontiguousarray(
        np.asarray(pos_idx).reshape(N, 1).astype(np.int32))
    arr["wemb"] = np.ascontiguousarray(np.asarray(word_emb, dtype=np.float32))
    arr["pemb"] = np.ascontiguousarray(np.asarray(pos_emb, dtype=np.float32))

    Wih = [np.asarray(Wih0, np.float64), np.asarray(Wih1, np.float64)]
    Whh = [np.asarray(Whh0, np.float64), np.asarray(Whh1, np.float64)]
    bih = [np.asarray(bih0, np.float64), np.asarray(bih1, np.float64)]
    bhh = [np.asarray(bhh0, np.float64), np.asarray(bhh1, np.float64)]

    whhT = np.zeros((4, 100, NG * 4 * 100), np.float32)
    biasT = np.zeros((4, 100, 512), np.float32)
    wih0T = np.zeros((2, 100, NG * 4 * 100), np.float32)
    wih1T = np.zeros((2, 100, NG * 8 * 100), np.float32)
    for l in range(2):
        for d in range(2):
            dl = 2 * l + d
            whhT[dl] = _wblocks(_scale_g(Whh[l][d]), 4)
            b = _scale_g(bih[l][d] + bhh[l][d])[_R]          # [100, 16]
            biasT[dl] = np.tile(b, (1, 32)).astype(np.float32)
    for d in range(2):
        wih0T[d] = _wblocks(_scale_g(Wih[0][d]), 4)
        wih1T[d] = _wblocks(_scale_g(Wih[1][d]), 8)
    arr["whhT"] = _bf(whhT)
    arr["biasT"] = np.ascontiguousarray(biasT)
    arr["wih0T"] = _bf(wih0T)
    arr["wih1T"] = _bf(wih1T)

    # identity for the xg injection matmul
    arr["id100"] = _bf(np.eye(P, dtype=np.float32))

    # edge MLP: uhT/umT [100, 800]: block uc at cols 100*uc holds
    # lhsT[k, a] = fc1_W[a, 100*uc + k]
    f1 = np.asarray(fc1_W, np.float64)               # [100, 1600]
    arr["uhT"] = _bf(np.concatenate(
        [f1[:, 100 * u:100 * u + 100].T for u in range(8)], axis=1))
    arr["umT"] = _bf(np.concatenate(
        [f1[:, 800 + 100 * u:800 + 100 * u + 100].T for u in range(8)],
        axis=1))
    arr["b1row"] = np.ascontiguousarray(
        np.asarray(fc1_b, np.float32).reshape(1, 100))
    arr["w2"] = _bf(np.asarray(fc2_W, np.float32).reshape(100, 1))
    arr["b2"] = np.ascontiguousarray(
        np.full((128, 1), np.float32(np.asarray(fc2_b).reshape(())),
                dtype=np.float32))
    return arr


def _make_selT(core):
    s = np.zeros((2, 128, 32), np.float32)
    for r in range(32):
        t = 32 * core + r
        s[t // 128, t % 128, r] = 1.0
    return np.ascontiguousarray(s)


# ---------------------------------------------------------------------------
# device kernel build
# ---------------------------------------------------------------------------

def _emit_xg_group(nc, nuc, ucs, wih_sb_d, rhs_chunk, d, tc, dst_sb,
                   bias_sb, ps_pool, tag):
    """One t-chunk (32 tokens) of an input-projection GEMM: 16*len(ucs)
    weights-stationary matmuls accumulating into a PSUM bank, then one
    PSUM->SBUF copy (adding bias if given)."""
    ps = ps_pool.tile([128, 512], F32, name=tag, tag=tag)
    for n in range(NG):
        for i, uc in enumerate(ucs):
            nc.tensor.matmul(
                ps[0:P, n:512:16],
                lhsT=wih_sb_d[0:P, (n * nuc + uc) * 100:
                              (n * nuc + uc) * 100 + 100],
                rhs=rhs_chunk(d, uc, tc),
                start=(i == 0), stop=(i == len(ucs) - 1),
                skip_group_check=True)
    if bias_sb is not None:
        nc.vector.tensor_tensor(
            out=dst_sb[0:P, 512 * tc: 512 * tc + 512],
            in0=ps[0:P, 0:512], in1=bias_sb[0:P, 0:512], op=OP.add)
    else:
        nc.vector.tensor_copy(
            out=dst_sb[0:P, 512 * tc: 512 * tc + 512],
            in_=ps[0:P, 0:512])


def _emit_xg(nc, l, wih_sb, rhs_chunk, xg_sbs, bias_sbs, ps_pool):
    """Full xg for layer l: xg[d][p, 16*t + n] = sum_u W[r(p,n), u]*in[t,u]+b."""
    nuc = 4 if l == 0 else 8
    for d in range(2):
        for tc in range(8):
            _emit_xg_group(nc, nuc, list(range(nuc)), wih_sb[d], rhs_chunk,
                           d, tc, xg_sbs[d], bias_sbs[d], ps_pool, "xgps")


def _emit_rec(nc, l, whh_sb, xg_sbs, id_sb, H_out, pools, xg2_sbs=None,
              extra=None):
    """STEPS wall-steps, both directions interleaved. xg2_sbs: optional
    second injection source (bwd-half input projections for layer 1).
    extra(t): called after each wall-step to emit overlapped work."""
    state_pool, sg_pool, tmp_pool, ps_pool = pools
    cs = []
    for d in range(2):
        c = state_pool.tile([P, 4], F32, name=f"c{d}", tag=f"c{d}")
        nc.gpsimd.memset(c[:, :], 0.0)
        cs.append(c)

    for t in range(STEPS):
        ps_t, sg_t, th_t = {}, {}, {}
        dorder = (0, 1)
        # --- PE: injection + 64 weight matmuls per direction ---
        for d in dorder:
            tdx = t if d == 0 else (STEPS - 1 - t)
            ps = ps_pool.tile([128, 512], F32, name=f"ps{d}", tag=f"ps{d}")
            ps_t[d] = ps
            first = (t == 0)
            nc.tensor.matmul(
                ps[0:P, 0:NG],
                lhsT=id_sb[0:P, 0:P],
                rhs=xg_sbs[d][0:P, NG * tdx: NG * tdx + NG],
                start=True, stop=(first and xg2_sbs is None),
                skip_group_check=True)
            if xg2_sbs is not None:
                nc.tensor.matmul(
                    ps[0:P, 0:NG],
                    lhsT=id_sb[0:P, 0:P],
                    rhs=xg2_sbs[d][0:P, NG * tdx: NG * tdx + NG],
                    start=False, stop=first, skip_group_check=True)
            if not first:
                pdx = tdx - 1 if d == 0 else tdx + 1
                # even cols (i, g gates) first: they gate the c-update chain
                for n in list(range(0, NG, 2)) + list(range(1, NG, 2)):
                    for uc in range(4):
                        nc.tensor.matmul(
                            ps[0:P, n:n + 1],
                            lhsT=whh_sb[d][0:P, (4 * n + uc) * 100:
                                           (4 * n + uc) * 100 + 100],
                            rhs=H_out[d][0:P, 4 * pdx + uc: 4 * pdx + uc + 1],
                            start=False, stop=(uc == 3),
                            skip_group_check=True)
        # --- Act: sigmoid over all gates (g pre-scaled by 2) ---
        for d in dorder:
            sg = sg_pool.tile([P, NG], F32, name=f"sg{d}", tag=f"sg{d}")
            sg_t[d] = sg
            nc.scalar.activation(sg[0:P, 0:NG], ps_t[d][0:P, 0:NG], AF.Sigmoid)
        # --- DVE per direction: c = sig(f)*c + sig(i)*(2*sig(2g) - 1),
        #     then tanh(c) via Pade [3/2] (|c| < 0.5 here, err < 1e-6):
        #     tanh(c) ~= c*(15 + c^2) / (15 + 6*c^2);  h = sig(o)*tanh ---
        for d in dorder:
            tdx = t if d == 0 else (STEPS - 1 - t)
            sg, c = sg_t[d], cs[d]
            tg = tmp_pool.tile([P, 4], F32, name=f"tg{d}", tag=f"tg{d}")
            t1 = tmp_pool.tile([P, 4], F32, name=f"t1{d}", tag=f"t1{d}")
            cf = tmp_pool.tile([P, 4], F32, name=f"cf{d}", tag=f"cf{d}")
            nc.vector.tensor_scalar(
                out=tg[0:P, 0:4], in0=sg[0:P, 2:NG:4],
                scalar1=2.0, scalar2=-1.0, op0=OP.mult, op1=OP.add)
            nc.vector.tensor_tensor(
                out=cf[0:P, 0:4], in0=sg[0:P, 1:NG:4],
                in1=c[0:P, 0:4], op=OP.mult)
            nc.vector.tensor_tensor(
                out=t1[0:P, 0:4], in0=sg[0:P, 0:NG:4],
                in1=tg[0:P, 0:4], op=OP.mult)
            nc.vector.tensor_tensor(
                out=c[0:P, 0:4], in0=cf[0:P, 0:4],
                in1=t1[0:P, 0:4], op=OP.add)
            th = tmp_pool.tile([P, 4], F32, name=f"th{d}", tag=f"th{d}")
            th_t[d] = th
            nc.scalar.activation(th[0:P, 0:4], c[0:P, 0:4], AF.Tanh)
        for d in dorder:
            tdx = t if d == 0 else (STEPS - 1 - t)
            nc.vector.tensor_tensor(
                out=H_out[d][0:P, 4 * tdx: 4 * tdx + 4],
                in0=sg_t[d][0:P, 3:NG:4], in1=th_t[d][0:P, 0:4], op=OP.mult)
        if extra is not None:
            extra(t)


def build_nc():
    nc = bacc.Bacc("TRN2", target_bir_lowering=False, debug=False,
                   num_devices=NC)
    wemb = nc.dram_tensor("wemb", [50000, 300], F32, kind="ExternalInput").ap()
    pemb = nc.dram_tensor("pemb", [50, 100], F32, kind="ExternalInput").ap()
    widx = nc.dram_tensor("widx", [N, 1], I32, kind="ExternalInput").ap()
    pidx = nc.dram_tensor("pidx", [N, 1], I32, kind="ExternalInput").ap()
    whhTd = nc.dram_tensor("whhT", [4, 100, 6400], BF16, kind="ExternalInput").ap()
    wih0Td = nc.dram_tensor("wih0T", [2, 100, 6400], BF16, kind="ExternalInput").ap()
    wih1Td = nc.dram_tensor("wih1T", [2, 100, 12800], BF16, kind="ExternalInput").ap()
    biasTd = nc.dram_tensor("biasT", [4, 100, 512], F32, kind="ExternalInput").ap()
    id100d = nc.dram_tensor("id100", [100, 100], BF16, kind="ExternalInput").ap()
    uhTd = nc.dram_tensor("uhT", [100, 800], BF16, kind="ExternalInput").ap()
    umTd = nc.dram_tensor("umT", [100, 800], BF16, kind="ExternalInput").ap()
    b1rowd = nc.dram_tensor("b1row", [1, 100], F32, kind="ExternalInput").ap()
    w2d = nc.dram_tensor("w2", [100, 1], BF16, kind="ExternalInput").ap()
    b2d = nc.dram_tensor("b2", [128, 1], F32, kind="ExternalInput").ap()
    selTd = nc.dram_tensor("selT", [2, 128, 32], F32, kind="ExternalInput").ap()
    grid = nc.dram_tensor("grid", [32, N], F32, kind="ExternalOutput").ap()

    from contextlib import ExitStack
    with TileContext(nc) as tc, ExitStack() as ctx:
        top = ctx.enter_context(tc.tile_pool(name="top", bufs=1))
        # ---- persistent SBUF tiles (DMAs emitted in priority order) ----
        idn = top.tile([128, 128], F32, name="idn", tag="idn")
        make_identity(nc, idn[:, :])
        ones_sb = top.tile([1, 256], F32, name="ones", tag="ones")
        nc.gpsimd.memset(ones_sb[:, :], 1.0)
        whh_sb = [top.tile([100, 6400], BF16, name=f"whh{dl}", tag=f"whh{dl}")
                  for dl in range(4)]
        bias_sb = [top.tile([100, 512], F32, name=f"bias{dl}", tag=f"bias{dl}")
                   for dl in range(4)]
        id_sb = top.tile([100, 100], BF16, name="id100", tag="id100")
        wih1_sb = [top.tile([100, 12800], BF16, name=f"wih1{d}", tag=f"wih1{d}")
                   for d in range(2)]
        xg_sbs = [top.tile([100, 4096], BF16, name=f"xg{d}", tag=f"xg{d}")
                  for d in range(2)]
        H = [[top.tile([100, 4 * N], BF16, name=f"H{l}{d}", tag=f"H{l}{d}")
              for d in range(2)] for l in range(2)]
        xT = top.tile([100, 4 * N], BF16, name="xT", tag="xT")
        wih0_sb = [top.tile([100, 6400], BF16, name=f"wih0{d}",
                            tag=f"wih0{d}") for d in range(2)]
        if STEPS < N:
            for l in range(2):
                for d in range(2):
                    nc.gpsimd.memset(H[l][d][:, :], 0.0)

        # ========= embedding gather (first DMAs in the queue) =========
        with tc.tile_pool(name="wih0p", bufs=1) as w0p, \
             tc.tile_pool(name="embps", bufs=2, space="PSUM") as eps:
            idx_sb = w0p.tile([128, 4], I32, name="idx", tag="idx")
            nc.sync.dma_start(out=idx_sb[0:128, 0:1], in_=widx[0:128, 0:1])
            nc.sync.dma_start(out=idx_sb[0:128, 1:2], in_=widx[128:256, 0:1])
            nc.sync.dma_start(out=idx_sb[0:128, 2:3], in_=pidx[0:128, 0:1])
            nc.sync.dma_start(out=idx_sb[0:128, 3:4], in_=pidx[128:256, 0:1])
            x_sb = w0p.tile([128, 800], F32, name="xsb", tag="xsb")
            for cch in range(2):
                nc.gpsimd.indirect_dma_start(
                    out=x_sb[0:128, 400 * cch: 400 * cch + 300],
                    out_offset=None,
                    in_=wemb[:, :],
                    in_offset=IndirectOffsetOnAxis(
                        ap=idx_sb[0:128, cch:cch + 1], axis=0))
                nc.gpsimd.indirect_dma_start(
                    out=x_sb[0:128, 400 * cch + 300: 400 * cch + 400],
                    out_offset=None,
                    in_=pemb[:, :],
                    in_offset=IndirectOffsetOnAxis(
                        ap=idx_sb[0:128, 2 + cch:3 + cch], axis=0))
            # layer-0 weights + rec0 needs, in DMA-queue priority order
            nc.sync.dma_start(out=bias_sb[0][:, :], in_=biasTd[0])
            nc.sync.dma_start(out=id_sb[:, :], in_=id100d[:, :])
            nc.sync.dma_start(out=wih0_sb[0][:, :], in_=wih0Td[0])
            nc.sync.dma_start(out=whh_sb[0][:, :], in_=whhTd[0])
            nc.sync.dma_start(out=bias_sb[1][:, :], in_=biasTd[1])
            nc.sync.dma_start(out=wih0_sb[1][:, :], in_=wih0Td[1])
            nc.sync.dma_start(out=whh_sb[1][:, :], in_=whhTd[1])

            # x -> xT transpose
            for cch in range(2):
                for uc in range(4):
                    ptr = eps.tile([128, 128], F32, name="ptr", tag="ptr")
                    nc.tensor.transpose(
                        out=ptr[0:100, 0:128],
                        in_=x_sb[0:128, 400 * cch + 100 * uc:
                                 400 * cch + 100 * uc + 100],
                        identity=idn[:, :])
                    nc.vector.tensor_copy(
                        out=xT[0:100, 256 * uc + 128 * cch:
                               256 * uc + 128 * cch + 128],
                        in_=ptr[0:100, 0:128])

            # ========= layer 0 xg: only the chunks needed at rec0 start
            # (rest are interleaved into rec0's idle PE time) =========
            def rhs_l0(d, uc, tc):
                return xT[0:P, 256 * uc + 32 * tc: 256 * uc + 32 * tc + 32]

            with tc.tile_pool(name="xg0ps", bufs=2, space="PSUM") as xg_ps:
                _emit_xg_group(nc, 4, [0, 1, 2, 3], wih0_sb[0], rhs_l0,
                               0, 0, xg_sbs[0], bias_sb[0], xg_ps, "xgps")
                _emit_xg_group(nc, 4, [0, 1, 2, 3], wih0_sb[1], rhs_l0,
                               1, 7, xg_sbs[1], bias_sb[1], xg_ps, "xgps")

        # remaining big DMAs: execute during rec0
        for d in range(2):
            nc.sync.dma_start(out=wih1_sb[d][:, :], in_=wih1Td[d])
        for dl in range(2, 4):
            nc.sync.dma_start(out=whh_sb[dl][:, :], in_=whhTd[dl])
            nc.sync.dma_start(out=bias_sb[dl][:, :], in_=biasTd[dl])

        # ========= rec0 with layer-1 xg interleaved =========
        def rhs_l1(d, uc, tc):
            src = H[0][uc // 4]
            j = uc % 4
            return src[0:P, 128 * tc + j: 128 * tc + 128: 4]

        with tc.tile_pool(name="xg1buf", bufs=1) as xgbuf, \
             tc.tile_pool(name="xg1ps", bufs=2, space="PSUM") as xg1_ps:
            xgf_sbs = [xgbuf.tile([100, 4096], BF16, name=f"xgf{d}",
                                  tag=f"xgf{d}") for d in range(2)]
            xgb_sbs = [xgbuf.tile([100, 4096], BF16, name=f"xgb{d}",
                                  tag=f"xgb{d}") for d in range(2)]

            def mk_group(d, tc_, half):
                def emit():
                    _emit_xg_group(
                        nc, 8, list(range(4 * half, 4 * half + 4)),
                        wih1_sb[d], rhs_l1, d, tc_,
                        xgf_sbs[d] if half == 0 else xgb_sbs[d],
                        bias_sb[2 + d] if half == 0 else None,
                        xg1_ps, "xg1ps")
                return emit

            def mk_group0(d, tc_):
                def emit():
                    _emit_xg_group(nc, 4, [0, 1, 2, 3], wih0_sb[d], rhs_l0,
                                   d, tc_, xg_sbs[d], bias_sb[d], xg1_ps,
                                   "xg1ps")
                return emit

            # xg0 leftovers first (avail immediately), ordered by deadline:
            # fwd chunk tc needed by wall step 32*tc, bwd chunk by 224-32*tc
            pend = []
            for tc_ in range(1, 8):
                pend.append((0, mk_group0(0, tc_)))        # deadline 32*tc_
                pend.append((0, mk_group0(1, 7 - tc_)))    # same deadline
            for d in range(2):
                for tc_ in range(8):
                    pend.append((32 * tc_ + 32, mk_group(d, tc_, 0)))
                    pend.append((N - 32 * tc_, mk_group(d, tc_, 1)))
            pend.sort(key=lambda x: x[0])
            # end-gated groups: rec1 needs xgb[0]c0 / xgf[1]c7 at its step 0,
            # but xgf[0]c7 / xgb[1]c0 only by step ~224 -- emit those two
            # inside rec1's idle time instead
            pend = [e for e in pend if e[0] < N]
            urgent = [mk_group(1, 7, 0), mk_group(0, 0, 1)]
            late = [mk_group(0, 7, 0), mk_group(1, 0, 1)]
            st = {"i": 0, "last": -10}

            def extra(t):
                n_emit = 0
                while (st["i"] < len(pend) and pend[st["i"]][0] <= t
                       and t > st["last"] and n_emit < 2):
                    pend[st["i"]][1]()
                    st["i"] += 1
                    n_emit += 1
                if n_emit:
                    st["last"] = t

            with tc.tile_pool(name="rst0", bufs=1) as state_pool, \
                 tc.tile_pool(name="sg0", bufs=4) as sg_pool, \
                 tc.tile_pool(name="tmp0", bufs=4) as tmp_pool, \
                 tc.tile_pool(name="rec0ps", bufs=3, space="PSUM") as rec_ps:
                _emit_rec(nc, 0, whh_sb[0:2], xg_sbs, id_sb, H[0],
                          (state_pool, sg_pool, tmp_pool, rec_ps),
                          extra=extra)
            # leftover in-rec0 groups, then the two urgently needed ones
            while st["i"] < len(pend):
                pend[st["i"]][1]()
                st["i"] += 1
            for fn in urgent:
                fn()

            st1 = {"i": 0, "last": -10}

            def extra1(t):
                if st1["i"] < len(late) and t - st1["last"] >= 2:
                    late[st1["i"]]()
                    st1["i"] += 1
                    st1["last"] = t

            # ========= rec1 (dual injection: fwd + bwd halves) =========
            with tc.tile_pool(name="rst1", bufs=1) as state_pool, \
                 tc.tile_pool(name="sg1", bufs=4) as sg_pool, \
                 tc.tile_pool(name="tmp1", bufs=4) as tmp_pool, \
                 tc.tile_pool(name="rec1ps", bufs=3, space="PSUM") as rec_ps:
                _emit_rec(nc, 1, whh_sb[2:4], xgf_sbs, id_sb, H[1],
                          (state_pool, sg_pool, tmp_pool, rec_ps),
                          xg2_sbs=xgb_sbs, extra=extra1)
            while st1["i"] < len(late):
                late[st1["i"]]()
                st1["i"] += 1

        # ========= edge scorer =========
        with tc.tile_pool(name="edge", bufs=1) as ep, \
             tc.tile_pool(name="edgeth", bufs=4) as thp, \
             tc.tile_pool(name="edgeps", bufs=1, space="PSUM") as epps, \
             tc.tile_pool(name="edgepsS", bufs=1, space="PSUM") as spps:
            uh_sb = ep.tile([100, 800], BF16, name="uhT", tag="uhT")
            nc.sync.dma_start(out=uh_sb[:, :], in_=uhTd[:, :])
            um_sb = ep.tile([100, 800], BF16, name="umT", tag="umT")
            nc.sync.dma_start(out=um_sb[:, :], in_=umTd[:, :])
            b1_sb = ep.tile([1, 100], F32, name="b1row", tag="b1row")
            nc.sync.dma_start(out=b1_sb[:, :], in_=b1rowd[:, :])
            w2_sb = ep.tile([100, 1], BF16, name="w2", tag="w2")
            nc.sync.dma_start(out=w2_sb[:, :], in_=w2d[:, :])
            b2_sb = ep.tile([128, 1], F32, name="b2", tag="b2")
            nc.sync.dma_start(out=b2_sb[:, :], in_=b2d[:, :])
            selT_sb = ep.tile([128, 64], F32, name="selT", tag="selT")
            nc.sync.dma_start(out=selT_sb[0:128, 0:32], in_=selTd[0])
            nc.sync.dma_start(out=selT_sb[0:128, 32:64], in_=selTd[1])

            def h1_rhs(uc):
                return H[1][uc // 4][0:P, uc % 4: 4 * N: 4]

            # A^T [100, 256] (head half of fc1)
            pA = epps.tile([128, 512], F32, name="e1", tag="e1")
            for uc in range(8):
                nc.tensor.matmul(
                    pA[0:P, 0:256],
                    lhsT=uh_sb[0:P, 100 * uc: 100 * uc + 100],
                    rhs=h1_rhs(uc),
                    start=(uc == 0), stop=(uc == 7), skip_group_check=True)
            A_sb = ep.tile([100, 256], F32, name="A", tag="A")
            nc.vector.tensor_copy(out=A_sb[0:P, 0:256], in_=pA[0:P, 0:256])
            # B^T [100, 256] + b1 (modifier half)
            pB = epps.tile([128, 512], F32, name="e3", tag="e3")
            for uc in range(8):
                nc.tensor.matmul(
                    pB[0:P, 0:256],
                    lhsT=um_sb[0:P, 100 * uc: 100 * uc + 100],
                    rhs=h1_rhs(uc),
                    start=(uc == 0), stop=False, skip_group_check=True)
            nc.tensor.matmul(
                pB[0:P, 0:256],
                lhsT=b1_sb[0:1, 0:100],
                rhs=ones_sb[0:1, 0:256],
                start=False, stop=True, skip_group_check=True)
            B_sb = ep.tile([100, 256], BF16, name="Bsb", tag="Bsb")
            nc.vector.tensor_copy(out=B_sb[0:P, 0:256], in_=pB[0:P, 0:256])
            # A -> token-major via transpose, then per-core 32-head select
            A_tok = ep.tile([128, 256], F32, name="Atok", tag="Atok")
            for m in range(2):
                pT = epps.tile([128, 512], F32, name="e2", tag="e2")
                nc.tensor.transpose(
                    out=pT[0:128, 0:100],
                    in_=A_sb[0:100, 128 * m: 128 * m + 128],
                    identity=idn[0:100, 0:100])
                nc.vector.tensor_copy(
                    out=A_tok[0:128, 128 * m: 128 * m + 100],
                    in_=pT[0:128, 0:100])
            pS = epps.tile([128, 512], F32, name="e1", tag="e1")
            for m in range(2):
                nc.tensor.matmul(
                    pS[0:32, 0:100],
                    lhsT=selT_sb[0:128, 32 * m: 32 * m + 32],
                    rhs=A_tok[0:128, 128 * m: 128 * m + 100],
                    start=(m == 0), stop=(m == 1), skip_group_check=True)
            AselS = ep.tile([128, 128], F32, name="AselS", tag="AselS")
            nc.gpsimd.memset(AselS[:, :], 0.0)
            nc.vector.tensor_copy(out=AselS[0:32, 0:100], in_=pS[0:32, 0:100])
            pAT = epps.tile([128, 512], F32, name="e2", tag="e2")
            nc.tensor.transpose(out=pAT[0:128, 0:128],
                                in_=AselS[0:128, 0:128], identity=idn[:, :])
            AT_sb = ep.tile([128, 32], F32, name="AT", tag="AT")
            nc.vector.tensor_copy(out=AT_sb[0:128, 0:32], in_=pAT[0:128, 0:32])

            # per-head tanh + w2 dot
            psS_tiles = [spps.tile([128, 512], F32, name=f"psS{q}", tag=f"psS{q}")
                         for q in range(4)]
            for q in range(4):
                nc.vector.memset(psS_tiles[q][:, :], 0.0)
            gsb_tiles = [ep.tile([128, 512], F32, name=f"gsb{q}", tag=f"gsb{q}")
                         for q in range(4)]
            for r in range(32):
                th_t = thp.tile([100, 256], BF16, name=f"th{r % 4}",
                                tag=f"th{r % 4}")
                nc.scalar.activation(
                    th_t[0:100, 0:256], B_sb[0:100, 0:256], AF.Tanh,
                    bias=AT_sb[0:100, r:r + 1], scale=1.0)
                q, half = divmod(r // 4, 2)
                nc.tensor.matmul(
                    psS_tiles[q][32 * (r % 4): 32 * (r % 4) + 1,
                                 256 * half: 256 * half + 256],
                    lhsT=w2_sb[0:100, 0:1],
                    rhs=th_t[0:100, 0:256],
                    start=True, stop=True,
                    skip_group_check=True,
                    tile_position=(0, 32 * (r % 4)))
            for q in range(4):
                nc.vector.tensor_scalar(
                    out=gsb_tiles[q][0:128, 0:512],
                    in0=psS_tiles[q][0:128, 0:512],
                    scalar1=b2_sb[0:128, 0:1], scalar2=None, op0=OP.add)
                for half in range(2):
                    rb = 4 * (2 * q + half)
                    nc.sync.dma_start(
                        out=grid[rb:rb + 4, 0:256],
                        in_=gsb_tiles[q][0:128:32, 256 * half: 256 * half + 256])

    nc.compile()
    return nc


_NC_CACHE = None


def _get_nc():
    global _NC_CACHE
    if _NC_CACHE is None:
        _NC_CACHE = build_nc()
    return _NC_CACHE


def kernel(**inputs) -> np.ndarray:
    from concourse.bass_utils import run_bass_kernel_spmd

    arr = _prep_inputs(**inputs)
    nc = _get_nc()
    in_maps = []
    for k in range(NC):
        m = dict(arr)
        m["selT"] = _make_selT(k)
        in_maps.append(m)
    res = run_bass_kernel_spmd(nc, in_maps, core_ids=list(range(NC)))
    grid = np.concatenate([res.results[k]["grid"] for k in range(NC)], axis=0)
    mask = np.ones((N, N), dtype=bool)
    np.fill_diagonal(mask, False)
    mask[:, 0] = False
    return grid[mask].reshape(-1, 1).astype(np.float32)


# revision 37
# speedup vs baseline: 3.1607x; 1.0020x over previous
"""Trainium2 Bass kernel: BiLSTM dependency-parser edge scorer (v2).

Self-contained. Accepts FULL inputs (as produced by setup_inputs()), returns
the FULL [65280, 1] float32 score tensor.

Key idea vs v1: all recurrence matmuls are WEIGHTS-STATIONARY (weights in
lhsT, the tiny h vector streams as rhs), so each step's 64 gate matmuls have
output free-size 1 instead of streaming 6400 PSUM rows.

Layouts (per direction d, layer l):
  gates PSUM tile [100, 16]: partition p, col n = 4*j + g where the LSTM
    unit is u = 100*j + p (j in 0..4) and g in {0:i, 1:f, 2:g, 3:o}.
  h storage H[l][d] [100, 4*256] bf16: h_t for unit (j, p) at col 4*t + j.
    Column 4*t+j is directly the rhs [100, 1] for K-chunk j of the next
    step's matmul -- no transpose inside the loop.
  c state [100, 4] f32.
  xg_sb[d] [100, 16*256] bf16: precomputed input projections + bias,
    injected into the PSUM accumulation via an identity-weight matmul.
g-gate rows are pre-scaled by 2 on host: tanh(x) = 2*sigmoid(2x) - 1.
"""

import os
import sys

sys.path.insert(0, "/opt/trn_rl_repo")

import numpy as np

import concourse.bass as bass
import concourse.mybir as mybir
from concourse import bacc
from concourse.bass import IndirectOffsetOnAxis
from concourse.masks import make_identity
from concourse.tile import TileContext

N = 256          # sequence length
HID = 400        # hidden per direction
NC = 8           # cores
P = 100          # partitions used for unit math
NG = 16          # gate cols per step
F32 = mybir.dt.float32
BF16 = mybir.dt.float16
I32 = mybir.dt.int32
AF = mybir.ActivationFunctionType
OP = mybir.AluOpType

STEPS = int(os.environ.get("DP_STEPS", str(N)))


# ---------------------------------------------------------------------------
# host-side weight layout prep
# ---------------------------------------------------------------------------

def _bf(a):
    return np.ascontiguousarray(np.asarray(a).astype(np.float16))


# R[p, n] = original torch gate-row for (partition p, col n)
_PP, _NN = np.meshgrid(np.arange(P), np.arange(NG), indexing="ij")
_R = 400 * (_NN // 4) + 100 * (_NN % 4) + _PP      # [100, 16]


def _scale_g(W):
    """Scale g-gate rows (orig rows 800:1200) by 2."""
    Ws = np.array(W, dtype=np.float64)
    Ws[800:1200] *= 2.0
    return Ws


def _wblocks(W, nuc):
    """W: [1600, U] scaled gate-major weights, U = 100*nuc.
    Returns [100, 16*nuc*100]: block (n, uc) at cols (n*nuc+uc)*100 holds
    lhsT[k, m] = W[R[m, n], 100*uc + k]."""
    arr = W[_R]                                    # [100p, 16n, U]
    A4 = arr.reshape(P, NG, nuc, 100)              # [p, n, uc, k]
    return A4.transpose(3, 1, 2, 0).reshape(100, NG * nuc * 100)


def _prep_inputs(word_idx, pos_idx, word_emb, pos_emb,
                 Wih0, Whh0, bih0, bhh0, Wih1, Whh1, bih1, bhh1,
                 fc1_W, fc1_b, fc2_W, fc2_b):
    arr = {}
    arr["widx"] = np.ascontiguousarray(
        np.asarray(word_idx).reshape(N, 1).astype(np.int32))
    arr["pidx"] = np.ascontiguousarray(
        np.asarray(pos_idx).reshape(N, 1).astype(np.int32))
    arr["wemb"] = np.ascontiguousarray(np.asarray(word_emb, dtype=np.float32))
    arr["pemb"] = np.ascontiguousarray(np.asarray(pos_emb, dtype=np.float32))

    Wih = [np.asarray(Wih0, np.float64), np.asarray(Wih1, np.float64)]
    Whh = [np.asarray(Whh0, np.float64), np.asarray(Whh1, np.float64)]
    bih = [np.asarray(bih0, np.float64), np.asarray(bih1, np.float64)]
    bhh = [np.asarray(bhh0, np.float64), np.asarray(bhh1, np.float64)]

    whhT = np.zeros((4, 100, NG * 4 * 100), np.float32)
    biasT = np.zeros((4, 100, 512), np.float32)
    wih0T = np.zeros((2, 100, NG * 4 * 100), np.float32)
    wih1T = np.zeros((2, 100, NG * 8 * 100), np.float32)
    for l in range(2):
        for d in range(2):
            dl = 2 * l + d
            whhT[dl] = _wblocks(_scale_g(Whh[l][d]), 4)
            b = _scale_g(bih[l][d] + bhh[l][d])[_R]          # [100, 16]
            biasT[dl] = np.tile(b, (1, 32)).astype(np.float32)
    for d in range(2):
        wih0T[d] = _wblocks(_scale_g(Wih[0][d]), 4)
        wih1T[d] = _wblocks(_scale_g(Wih[1][d]), 8)
    arr["whhT"] = _bf(whhT)
    arr["biasT"] = np.ascontiguousarray(biasT)
    arr["wih0T"] = _bf(wih0T)
    arr["wih1T"] = _bf(wih1T)

    # identity for the xg injection matmul
    arr["id100"] = _bf(np.eye(P, dtype=np.float32))

    # edge MLP: uhT/umT [100, 800]: block uc at cols 100*uc holds
    # lhsT[k, a] = fc1_W[a, 100*uc + k]
    f1 = np.asarray(fc1_W, np.float64)               # [100, 1600]
    arr["uhT"] = _bf(np.concatenate(
        [f1[:, 100 * u:100 * u + 100].T for u in range(8)], axis=1))
    arr["umT"] = _bf(np.concatenate(
        [f1[:, 800 + 100 * u:800 + 100 * u + 100].T for u in range(8)],
        axis=1))
    arr["b1row"] = np.ascontiguousarray(
        np.asarray(fc1_b, np.float32).reshape(1, 100))
    arr["w2"] = _bf(np.asarray(fc2_W, np.float32).reshape(100, 1))
    arr["b2"] = np.ascontiguousarray(
        np.full((128, 1), np.float32(np.asarray(fc2_b).reshape(())),
                dtype=np.float32))
    return arr


def _make_selT(core):
    s = np.zeros((2, 128, 32), np.float32)
    for r in range(32):
        t = 32 * core + r
        s[t // 128, t % 128, r] = 1.0
    return np.ascontiguousarray(s)


# ---------------------------------------------------------------------------
# device kernel build
# ---------------------------------------------------------------------------

def _emit_xg_group(nc, nuc, ucs, wih_sb_d, rhs_chunk, d, tc, dst_sb,
                   bias_sb, ps_pool, tag):
    """One t-chunk (32 tokens) of an input-projection GEMM: 16*len(ucs)
    weights-stationary matmuls accumulating into a PSUM bank, then one
    PSUM->SBUF copy (adding bias if given)."""
    ps = ps_pool.tile([128, 512], F32, name=tag, tag=tag)
    for n in range(NG):
        for i, uc in enumerate(ucs):
            nc.tensor.matmul(
                ps[0:P, n:512:16],
                lhsT=wih_sb_d[0:P, (n * nuc + uc) * 100:
                              (n * nuc + uc) * 100 + 100],
                rhs=rhs_chunk(d, uc, tc),
                start=(i == 0), stop=(i == len(ucs) - 1),
                skip_group_check=True)
    if bias_sb is not None:
        nc.vector.tensor_tensor(
            out=dst_sb[0:P, 512 * tc: 512 * tc + 512],
            in0=ps[0:P, 0:512], in1=bias_sb[0:P, 0:512], op=OP.add)
    else:
        nc.vector.tensor_copy(
            out=dst_sb[0:P, 512 * tc: 512 * tc + 512],
            in_=ps[0:P, 0:512])


def _emit_xg(nc, l, wih_sb, rhs_chunk, xg_sbs, bias_sbs, ps_pool):
    """Full xg for layer l: xg[d][p, 16*t + n] = sum_u W[r(p,n), u]*in[t,u]+b."""
    nuc = 4 if l == 0 else 8
    for d in range(2):
        for tc in range(8):
            _emit_xg_group(nc, nuc, list(range(nuc)), wih_sb[d], rhs_chunk,
                           d, tc, xg_sbs[d], bias_sbs[d], ps_pool, "xgps")


def _emit_rec(nc, l, whh_sb, xg_sbs, id_sb, H_out, pools, xg2_sbs=None,
              extra=None):
    """STEPS wall-steps, both directions interleaved. xg2_sbs: optional
    second injection source (bwd-half input projections for layer 1).
    extra(t): called after each wall-step to emit overlapped work."""
    state_pool, sg_pool, tmp_pool, ps_pool = pools
    cs = []
    for d in range(2):
        c = state_pool.tile([P, 4], F32, name=f"c{d}", tag=f"c{d}")
        nc.gpsimd.memset(c[:, :], 0.0)
        cs.append(c)

    for t in range(STEPS):
        ps_t, sg_t, th_t = {}, {}, {}
        dorder = (0, 1)
        # --- PE: injection + 64 weight matmuls per direction ---
        for d in dorder:
            tdx = t if d == 0 else (STEPS - 1 - t)
            ps = ps_pool.tile([128, 512], F32, name=f"ps{d}", tag=f"ps{d}")
            ps_t[d] = ps
            first = (t == 0)
            nc.tensor.matmul(
                ps[0:P, 0:NG],
                lhsT=id_sb[0:P, 0:P],
                rhs=xg_sbs[d][0:P, NG * tdx: NG * tdx + NG],
                start=True, stop=(first and xg2_sbs is None),
                skip_group_check=True)
            if xg2_sbs is not None:
                nc.tensor.matmul(
                    ps[0:P, 0:NG],
                    lhsT=id_sb[0:P, 0:P],
                    rhs=xg2_sbs[d][0:P, NG * tdx: NG * tdx + NG],
                    start=False, stop=first, skip_group_check=True)
            if not first:
                pdx = tdx - 1 if d == 0 else tdx + 1
                # uc-major: uc 0/1 depend only on the first h half-write, so
                # their issue overlaps the second half-write
                for uc in range(4):
                    for n in range(NG):
                        nc.tensor.matmul(
                            ps[0:P, n:n + 1],
                            lhsT=whh_sb[d][0:P, (4 * n + uc) * 100:
                                           (4 * n + uc) * 100 + 100],
                            rhs=H_out[d][0:P, 4 * pdx + uc: 4 * pdx + uc + 1],
                            start=False, stop=(uc == 3),
                            skip_group_check=True)
        # --- Act: sigmoid over all gates (g pre-scaled by 2) ---
        for d in dorder:
            sg = sg_pool.tile([P, NG], F32, name=f"sg{d}", tag=f"sg{d}")
            sg_t[d] = sg
            nc.scalar.activation(sg[0:P, 0:NG], ps_t[d][0:P, 0:NG], AF.Sigmoid)
        # --- DVE per direction: c = sig(f)*c + sig(i)*(2*sig(2g) - 1),
        #     then tanh(c) via Pade [3/2] (|c| < 0.5 here, err < 1e-6):
        #     tanh(c) ~= c*(15 + c^2) / (15 + 6*c^2);  h = sig(o)*tanh ---
        for d in dorder:
            tdx = t if d == 0 else (STEPS - 1 - t)
            sg, c = sg_t[d], cs[d]
            tg = tmp_pool.tile([P, 4], F32, name=f"tg{d}", tag=f"tg{d}")
            t1 = tmp_pool.tile([P, 4], F32, name=f"t1{d}", tag=f"t1{d}")
            cf = tmp_pool.tile([P, 4], F32, name=f"cf{d}", tag=f"cf{d}")
            nc.vector.tensor_scalar(
                out=tg[0:P, 0:4], in0=sg[0:P, 8:12],
                scalar1=2.0, scalar2=-1.0, op0=OP.mult, op1=OP.add)
            nc.vector.tensor_tensor(
                out=cf[0:P, 0:4], in0=sg[0:P, 4:8],
                in1=c[0:P, 0:4], op=OP.mult)
            nc.vector.tensor_tensor(
                out=t1[0:P, 0:4], in0=sg[0:P, 0:4],
                in1=tg[0:P, 0:4], op=OP.mult)
            nc.vector.tensor_tensor(
                out=c[0:P, 0:4], in0=cf[0:P, 0:4],
                in1=t1[0:P, 0:4], op=OP.add)
            th = tmp_pool.tile([P, 4], F32, name=f"th{d}", tag=f"th{d}")
            th_t[d] = th
            nc.scalar.activation(th[0:P, 0:4], c[0:P, 0:4], AF.Tanh)
        for d in dorder:
            tdx = t if d == 0 else (STEPS - 1 - t)
            nc.vector.tensor_tensor(
                out=H_out[d][0:P, 4 * tdx: 4 * tdx + 2],
                in0=sg_t[d][0:P, 12:14], in1=th_t[d][0:P, 0:2], op=OP.mult)
            nc.vector.tensor_tensor(
                out=H_out[d][0:P, 4 * tdx + 2: 4 * tdx + 4],
                in0=sg_t[d][0:P, 14:NG], in1=th_t[d][0:P, 2:4], op=OP.mult)
        if extra is not None:
            extra(t)


def build_nc():
    nc = bacc.Bacc("TRN2", target_bir_lowering=False, debug=False,
                   num_devices=NC)
    wemb = nc.dram_tensor("wemb", [50000, 300], F32, kind="ExternalInput").ap()
    pemb = nc.dram_tensor("pemb", [50, 100], F32, kind="ExternalInput").ap()
    widx = nc.dram_tensor("widx", [N, 1], I32, kind="ExternalInput").ap()
    pidx = nc.dram_tensor("pidx", [N, 1], I32, kind="ExternalInput").ap()
    whhTd = nc.dram_tensor("whhT", [4, 100, 6400], BF16, kind="ExternalInput").ap()
    wih0Td = nc.dram_tensor("wih0T", [2, 100, 6400], BF16, kind="ExternalInput").ap()
    wih1Td = nc.dram_tensor("wih1T", [2, 100, 12800], BF16, kind="ExternalInput").ap()
    biasTd = nc.dram_tensor("biasT", [4, 100, 512], F32, kind="ExternalInput").ap()
    id100d = nc.dram_tensor("id100", [100, 100], BF16, kind="ExternalInput").ap()
    uhTd = nc.dram_tensor("uhT", [100, 800], BF16, kind="ExternalInput").ap()
    umTd = nc.dram_tensor("umT", [100, 800], BF16, kind="ExternalInput").ap()
    b1rowd = nc.dram_tensor("b1row", [1, 100], F32, kind="ExternalInput").ap()
    w2d = nc.dram_tensor("w2", [100, 1], BF16, kind="ExternalInput").ap()
    b2d = nc.dram_tensor("b2", [128, 1], F32, kind="ExternalInput").ap()
    selTd = nc.dram_tensor("selT", [2, 128, 32], F32, kind="ExternalInput").ap()
    grid = nc.dram_tensor("grid", [32, N], F32, kind="ExternalOutput").ap()

    from contextlib import ExitStack
    with TileContext(nc) as tc, ExitStack() as ctx:
        top = ctx.enter_context(tc.tile_pool(name="top", bufs=1))
        # ---- persistent SBUF tiles (DMAs emitted in priority order) ----
        idn = top.tile([128, 128], F32, name="idn", tag="idn")
        make_identity(nc, idn[:, :])
        ones_sb = top.tile([1, 256], F32, name="ones", tag="ones")
        nc.gpsimd.memset(ones_sb[:, :], 1.0)
        whh_sb = [top.tile([100, 6400], BF16, name=f"whh{dl}", tag=f"whh{dl}")
                  for dl in range(4)]
        bias_sb = [top.tile([100, 512], F32, name=f"bias{dl}", tag=f"bias{dl}")
                   for dl in range(4)]
        id_sb = top.tile([100, 100], BF16, name="id100", tag="id100")
        wih1_sb = [top.tile([100, 12800], BF16, name=f"wih1{d}", tag=f"wih1{d}")
                   for d in range(2)]
        xg_sbs = [top.tile([100, 4096], BF16, name=f"xg{d}", tag=f"xg{d}")
                  for d in range(2)]
        H = [[top.tile([100, 4 * N], BF16, name=f"H{l}{d}", tag=f"H{l}{d}")
              for d in range(2)] for l in range(2)]
        xT = top.tile([100, 4 * N], BF16, name="xT", tag="xT")
        wih0_sb = [top.tile([100, 6400], BF16, name=f"wih0{d}",
                            tag=f"wih0{d}") for d in range(2)]
        if STEPS < N:
            for l in range(2):
                for d in range(2):
                    nc.gpsimd.memset(H[l][d][:, :], 0.0)

        # ========= embedding gather (first DMAs in the queue) =========
        with tc.tile_pool(name="wih0p", bufs=1) as w0p, \
             tc.tile_pool(name="embps", bufs=2, space="PSUM") as eps:
            idx_sb = w0p.tile([128, 4], I32, name="idx", tag="idx")
            nc.sync.dma_start(out=idx_sb[0:128, 0:1], in_=widx[0:128, 0:1])
            nc.sync.dma_start(out=idx_sb[0:128, 1:2], in_=widx[128:256, 0:1])
            nc.sync.dma_start(out=idx_sb[0:128, 2:3], in_=pidx[0:128, 0:1])
            nc.sync.dma_start(out=idx_sb[0:128, 3:4], in_=pidx[128:256, 0:1])
            x_sb = w0p.tile([128, 800], F32, name="xsb", tag="xsb")
            for cch in range(2):
                nc.gpsimd.indirect_dma_start(
                    out=x_sb[0:128, 400 * cch: 400 * cch + 300],
                    out_offset=None,
                    in_=wemb[:, :],
                    in_offset=IndirectOffsetOnAxis(
                        ap=idx_sb[0:128, cch:cch + 1], axis=0))
                nc.gpsimd.indirect_dma_start(
                    out=x_sb[0:128, 400 * cch + 300: 400 * cch + 400],
                    out_offset=None,
                    in_=pemb[:, :],
                    in_offset=IndirectOffsetOnAxis(
                        ap=idx_sb[0:128, 2 + cch:3 + cch], axis=0))
            # layer-0 weights + rec0 needs, in DMA-queue priority order
            nc.sync.dma_start(out=bias_sb[0][:, :], in_=biasTd[0])
            nc.sync.dma_start(out=id_sb[:, :], in_=id100d[:, :])
            nc.sync.dma_start(out=wih0_sb[0][:, :], in_=wih0Td[0])
            nc.sync.dma_start(out=whh_sb[0][:, :], in_=whhTd[0])
            nc.sync.dma_start(out=bias_sb[1][:, :], in_=biasTd[1])
            nc.sync.dma_start(out=wih0_sb[1][:, :], in_=wih0Td[1])
            nc.sync.dma_start(out=whh_sb[1][:, :], in_=whhTd[1])

            # x -> xT transpose
            for cch in range(2):
                for uc in range(4):
                    ptr = eps.tile([128, 128], F32, name="ptr", tag="ptr")
                    nc.tensor.transpose(
                        out=ptr[0:100, 0:128],
                        in_=x_sb[0:128, 400 * cch + 100 * uc:
                                 400 * cch + 100 * uc + 100],
                        identity=idn[:, :])
                    nc.vector.tensor_copy(
                        out=xT[0:100, 256 * uc + 128 * cch:
                               256 * uc + 128 * cch + 128],
                        in_=ptr[0:100, 0:128])

            # ========= layer 0 xg: only the chunks needed at rec0 start
            # (rest are interleaved into rec0's idle PE time) =========
            def rhs_l0(d, uc, tc):
                return xT[0:P, 256 * uc + 32 * tc: 256 * uc + 32 * tc + 32]

            with tc.tile_pool(name="xg0ps", bufs=2, space="PSUM") as xg_ps:
                _emit_xg_group(nc, 4, [0, 1, 2, 3], wih0_sb[0], rhs_l0,
                               0, 0, xg_sbs[0], bias_sb[0], xg_ps, "xgps")
                _emit_xg_group(nc, 4, [0, 1, 2, 3], wih0_sb[1], rhs_l0,
                               1, 7, xg_sbs[1], bias_sb[1], xg_ps, "xgps")

        # remaining big DMAs: execute during rec0
        for d in range(2):
            nc.sync.dma_start(out=wih1_sb[d][:, :], in_=wih1Td[d])
        for dl in range(2, 4):
            nc.sync.dma_start(out=whh_sb[dl][:, :], in_=whhTd[dl])
            nc.sync.dma_start(out=bias_sb[dl][:, :], in_=biasTd[dl])

        # ========= rec0 with layer-1 xg interleaved =========
        def rhs_l1(d, uc, tc):
            src = H[0][uc // 4]
            j = uc % 4
            return src[0:P, 128 * tc + j: 128 * tc + 128: 4]

        with tc.tile_pool(name="xg1buf", bufs=1) as xgbuf, \
             tc.tile_pool(name="xg1ps", bufs=2, space="PSUM") as xg1_ps:
            xgf_sbs = [xgbuf.tile([100, 4096], BF16, name=f"xgf{d}",
                                  tag=f"xgf{d}") for d in range(2)]
            xgb_sbs = [xgbuf.tile([100, 4096], BF16, name=f"xgb{d}",
                                  tag=f"xgb{d}") for d in range(2)]

            def mk_group(d, tc_, half):
                def emit():
                    _emit_xg_group(
                        nc, 8, list(range(4 * half, 4 * half + 4)),
                        wih1_sb[d], rhs_l1, d, tc_,
                        xgf_sbs[d] if half == 0 else xgb_sbs[d],
                        bias_sb[2 + d] if half == 0 else None,
                        xg1_ps, "xg1ps")
                return emit

            def mk_group0(d, tc_):
                def emit():
                    _emit_xg_group(nc, 4, [0, 1, 2, 3], wih0_sb[d], rhs_l0,
                                   d, tc_, xg_sbs[d], bias_sb[d], xg1_ps,
                                   "xg1ps")
                return emit

            # xg0 leftovers first (avail immediately), ordered by deadline:
            # fwd chunk tc needed by wall step 32*tc, bwd chunk by 224-32*tc
            pend = []
            for tc_ in range(1, 8):
                pend.append((0, mk_group0(0, tc_)))        # deadline 32*tc_
                pend.append((0, mk_group0(1, 7 - tc_)))    # same deadline
            for d in range(2):
                for tc_ in range(8):
                    pend.append((32 * tc_ + 32, mk_group(d, tc_, 0)))
                    pend.append((N - 32 * tc_, mk_group(d, tc_, 1)))
            pend.sort(key=lambda x: x[0])
            # end-gated groups: rec1 needs xgb[0]c0 / xgf[1]c7 at its step 0,
            # but xgf[0]c7 / xgb[1]c0 only by step ~224 -- emit those two
            # inside rec1's idle time instead
            pend = [e for e in pend if e[0] < N]
            urgent = [mk_group(1, 7, 0), mk_group(0, 0, 1)]
            late = [mk_group(0, 7, 0), mk_group(1, 0, 1)]
            st = {"i": 0, "last": -10}

            def extra(t):
                n_emit = 0
                while (st["i"] < len(pend) and pend[st["i"]][0] <= t
                       and t > st["last"] and n_emit < 2):
                    pend[st["i"]][1]()
                    st["i"] += 1
                    n_emit += 1
                if n_emit:
                    st["last"] = t

            with tc.tile_pool(name="rst0", bufs=1) as state_pool, \
                 tc.tile_pool(name="sg0", bufs=6) as sg_pool, \
                 tc.tile_pool(name="tmp0", bufs=6) as tmp_pool, \
                 tc.tile_pool(name="rec0ps", bufs=3, space="PSUM") as rec_ps:
                _emit_rec(nc, 0, whh_sb[0:2], xg_sbs, id_sb, H[0],
                          (state_pool, sg_pool, tmp_pool, rec_ps),
                          extra=extra)
            # leftover in-rec0 groups, then the two urgently needed ones
            while st["i"] < len(pend):
                pend[st["i"]][1]()
                st["i"] += 1
            for fn in urgent:
                fn()

            st1 = {"i": 0, "last": -10}

            def extra1(t):
                if st1["i"] < len(late) and t - st1["last"] >= 2:
                    late[st1["i"]]()
                    st1["i"] += 1
                    st1["last"] = t

            # ========= rec1 (dual injection: fwd + bwd halves) =========
            with tc.tile_pool(name="rst1", bufs=1) as state_pool, \
                 tc.tile_pool(name="sg1", bufs=6) as sg_pool, \
                 tc.tile_pool(name="tmp1", bufs=6) as tmp_pool, \
                 tc.tile_pool(name="rec1ps", bufs=3, space="PSUM") as rec_ps:
                _emit_rec(nc, 1, whh_sb[2:4], xgf_sbs, id_sb, H[1],
                          (state_pool, sg_pool, tmp_pool, rec_ps),
                          xg2_sbs=xgb_sbs, extra=extra1)
            while st1["i"] < len(late):
                late[st1["i"]]()
                st1["i"] += 1

        # ========= edge scorer =========
        with tc.tile_pool(name="edge", bufs=1) as ep, \
             tc.tile_pool(name="edgeth", bufs=4) as thp, \
             tc.tile_pool(name="edgeps", bufs=1, space="PSUM") as epps, \
             tc.tile_pool(name="edgepsS", bufs=1, space="PSUM") as spps:
            uh_sb = ep.tile([100, 800], BF16, name="uhT", tag="uhT")
            nc.sync.dma_start(out=uh_sb[:, :], in_=uhTd[:, :])
            um_sb = ep.tile([100, 800], BF16, name="umT", tag="umT")
            nc.sync.dma_start(out=um_sb[:, :], in_=umTd[:, :])
            b1_sb = ep.tile([1, 100], F32, name="b1row", tag="b1row")
            nc.sync.dma_start(out=b1_sb[:, :], in_=b1rowd[:, :])
            w2_sb = ep.tile([100, 1], BF16, name="w2", tag="w2")
            nc.sync.dma_start(out=w2_sb[:, :], in_=w2d[:, :])
            b2_sb = ep.tile([128, 1], F32, name="b2", tag="b2")
            nc.sync.dma_start(out=b2_sb[:, :], in_=b2d[:, :])
            selT_sb = ep.tile([128, 64], F32, name="selT", tag="selT")
            nc.sync.dma_start(out=selT_sb[0:128, 0:32], in_=selTd[0])
            nc.sync.dma_start(out=selT_sb[0:128, 32:64], in_=selTd[1])

            def h1_rhs(uc):
                return H[1][uc // 4][0:P, uc % 4: 4 * N: 4]

            # A^T [100, 256] (head half of fc1)
            pA = epps.tile([128, 512], F32, name="e1", tag="e1")
            for uc in range(8):
                nc.tensor.matmul(
                    pA[0:P, 0:256],
                    lhsT=uh_sb[0:P, 100 * uc: 100 * uc + 100],
                    rhs=h1_rhs(uc),
                    start=(uc == 0), stop=(uc == 7), skip_group_check=True)
            A_sb = ep.tile([100, 256], F32, name="A", tag="A")
            nc.vector.tensor_copy(out=A_sb[0:P, 0:256], in_=pA[0:P, 0:256])
            # B^T [100, 256] + b1 (modifier half)
            pB = epps.tile([128, 512], F32, name="e3", tag="e3")
            for uc in range(8):
                nc.tensor.matmul(
                    pB[0:P, 0:256],
                    lhsT=um_sb[0:P, 100 * uc: 100 * uc + 100],
                    rhs=h1_rhs(uc),
                    start=(uc == 0), stop=False, skip_group_check=True)
            nc.tensor.matmul(
                pB[0:P, 0:256],
                lhsT=b1_sb[0:1, 0:100],
                rhs=ones_sb[0:1, 0:256],
                start=False, stop=True, skip_group_check=True)
            B_sb = ep.tile([100, 256], BF16, name="Bsb", tag="Bsb")
            nc.vector.tensor_copy(out=B_sb[0:P, 0:256], in_=pB[0:P, 0:256])
            # A -> token-major via transpose, then per-core 32-head select
            A_tok = ep.tile([128, 256], F32, name="Atok", tag="Atok")
            for m in range(2):
                pT = epps.tile([128, 512], F32, name="e2", tag="e2")
                nc.tensor.transpose(
                    out=pT[0:128, 0:100],
                    in_=A_sb[0:100, 128 * m: 128 * m + 128],
                    identity=idn[0:100, 0:100])
                nc.vector.tensor_copy(
                    out=A_tok[0:128, 128 * m: 128 * m + 100],
                    in_=pT[0:128, 0:100])
            pS = epps.tile([128, 512], F32, name="e1", tag="e1")
            for m in range(2):
                nc.tensor.matmul(
                    pS[0:32, 0:100],
                    lhsT=selT_sb[0:128, 32 * m: 32 * m + 32],
                    rhs=A_tok[0:128, 128 * m: 128 * m + 100],
                    start=(m == 0), stop=(m == 1), skip_group_check=True)
            AselS = ep.tile([128, 128], F32, name="AselS", tag="AselS")
            nc.gpsimd.memset(AselS[:, :], 0.0)
            nc.vector.tensor_copy(out=AselS[0:32, 0:100], in_=pS[0:32, 0:100])
            pAT = epps.tile([128, 512], F32, name="e2", tag="e2")
            nc.tensor.transpose(out=pAT[0:128, 0:128],
                                in_=AselS[0:128, 0:128], identity=idn[:, :])
            AT_sb = ep.tile([128, 32], F32, name="AT", tag="AT")
            nc.vector.tensor_copy(out=AT_sb[0:128, 0:32], in_=pAT[0:128, 0:32])

            # per-head tanh + w2 dot
            psS_tiles = [spps.tile([128, 512], F32, name=f"psS{q}", tag=f"psS{q}")
                         for q in range(4)]
            for q in range(4):
                nc.vector.memset(psS_tiles[q][:, :], 0.0)
            gsb_tiles = [ep.tile([128, 512], F32, name=f"gsb{q}", tag=f"gsb{q}")
                         for q in range(4)]
            for r in range(32):
                th_t = thp.tile([100, 256], BF16, name=f"th{r % 4}",
                                tag=f"th{r % 4}")
                nc.scalar.activation(
                    th_t[0:100, 0:256], B_sb[0:100, 0:256], AF.Tanh,
                    bias=AT_sb[0:100, r:r + 1], scale=1.0)
                q, half = divmod(r // 4, 2)
                nc.tensor.matmul(
                    psS_tiles[q][32 * (r % 4): 32 * (r % 4) + 1,
                                 256 * half: 256 * half + 256],
                    lhsT=w2_sb[0:100, 0:1],
                    rhs=th_t[0:100, 0:256],
                    start=True, stop=True,
                    skip_group_check=True,
                    tile_position=(0, 32 * (r % 4)))
            for q in range(4):
                nc.vector.tensor_scalar(
                    out=gsb_tiles[q][0:128, 0:512],
                    in0=psS_tiles[q][0:128, 0:512],
                    scalar1=b2_sb[0:128, 0:1], scalar2=None, op0=OP.add)
                for half in range(2):
                    rb = 4 * (2 * q + half)
                    nc.sync.dma_start(
                        out=grid[rb:rb + 4, 0:256],
                        in_=gsb_tiles[q][0:128:32, 256 * half: 256 * half + 256])

    nc.compile()
    return nc


_NC_CACHE = None


def _get_nc():
    global _NC_CACHE
    if _NC_CACHE is None:
        _NC_CACHE = build_nc()
    return _NC_CACHE


def kernel(**inputs) -> np.ndarray:
    from concourse.bass_utils import run_bass_kernel_spmd

    arr = _prep_inputs(**inputs)
    nc = _get_nc()
    in_maps = []
    for k in range(NC):
        m = dict(arr)
        m["selT"] = _make_selT(k)
        in_maps.append(m)
    res = run_bass_kernel_spmd(nc, in_maps, core_ids=list(range(NC)))
    grid = np.concatenate([res.results[k]["grid"] for k in range(NC)], axis=0)
    mask = np.ones((N, N), dtype=bool)
    np.fill_diagonal(mask, False)
    mask[:, 0] = False
    return grid[mask].reshape(-1, 1).astype(np.float32)


# revision 42
# speedup vs baseline: 3.2966x; 1.0430x over previous
"""Trainium2 Bass kernel: BiLSTM dependency-parser edge scorer (v2).

Self-contained. Accepts FULL inputs (as produced by setup_inputs()), returns
the FULL [65280, 1] float32 score tensor.

Key idea vs v1: all recurrence matmuls are WEIGHTS-STATIONARY (weights in
lhsT, the tiny h vector streams as rhs), so each step's 64 gate matmuls have
output free-size 1 instead of streaming 6400 PSUM rows.

Layouts (per direction d, layer l):
  gates PSUM tile [100, 16]: partition p, col n = 4*j + g where the LSTM
    unit is u = 100*j + p (j in 0..4) and g in {0:i, 1:f, 2:g, 3:o}.
  h storage H[l][d] [100, 4*256] bf16: h_t for unit (j, p) at col 4*t + j.
    Column 4*t+j is directly the rhs [100, 1] for K-chunk j of the next
    step's matmul -- no transpose inside the loop.
  c state [100, 4] f32.
  xg_sb[d] [100, 16*256] bf16: precomputed input projections + bias,
    injected into the PSUM accumulation via an identity-weight matmul.
g-gate rows are pre-scaled by 2 on host: tanh(x) = 2*sigmoid(2x) - 1.
"""

import os
import sys

sys.path.insert(0, "/opt/trn_rl_repo")

import numpy as np

import concourse.bass as bass
import concourse.mybir as mybir
from concourse import bacc
from concourse.bass import IndirectOffsetOnAxis
from concourse.masks import make_identity
from concourse.tile import TileContext

N = 256          # sequence length
HID = 400        # hidden per direction
NC = 8           # cores
P = 100          # partitions used for unit math
NG = 16          # gate cols per step
F32 = mybir.dt.float32
BF16 = mybir.dt.float16
I32 = mybir.dt.int32
AF = mybir.ActivationFunctionType
OP = mybir.AluOpType

STEPS = int(os.environ.get("DP_STEPS", str(N)))


# ---------------------------------------------------------------------------
# host-side weight layout prep
# ---------------------------------------------------------------------------

def _bf(a):
    return np.ascontiguousarray(np.asarray(a).astype(np.float16))


# R[p, n] = original torch gate-row for (partition p, col n)
_PP, _NN = np.meshgrid(np.arange(P), np.arange(NG), indexing="ij")
_R = 400 * (_NN // 4) + 100 * (_NN % 4) + _PP      # [100, 16]


def _scale_g(W):
    """Scale g-gate rows (orig rows 800:1200) by 2."""
    Ws = np.array(W, dtype=np.float64)
    Ws[800:1200] *= 2.0
    return Ws


def _wblocks(W, nuc):
    """W: [1600, U] scaled gate-major weights, U = 100*nuc.
    Returns [100, 16*nuc*100]: block (n, uc) at cols (n*nuc+uc)*100 holds
    lhsT[k, m] = W[R[m, n], 100*uc + k]."""
    arr = W[_R]                                    # [100p, 16n, U]
    A4 = arr.reshape(P, NG, nuc, 100)              # [p, n, uc, k]
    return A4.transpose(3, 1, 2, 0).reshape(100, NG * nuc * 100)


def _prep_inputs(word_idx, pos_idx, word_emb, pos_emb,
                 Wih0, Whh0, bih0, bhh0, Wih1, Whh1, bih1, bhh1,
                 fc1_W, fc1_b, fc2_W, fc2_b):
    arr = {}
    arr["widx"] = np.ascontiguousarray(
        np.asarray(word_idx).reshape(N, 1).astype(np.int32))
    arr["pidx"] = np.ascontiguousarray(
        np.asarray(pos_idx).reshape(N, 1).astype(np.int32))
    arr["wemb"] = np.ascontiguousarray(np.asarray(word_emb, dtype=np.float32))
    arr["pemb"] = np.ascontiguousarray(np.asarray(pos_emb, dtype=np.float32))

    Wih = [np.asarray(Wih0, np.float64), np.asarray(Wih1, np.float64)]
    Whh = [np.asarray(Whh0, np.float64), np.asarray(Whh1, np.float64)]
    bih = [np.asarray(bih0, np.float64), np.asarray(bih1, np.float64)]
    bhh = [np.asarray(bhh0, np.float64), np.asarray(bhh1, np.float64)]

    whhT = np.zeros((4, 100, NG * 4 * 100), np.float32)
    biasT = np.zeros((4, 100, 512), np.float32)
    wih0T = np.zeros((2, 100, NG * 4 * 100), np.float32)
    wih1T = np.zeros((2, 100, NG * 8 * 100), np.float32)
    for l in range(2):
        for d in range(2):
            dl = 2 * l + d
            whhT[dl] = _wblocks(_scale_g(Whh[l][d]), 4)
            b = _scale_g(bih[l][d] + bhh[l][d])[_R]          # [100, 16]
            biasT[dl] = np.tile(b, (1, 32)).astype(np.float32)
    for d in range(2):
        wih0T[d] = _wblocks(_scale_g(Wih[0][d]), 4)
        wih1T[d] = _wblocks(_scale_g(Wih[1][d]), 8)
    arr["whhT"] = _bf(whhT)
    arr["biasT"] = np.ascontiguousarray(biasT)
    arr["wih0T"] = _bf(wih0T)
    arr["wih1T"] = _bf(wih1T)

    # identity for the xg injection matmul
    arr["id100"] = _bf(np.eye(P, dtype=np.float32))

    # edge MLP: uhT/umT [100, 800]: block uc at cols 100*uc holds
    # lhsT[k, a] = fc1_W[a, 100*uc + k]
    f1 = np.asarray(fc1_W, np.float64)               # [100, 1600]
    arr["uhT"] = _bf(np.concatenate(
        [f1[:, 100 * u:100 * u + 100].T for u in range(8)], axis=1))
    arr["umT"] = _bf(np.concatenate(
        [f1[:, 800 + 100 * u:800 + 100 * u + 100].T for u in range(8)],
        axis=1))
    arr["b1row"] = np.ascontiguousarray(
        np.asarray(fc1_b, np.float32).reshape(1, 100))
    arr["w2"] = _bf(np.asarray(fc2_W, np.float32).reshape(100, 1))
    arr["b2"] = np.ascontiguousarray(
        np.full((128, 1), np.float32(np.asarray(fc2_b).reshape(())),
                dtype=np.float32))
    return arr


def _make_selT(core):
    s = np.zeros((2, 128, 32), np.float32)
    for r in range(32):
        t = 32 * core + r
        s[t // 128, t % 128, r] = 1.0
    return np.ascontiguousarray(s)


# ---------------------------------------------------------------------------
# device kernel build
# ---------------------------------------------------------------------------

def _emit_xg_group(nc, nuc, ucs, wih_sb_d, rhs_chunk, d, tc, dst_sb,
                   bias_sb, ps_pool, tag):
    """One t-chunk (32 tokens) of an input-projection GEMM: 16*len(ucs)
    weights-stationary matmuls accumulating into a PSUM bank, then one
    PSUM->SBUF copy (adding bias if given)."""
    ps = ps_pool.tile([128, 512], F32, name=tag, tag=tag)
    for n in range(NG):
        for i, uc in enumerate(ucs):
            nc.tensor.matmul(
                ps[0:P, n:512:16],
                lhsT=wih_sb_d[0:P, (n * nuc + uc) * 100:
                              (n * nuc + uc) * 100 + 100],
                rhs=rhs_chunk(d, uc, tc),
                start=(i == 0), stop=(i == len(ucs) - 1),
                skip_group_check=True)
    if bias_sb is not None:
        nc.vector.tensor_tensor(
            out=dst_sb[0:P, 512 * tc: 512 * tc + 512],
            in0=ps[0:P, 0:512], in1=bias_sb[0:P, 0:512], op=OP.add)
    else:
        nc.vector.tensor_copy(
            out=dst_sb[0:P, 512 * tc: 512 * tc + 512],
            in_=ps[0:P, 0:512])


def _emit_xg(nc, l, wih_sb, rhs_chunk, xg_sbs, bias_sbs, ps_pool):
    """Full xg for layer l: xg[d][p, 16*t + n] = sum_u W[r(p,n), u]*in[t,u]+b."""
    nuc = 4 if l == 0 else 8
    for d in range(2):
        for tc in range(8):
            _emit_xg_group(nc, nuc, list(range(nuc)), wih_sb[d], rhs_chunk,
                           d, tc, xg_sbs[d], bias_sbs[d], ps_pool, "xgps")


def _emit_rec(nc, l, whh_sb, xg_sbs, id_sb, H_out, pools, xg2_sbs=None,
              extra=None):
    """STEPS wall-steps, both directions interleaved. xg2_sbs: optional
    second injection source (bwd-half input projections for layer 1).
    extra(t): called after each wall-step to emit overlapped work."""
    state_pool, sg_pool, tmp_pool, ps_pool = pools
    cs = []
    for d in range(2):
        c = state_pool.tile([P, 4], F32, name=f"c{d}", tag=f"c{d}")
        nc.gpsimd.memset(c[:, :], 0.0)
        cs.append(c)

    for t in range(STEPS):
        ps_t, sg_t, th_t = {}, {}, {}
        dorder = (0, 1)
        # --- PE: injection + 64 weight matmuls per direction ---
        for d in dorder:
            tdx = t if d == 0 else (STEPS - 1 - t)
            ps = ps_pool.tile([128, 512], F32, name=f"ps{d}", tag=f"ps{d}")
            ps_t[d] = ps
            first = (t == 0)
            nc.tensor.matmul(
                ps[0:P, 0:NG],
                lhsT=id_sb[0:P, 0:P],
                rhs=xg_sbs[d][0:P, NG * tdx: NG * tdx + NG],
                start=True, stop=(first and xg2_sbs is None),
                skip_group_check=True)
            if xg2_sbs is not None:
                nc.tensor.matmul(
                    ps[0:P, 0:NG],
                    lhsT=id_sb[0:P, 0:P],
                    rhs=xg2_sbs[d][0:P, NG * tdx: NG * tdx + NG],
                    start=False, stop=first, skip_group_check=True)
            if not first:
                pdx = tdx - 1 if d == 0 else tdx + 1
                # uc-major: uc 0/1 depend only on the first h half-write, so
                # their issue overlaps the second half-write
                for uc in range(4):
                    for n in range(NG):
                        nc.tensor.matmul(
                            ps[0:P, n:n + 1],
                            lhsT=whh_sb[d][0:P, (4 * n + uc) * 100:
                                           (4 * n + uc) * 100 + 100],
                            rhs=H_out[d][0:P, 4 * pdx + uc: 4 * pdx + uc + 1],
                            start=False, stop=(uc == 3),
                            skip_group_check=True)
        # --- Act: sigmoid over all gates (g pre-scaled by 2) ---
        for d in dorder:
            sg = sg_pool.tile([P, NG], F32, name=f"sg{d}", tag=f"sg{d}")
            sg_t[d] = sg
            nc.scalar.activation(sg[0:P, 0:NG], ps_t[d][0:P, 0:NG], AF.Sigmoid)
        # --- DVE per direction: c = sig(f)*c + sig(i)*(2*sig(2g) - 1),
        #     then tanh(c) via Pade [3/2] (|c| < 0.5 here, err < 1e-6):
        #     tanh(c) ~= c*(15 + c^2) / (15 + 6*c^2);  h = sig(o)*tanh ---
        for d in dorder:
            tdx = t if d == 0 else (STEPS - 1 - t)
            sg, c = sg_t[d], cs[d]
            # c = sig(f)*c + sig(i)*(2*sig(2g)-1), fused as:
            #   q = (sig_g - 0.5)*sig_i;  c = 2*q + sig_f*c
            q = tmp_pool.tile([P, 4], F32, name=f"q{d}", tag=f"q{d}")
            cf = tmp_pool.tile([P, 4], F32, name=f"cf{d}", tag=f"cf{d}")
            nc.vector.scalar_tensor_tensor(
                out=q[0:P, 0:4], in0=sg[0:P, 8:12], scalar=0.5,
                in1=sg[0:P, 0:4], op0=OP.subtract, op1=OP.mult)
            nc.vector.tensor_tensor(
                out=cf[0:P, 0:4], in0=sg[0:P, 4:8],
                in1=c[0:P, 0:4], op=OP.mult)
            nc.vector.scalar_tensor_tensor(
                out=c[0:P, 0:4], in0=q[0:P, 0:4], scalar=2.0,
                in1=cf[0:P, 0:4], op0=OP.mult, op1=OP.add)
            th = tmp_pool.tile([P, 4], F32, name=f"th{d}", tag=f"th{d}")
            th_t[d] = th
            nc.scalar.activation(th[0:P, 0:4], c[0:P, 0:4], AF.Tanh)
        for d in dorder:
            tdx = t if d == 0 else (STEPS - 1 - t)
            nc.vector.tensor_tensor(
                out=H_out[d][0:P, 4 * tdx: 4 * tdx + 2],
                in0=sg_t[d][0:P, 12:14], in1=th_t[d][0:P, 0:2], op=OP.mult)
            nc.vector.tensor_tensor(
                out=H_out[d][0:P, 4 * tdx + 2: 4 * tdx + 4],
                in0=sg_t[d][0:P, 14:NG], in1=th_t[d][0:P, 2:4], op=OP.mult)
        if extra is not None:
            extra(t)


def build_nc():
    nc = bacc.Bacc("TRN2", target_bir_lowering=False, debug=False,
                   num_devices=NC)
    wemb = nc.dram_tensor("wemb", [50000, 300], F32, kind="ExternalInput").ap()
    pemb = nc.dram_tensor("pemb", [50, 100], F32, kind="ExternalInput").ap()
    widx = nc.dram_tensor("widx", [N, 1], I32, kind="ExternalInput").ap()
    pidx = nc.dram_tensor("pidx", [N, 1], I32, kind="ExternalInput").ap()
    whhTd = nc.dram_tensor("whhT", [4, 100, 6400], BF16, kind="ExternalInput").ap()
    wih0Td = nc.dram_tensor("wih0T", [2, 100, 6400], BF16, kind="ExternalInput").ap()
    wih1Td = nc.dram_tensor("wih1T", [2, 100, 12800], BF16, kind="ExternalInput").ap()
    biasTd = nc.dram_tensor("biasT", [4, 100, 512], F32, kind="ExternalInput").ap()
    id100d = nc.dram_tensor("id100", [100, 100], BF16, kind="ExternalInput").ap()
    uhTd = nc.dram_tensor("uhT", [100, 800], BF16, kind="ExternalInput").ap()
    umTd = nc.dram_tensor("umT", [100, 800], BF16, kind="ExternalInput").ap()
    b1rowd = nc.dram_tensor("b1row", [1, 100], F32, kind="ExternalInput").ap()
    w2d = nc.dram_tensor("w2", [100, 1], BF16, kind="ExternalInput").ap()
    b2d = nc.dram_tensor("b2", [128, 1], F32, kind="ExternalInput").ap()
    selTd = nc.dram_tensor("selT", [2, 128, 32], F32, kind="ExternalInput").ap()
    grid = nc.dram_tensor("grid", [32, N], F32, kind="ExternalOutput").ap()

    from contextlib import ExitStack
    with TileContext(nc) as tc, ExitStack() as ctx:
        top = ctx.enter_context(tc.tile_pool(name="top", bufs=1))
        # ---- persistent SBUF tiles (DMAs emitted in priority order) ----
        idn = top.tile([128, 128], F32, name="idn", tag="idn")
        make_identity(nc, idn[:, :])
        ones_sb = top.tile([1, 256], F32, name="ones", tag="ones")
        nc.gpsimd.memset(ones_sb[:, :], 1.0)
        whh_sb = [top.tile([100, 6400], BF16, name=f"whh{dl}", tag=f"whh{dl}")
                  for dl in range(4)]
        bias_sb = [top.tile([100, 512], F32, name=f"bias{dl}", tag=f"bias{dl}")
                   for dl in range(4)]
        id_sb = top.tile([100, 100], BF16, name="id100", tag="id100")
        wih1_sb = [top.tile([100, 12800], BF16, name=f"wih1{d}", tag=f"wih1{d}")
                   for d in range(2)]
        xg_sbs = [top.tile([100, 4096], BF16, name=f"xg{d}", tag=f"xg{d}")
                  for d in range(2)]
        H = [[top.tile([100, 4 * N], BF16, name=f"H{l}{d}", tag=f"H{l}{d}")
              for d in range(2)] for l in range(2)]
        xT = top.tile([100, 4 * N], BF16, name="xT", tag="xT")
        wih0_sb = [top.tile([100, 6400], BF16, name=f"wih0{d}",
                            tag=f"wih0{d}") for d in range(2)]
        if STEPS < N:
            for l in range(2):
                for d in range(2):
                    nc.gpsimd.memset(H[l][d][:, :], 0.0)

        # ========= embedding gather (first DMAs in the queue) =========
        with tc.tile_pool(name="wih0p", bufs=1) as w0p, \
             tc.tile_pool(name="embps", bufs=2, space="PSUM") as eps:
            idx_sb = w0p.tile([128, 4], I32, name="idx", tag="idx")
            nc.sync.dma_start(out=idx_sb[0:128, 0:1], in_=widx[0:128, 0:1])
            nc.sync.dma_start(out=idx_sb[0:128, 1:2], in_=widx[128:256, 0:1])
            nc.sync.dma_start(out=idx_sb[0:128, 2:3], in_=pidx[0:128, 0:1])
            nc.sync.dma_start(out=idx_sb[0:128, 3:4], in_=pidx[128:256, 0:1])
            x_sb = w0p.tile([128, 800], F32, name="xsb", tag="xsb")
            for cch in range(2):
                nc.gpsimd.indirect_dma_start(
                    out=x_sb[0:128, 400 * cch: 400 * cch + 300],
                    out_offset=None,
                    in_=wemb[:, :],
                    in_offset=IndirectOffsetOnAxis(
                        ap=idx_sb[0:128, cch:cch + 1], axis=0))
                nc.gpsimd.indirect_dma_start(
                    out=x_sb[0:128, 400 * cch + 300: 400 * cch + 400],
                    out_offset=None,
                    in_=pemb[:, :],
                    in_offset=IndirectOffsetOnAxis(
                        ap=idx_sb[0:128, 2 + cch:3 + cch], axis=0))
            # layer-0 weights + rec0 needs, in DMA-queue priority order
            nc.sync.dma_start(out=bias_sb[0][:, :], in_=biasTd[0])
            nc.sync.dma_start(out=id_sb[:, :], in_=id100d[:, :])
            nc.sync.dma_start(out=wih0_sb[0][:, :], in_=wih0Td[0])
            nc.sync.dma_start(out=whh_sb[0][:, :], in_=whhTd[0])
            nc.sync.dma_start(out=bias_sb[1][:, :], in_=biasTd[1])
            nc.sync.dma_start(out=wih0_sb[1][:, :], in_=wih0Td[1])
            nc.sync.dma_start(out=whh_sb[1][:, :], in_=whhTd[1])

            # x -> xT transpose
            for cch in range(2):
                for uc in range(4):
                    ptr = eps.tile([128, 128], F32, name="ptr", tag="ptr")
                    nc.tensor.transpose(
                        out=ptr[0:100, 0:128],
                        in_=x_sb[0:128, 400 * cch + 100 * uc:
                                 400 * cch + 100 * uc + 100],
                        identity=idn[:, :])
                    nc.vector.tensor_copy(
                        out=xT[0:100, 256 * uc + 128 * cch:
                               256 * uc + 128 * cch + 128],
                        in_=ptr[0:100, 0:128])

            # ========= layer 0 xg: only the chunks needed at rec0 start
            # (rest are interleaved into rec0's idle PE time) =========
            def rhs_l0(d, uc, tc):
                return xT[0:P, 256 * uc + 32 * tc: 256 * uc + 32 * tc + 32]

            with tc.tile_pool(name="xg0ps", bufs=2, space="PSUM") as xg_ps:
                _emit_xg_group(nc, 4, [0, 1, 2, 3], wih0_sb[0], rhs_l0,
                               0, 0, xg_sbs[0], bias_sb[0], xg_ps, "xgps")
                _emit_xg_group(nc, 4, [0, 1, 2, 3], wih0_sb[1], rhs_l0,
                               1, 7, xg_sbs[1], bias_sb[1], xg_ps, "xgps")

        # remaining big DMAs: execute during rec0
        for d in range(2):
            nc.sync.dma_start(out=wih1_sb[d][:, :], in_=wih1Td[d])
        for dl in range(2, 4):
            nc.sync.dma_start(out=whh_sb[dl][:, :], in_=whhTd[dl])
            nc.sync.dma_start(out=bias_sb[dl][:, :], in_=biasTd[dl])

        # ========= rec0 with layer-1 xg interleaved =========
        def rhs_l1(d, uc, tc):
            src = H[0][uc // 4]
            j = uc % 4
            return src[0:P, 128 * tc + j: 128 * tc + 128: 4]

        with tc.tile_pool(name="xg1buf", bufs=1) as xgbuf, \
             tc.tile_pool(name="xg1ps", bufs=2, space="PSUM") as xg1_ps:
            xgf_sbs = [xgbuf.tile([100, 4096], BF16, name=f"xgf{d}",
                                  tag=f"xgf{d}") for d in range(2)]
            xgb_sbs = [xgbuf.tile([100, 4096], BF16, name=f"xgb{d}",
                                  tag=f"xgb{d}") for d in range(2)]

            def mk_group(d, tc_, half):
                def emit():
                    _emit_xg_group(
                        nc, 8, list(range(4 * half, 4 * half + 4)),
                        wih1_sb[d], rhs_l1, d, tc_,
                        xgf_sbs[d] if half == 0 else xgb_sbs[d],
                        bias_sb[2 + d] if half == 0 else None,
                        xg1_ps, "xg1ps")
                return emit

            def mk_group0(d, tc_):
                def emit():
                    _emit_xg_group(nc, 4, [0, 1, 2, 3], wih0_sb[d], rhs_l0,
                                   d, tc_, xg_sbs[d], bias_sb[d], xg1_ps,
                                   "xg1ps")
                return emit

            # xg0 leftovers first (avail immediately), ordered by deadline:
            # fwd chunk tc needed by wall step 32*tc, bwd chunk by 224-32*tc
            pend = []
            for tc_ in range(1, 8):
                pend.append((0, mk_group0(0, tc_)))        # deadline 32*tc_
                pend.append((0, mk_group0(1, 7 - tc_)))    # same deadline
            for d in range(2):
                for tc_ in range(8):
                    pend.append((32 * tc_ + 32, mk_group(d, tc_, 0)))
                    pend.append((N - 32 * tc_, mk_group(d, tc_, 1)))
            pend.sort(key=lambda x: x[0])
            # end-gated groups: rec1 needs xgb[0]c0 / xgf[1]c7 at its step 0,
            # but xgf[0]c7 / xgb[1]c0 only by step ~224 -- emit those two
            # inside rec1's idle time instead
            pend = [e for e in pend if e[0] < N]
            urgent = [mk_group(1, 7, 0), mk_group(0, 0, 1)]
            late = [mk_group(0, 7, 0), mk_group(1, 0, 1)]
            st = {"i": 0, "last": -10}

            def extra(t):
                n_emit = 0
                while (st["i"] < len(pend) and pend[st["i"]][0] <= t
                       and t > st["last"] and n_emit < 2):
                    pend[st["i"]][1]()
                    st["i"] += 1
                    n_emit += 1
                if n_emit:
                    st["last"] = t

            with tc.tile_pool(name="rst0", bufs=1) as state_pool, \
                 tc.tile_pool(name="sg0", bufs=4) as sg_pool, \
                 tc.tile_pool(name="tmp0", bufs=4) as tmp_pool, \
                 tc.tile_pool(name="rec0ps", bufs=3, space="PSUM") as rec_ps:
                _emit_rec(nc, 0, whh_sb[0:2], xg_sbs, id_sb, H[0],
                          (state_pool, sg_pool, tmp_pool, rec_ps),
                          extra=extra)
            # leftover in-rec0 groups, then the two urgently needed ones
            while st["i"] < len(pend):
                pend[st["i"]][1]()
                st["i"] += 1
            for fn in urgent:
                fn()

            st1 = {"i": 0, "last": -10}

            def extra1(t):
                if st1["i"] < len(late) and t - st1["last"] >= 2:
                    late[st1["i"]]()
                    st1["i"] += 1
                    st1["last"] = t

            # ========= rec1 (dual injection: fwd + bwd halves) =========
            with tc.tile_pool(name="rst1", bufs=1) as state_pool, \
                 tc.tile_pool(name="sg1", bufs=4) as sg_pool, \
                 tc.tile_pool(name="tmp1", bufs=4) as tmp_pool, \
                 tc.tile_pool(name="rec1ps", bufs=3, space="PSUM") as rec_ps:
                _emit_rec(nc, 1, whh_sb[2:4], xgf_sbs, id_sb, H[1],
                          (state_pool, sg_pool, tmp_pool, rec_ps),
                          xg2_sbs=xgb_sbs, extra=extra1)
            while st1["i"] < len(late):
                late[st1["i"]]()
                st1["i"] += 1

        # ========= edge scorer =========
        with tc.tile_pool(name="edge", bufs=1) as ep, \
             tc.tile_pool(name="edgeth", bufs=4) as thp, \
             tc.tile_pool(name="edgeps", bufs=1, space="PSUM") as epps, \
             tc.tile_pool(name="edgepsS", bufs=1, space="PSUM") as spps:
            uh_sb = ep.tile([100, 800], BF16, name="uhT", tag="uhT")
            nc.sync.dma_start(out=uh_sb[:, :], in_=uhTd[:, :])
            um_sb = ep.tile([100, 800], BF16, name="umT", tag="umT")
            nc.sync.dma_start(out=um_sb[:, :], in_=umTd[:, :])
            b1_sb = ep.tile([1, 100], F32, name="b1row", tag="b1row")
            nc.sync.dma_start(out=b1_sb[:, :], in_=b1rowd[:, :])
            w2_sb = ep.tile([100, 1], BF16, name="w2", tag="w2")
            nc.sync.dma_start(out=w2_sb[:, :], in_=w2d[:, :])
            b2_sb = ep.tile([128, 1], F32, name="b2", tag="b2")
            nc.sync.dma_start(out=b2_sb[:, :], in_=b2d[:, :])
            selT_sb = ep.tile([128, 64], F32, name="selT", tag="selT")
            nc.sync.dma_start(out=selT_sb[0:128, 0:32], in_=selTd[0])
            nc.sync.dma_start(out=selT_sb[0:128, 32:64], in_=selTd[1])

            def h1_rhs(uc):
                return H[1][uc // 4][0:P, uc % 4: 4 * N: 4]

            # A^T [100, 256] (head half of fc1)
            pA = epps.tile([128, 512], F32, name="e1", tag="e1")
            for uc in range(8):
                nc.tensor.matmul(
                    pA[0:P, 0:256],
                    lhsT=uh_sb[0:P, 100 * uc: 100 * uc + 100],
                    rhs=h1_rhs(uc),
                    start=(uc == 0), stop=(uc == 7), skip_group_check=True)
            A_sb = ep.tile([100, 256], F32, name="A", tag="A")
            nc.vector.tensor_copy(out=A_sb[0:P, 0:256], in_=pA[0:P, 0:256])
            # B^T [100, 256] + b1 (modifier half)
            pB = epps.tile([128, 512], F32, name="e3", tag="e3")
            for uc in range(8):
                nc.tensor.matmul(
                    pB[0:P, 0:256],
                    lhsT=um_sb[0:P, 100 * uc: 100 * uc + 100],
                    rhs=h1_rhs(uc),
                    start=(uc == 0), stop=False, skip_group_check=True)
            nc.tensor.matmul(
                pB[0:P, 0:256],
                lhsT=b1_sb[0:1, 0:100],
                rhs=ones_sb[0:1, 0:256],
                start=False, stop=True, skip_group_check=True)
            B_sb = ep.tile([100, 256], BF16, name="Bsb", tag="Bsb")
            nc.vector.tensor_copy(out=B_sb[0:P, 0:256], in_=pB[0:P, 0:256])
            # A -> token-major via transpose, then per-core 32-head select
            A_tok = ep.tile([128, 256], F32, name="Atok", tag="Atok")
            for m in range(2):
                pT = epps.tile([128, 512], F32, name="e2", tag="e2")
                nc.tensor.transpose(
                    out=pT[0:128, 0:100],
                    in_=A_sb[0:100, 128 * m: 128 * m + 128],
                    identity=idn[0:100, 0:100])
                nc.vector.tensor_copy(
                    out=A_tok[0:128, 128 * m: 128 * m + 100],
                    in_=pT[0:128, 0:100])
            pS = epps.tile([128, 512], F32, name="e1", tag="e1")
            for m in range(2):
                nc.tensor.matmul(
                    pS[0:32, 0:100],
                    lhsT=selT_sb[0:128, 32 * m: 32 * m + 32],
                    rhs=A_tok[0:128, 128 * m: 128 * m + 100],
                    start=(m == 0), stop=(m == 1), skip_group_check=True)
            AselS = ep.tile([128, 128], F32, name="AselS", tag="AselS")
            nc.gpsimd.memset(AselS[:, :], 0.0)
            nc.vector.tensor_copy(out=AselS[0:32, 0:100], in_=pS[0:32, 0:100])
            pAT = epps.tile([128, 512], F32, name="e2", tag="e2")
            nc.tensor.transpose(out=pAT[0:128, 0:128],
                                in_=AselS[0:128, 0:128], identity=idn[:, :])
            AT_sb = ep.tile([128, 32], F32, name="AT", tag="AT")
            nc.vector.tensor_copy(out=AT_sb[0:128, 0:32], in_=pAT[0:128, 0:32])

            # per-head tanh + w2 dot
            psS_tiles = [spps.tile([128, 512], F32, name=f"psS{q}", tag=f"psS{q}")
                         for q in range(4)]
            for q in range(4):
                nc.vector.memset(psS_tiles[q][:, :], 0.0)
            gsb_tiles = [ep.tile([128, 512], F32, name=f"gsb{q}", tag=f"gsb{q}")
                         for q in range(4)]
            for r in range(32):
                th_t = thp.tile([100, 256], BF16, name=f"th{r % 4}",
                                tag=f"th{r % 4}")
                nc.scalar.activation(
                    th_t[0:100, 0:256], B_sb[0:100, 0:256], AF.Tanh,
                    bias=AT_sb[0:100, r:r + 1], scale=1.0)
                q, half = divmod(r // 4, 2)
                nc.tensor.matmul(
                    psS_tiles[q][32 * (r % 4): 32 * (r % 4) + 1,
                                 256 * half: 256 * half + 256],
                    lhsT=w2_sb[0:100, 0:1],
                    rhs=th_t[0:100, 0:256],
                    start=True, stop=True,
                    skip_group_check=True,
                    tile_position=(0, 32 * (r % 4)))
            for q in range(4):
                nc.vector.tensor_scalar(
                    out=gsb_tiles[q][0:128, 0:512],
                    in0=psS_tiles[q][0:128, 0:512],
                    scalar1=b2_sb[0:128, 0:1], scalar2=None, op0=OP.add)
                for half in range(2):
                    rb = 4 * (2 * q + half)
                    nc.sync.dma_start(
                        out=grid[rb:rb + 4, 0:256],
                        in_=gsb_tiles[q][0:128:32, 256 * half: 256 * half + 256])

    nc.compile()
    return nc


_NC_CACHE = None


def _get_nc():
    global _NC_CACHE
    if _NC_CACHE is None:
        _NC_CACHE = build_nc()
    return _NC_CACHE


def kernel(**inputs) -> np.ndarray:
    from concourse.bass_utils import run_bass_kernel_spmd

    arr = _prep_inputs(**inputs)
    nc = _get_nc()
    in_maps = []
    for k in range(NC):
        m = dict(arr)
        m["selT"] = _make_selT(k)
        in_maps.append(m)
    res = run_bass_kernel_spmd(nc, in_maps, core_ids=list(range(NC)))
    grid = np.concatenate([res.results[k]["grid"] for k in range(NC)], axis=0)
    mask = np.ones((N, N), dtype=bool)
    np.fill_diagonal(mask, False)
    mask[:, 0] = False
    return grid[mask].reshape(-1, 1).astype(np.float32)


# revision 46
# speedup vs baseline: 3.3036x; 1.0021x over previous
"""Trainium2 Bass kernel: BiLSTM dependency-parser edge scorer (v2).

Self-contained. Accepts FULL inputs (as produced by setup_inputs()), returns
the FULL [65280, 1] float32 score tensor.

Key idea vs v1: all recurrence matmuls are WEIGHTS-STATIONARY (weights in
lhsT, the tiny h vector streams as rhs), so each step's 64 gate matmuls have
output free-size 1 instead of streaming 6400 PSUM rows.

Layouts (per direction d, layer l):
  gates PSUM tile [100, 16]: partition p, col n = 4*j + g where the LSTM
    unit is u = 100*j + p (j in 0..4) and g in {0:i, 1:f, 2:g, 3:o}.
  h storage H[l][d] [100, 4*256] bf16: h_t for unit (j, p) at col 4*t + j.
    Column 4*t+j is directly the rhs [100, 1] for K-chunk j of the next
    step's matmul -- no transpose inside the loop.
  c state [100, 4] f32.
  xg_sb[d] [100, 16*256] bf16: precomputed input projections + bias,
    injected into the PSUM accumulation via an identity-weight matmul.
g-gate rows are pre-scaled by 2 on host: tanh(x) = 2*sigmoid(2x) - 1.
"""

import os
import sys

sys.path.insert(0, "/opt/trn_rl_repo")

import numpy as np

import concourse.bass as bass
import concourse.mybir as mybir
from concourse import bacc
from concourse.bass import IndirectOffsetOnAxis
from concourse.masks import make_identity
from concourse.tile import TileContext

N = 256          # sequence length
HID = 400        # hidden per direction
NC = 8           # cores
P = 100          # partitions used for unit math
NG = 16          # gate cols per step
F32 = mybir.dt.float32
BF16 = mybir.dt.float16
I32 = mybir.dt.int32
AF = mybir.ActivationFunctionType
OP = mybir.AluOpType

STEPS = int(os.environ.get("DP_STEPS", str(N)))


# ---------------------------------------------------------------------------
# host-side weight layout prep
# ---------------------------------------------------------------------------

def _bf(a):
    return np.ascontiguousarray(np.asarray(a).astype(np.float16))


# R[p, n] = original torch gate-row for (partition p, col n)
_PP, _NN = np.meshgrid(np.arange(P), np.arange(NG), indexing="ij")
_R = 400 * (_NN // 4) + 100 * (_NN % 4) + _PP      # [100, 16]


def _scale_g(W):
    """Scale g-gate rows (orig rows 800:1200) by 2."""
    Ws = np.array(W, dtype=np.float64)
    Ws[800:1200] *= 2.0
    return Ws


def _wblocks(W, nuc):
    """W: [1600, U] scaled gate-major weights, U = 100*nuc.
    Returns [100, 16*nuc*100]: block (n, uc) at cols (n*nuc+uc)*100 holds
    lhsT[k, m] = W[R[m, n], 100*uc + k]."""
    arr = W[_R]                                    # [100p, 16n, U]
    A4 = arr.reshape(P, NG, nuc, 100)              # [p, n, uc, k]
    return A4.transpose(3, 1, 2, 0).reshape(100, NG * nuc * 100)


def _prep_inputs(word_idx, pos_idx, word_emb, pos_emb,
                 Wih0, Whh0, bih0, bhh0, Wih1, Whh1, bih1, bhh1,
                 fc1_W, fc1_b, fc2_W, fc2_b):
    arr = {}
    arr["widx"] = np.ascontiguousarray(
        np.asarray(word_idx).reshape(N, 1).astype(np.int32))
    arr["pidx"] = np.ascontiguousarray(
        np.asarray(pos_idx).reshape(N, 1).astype(np.int32))
    arr["wemb"] = np.ascontiguousarray(np.asarray(word_emb, dtype=np.float32))
    arr["pemb"] = np.ascontiguousarray(np.asarray(pos_emb, dtype=np.float32))

    Wih = [np.asarray(Wih0, np.float64), np.asarray(Wih1, np.float64)]
    Whh = [np.asarray(Whh0, np.float64), np.asarray(Whh1, np.float64)]
    bih = [np.asarray(bih0, np.float64), np.asarray(bih1, np.float64)]
    bhh = [np.asarray(bhh0, np.float64), np.asarray(bhh1, np.float64)]

    whhT = np.zeros((4, 100, NG * 4 * 100), np.float32)
    biasT = np.zeros((4, 100, 512), np.float32)
    wih0T = np.zeros((2, 100, NG * 4 * 100), np.float32)
    wih1T = np.zeros((2, 100, NG * 8 * 100), np.float32)
    for l in range(2):
        for d in range(2):
            dl = 2 * l + d
            whhT[dl] = _wblocks(_scale_g(Whh[l][d]), 4)
            b = _scale_g(bih[l][d] + bhh[l][d])[_R]          # [100, 16]
            biasT[dl] = np.tile(b, (1, 32)).astype(np.float32)
    for d in range(2):
        wih0T[d] = _wblocks(_scale_g(Wih[0][d]), 4)
        wih1T[d] = _wblocks(_scale_g(Wih[1][d]), 8)
    arr["whhT"] = _bf(whhT)
    arr["biasT"] = np.ascontiguousarray(biasT)
    arr["wih0T"] = _bf(wih0T)
    arr["wih1T"] = _bf(wih1T)

    # identity for the xg injection matmul
    arr["id100"] = _bf(np.eye(P, dtype=np.float32))

    # edge MLP: uhT/umT [100, 800]: block uc at cols 100*uc holds
    # lhsT[k, a] = fc1_W[a, 100*uc + k]
    f1 = np.asarray(fc1_W, np.float64)               # [100, 1600]
    arr["uhT"] = _bf(np.concatenate(
        [f1[:, 100 * u:100 * u + 100].T for u in range(8)], axis=1))
    arr["umT"] = _bf(np.concatenate(
        [f1[:, 800 + 100 * u:800 + 100 * u + 100].T for u in range(8)],
        axis=1))
    arr["b1row"] = np.ascontiguousarray(
        np.asarray(fc1_b, np.float32).reshape(1, 100))
    arr["w2"] = _bf(np.asarray(fc2_W, np.float32).reshape(100, 1))
    arr["b2"] = np.ascontiguousarray(
        np.full((128, 1), np.float32(np.asarray(fc2_b).reshape(())),
                dtype=np.float32))
    return arr


def _make_selT(core):
    s = np.zeros((2, 128, 32), np.float32)
    for r in range(32):
        t = 32 * core + r
        s[t // 128, t % 128, r] = 1.0
    return np.ascontiguousarray(s)


# ---------------------------------------------------------------------------
# device kernel build
# ---------------------------------------------------------------------------

def _emit_xg_group(nc, nuc, ucs, wih_sb_d, rhs_chunk, d, tc, dst_sb,
                   bias_sb, ps_pool, tag):
    """One t-chunk (32 tokens) of an input-projection GEMM: 16*len(ucs)
    weights-stationary matmuls accumulating into a PSUM bank, then one
    PSUM->SBUF copy (adding bias if given)."""
    ps = ps_pool.tile([128, 512], F32, name=tag, tag=tag)
    for n in range(NG):
        for i, uc in enumerate(ucs):
            nc.tensor.matmul(
                ps[0:P, n:512:16],
                lhsT=wih_sb_d[0:P, (n * nuc + uc) * 100:
                              (n * nuc + uc) * 100 + 100],
                rhs=rhs_chunk(d, uc, tc),
                start=(i == 0), stop=(i == len(ucs) - 1),
                skip_group_check=True)
    if bias_sb is not None:
        nc.vector.tensor_tensor(
            out=dst_sb[0:P, 512 * tc: 512 * tc + 512],
            in0=ps[0:P, 0:512], in1=bias_sb[0:P, 0:512], op=OP.add)
    else:
        nc.vector.tensor_copy(
            out=dst_sb[0:P, 512 * tc: 512 * tc + 512],
            in_=ps[0:P, 0:512])


def _emit_xg(nc, l, wih_sb, rhs_chunk, xg_sbs, bias_sbs, ps_pool):
    """Full xg for layer l: xg[d][p, 16*t + n] = sum_u W[r(p,n), u]*in[t,u]+b."""
    nuc = 4 if l == 0 else 8
    for d in range(2):
        for tc in range(8):
            _emit_xg_group(nc, nuc, list(range(nuc)), wih_sb[d], rhs_chunk,
                           d, tc, xg_sbs[d], bias_sbs[d], ps_pool, "xgps")


def _emit_rec(nc, l, whh_sb, xg_sbs, id_sb, H_out, pools, xg2_sbs=None,
              extra=None):
    """STEPS wall-steps, both directions interleaved. xg2_sbs: optional
    second injection source (bwd-half input projections for layer 1).
    extra(t): called after each wall-step to emit overlapped work."""
    state_pool, sg_pool, tmp_pool, ps_pool = pools
    cs = []
    for d in range(2):
        c = state_pool.tile([P, 4], F32, name=f"c{d}", tag=f"c{d}")
        nc.gpsimd.memset(c[:, :], 0.0)
        cs.append(c)

    for t in range(STEPS):
        ps_t, sg_t, th_t = {}, {}, {}
        dorder = (0, 1)
        # --- PE: injection + 64 weight matmuls per direction ---
        for d in dorder:
            tdx = t if d == 0 else (STEPS - 1 - t)
            ps = ps_pool.tile([128, 512], F32, name=f"ps{d}", tag=f"ps{d}")
            ps_t[d] = ps
            first = (t == 0)
            nc.tensor.matmul(
                ps[0:P, 0:NG],
                lhsT=id_sb[0:P, 0:P],
                rhs=xg_sbs[d][0:P, NG * tdx: NG * tdx + NG],
                start=True, stop=(first and xg2_sbs is None),
                skip_group_check=True)
            if xg2_sbs is not None:
                nc.tensor.matmul(
                    ps[0:P, 0:NG],
                    lhsT=id_sb[0:P, 0:P],
                    rhs=xg2_sbs[d][0:P, NG * tdx: NG * tdx + NG],
                    start=False, stop=first, skip_group_check=True)
            if not first:
                pdx = tdx - 1 if d == 0 else tdx + 1
                # uc-major: uc 0/1 depend only on the first h half-write, so
                # their issue overlaps the second half-write
                for uc in range(4):
                    for n in range(NG):
                        nc.tensor.matmul(
                            ps[0:P, n:n + 1],
                            lhsT=whh_sb[d][0:P, (4 * n + uc) * 100:
                                           (4 * n + uc) * 100 + 100],
                            rhs=H_out[d][0:P, 4 * pdx + uc: 4 * pdx + uc + 1],
                            start=False, stop=(uc == 3),
                            skip_group_check=True)
        # --- Act: sigmoid over all gates (g pre-scaled by 2) ---
        for d in dorder:
            sg = sg_pool.tile([P, NG], F32, name=f"sg{d}", tag=f"sg{d}")
            sg_t[d] = sg
            nc.scalar.activation(sg[0:P, 0:NG], ps_t[d][0:P, 0:NG], AF.Sigmoid)
        # --- DVE per direction: c = sig(f)*c + sig(i)*(2*sig(2g) - 1),
        #     then tanh(c) via Pade [3/2] (|c| < 0.5 here, err < 1e-6):
        #     tanh(c) ~= c*(15 + c^2) / (15 + 6*c^2);  h = sig(o)*tanh ---
        for d in dorder:
            tdx = t if d == 0 else (STEPS - 1 - t)
            sg, c = sg_t[d], cs[d]
            # c = sig(f)*c + sig(i)*(2*sig(2g)-1), fused as:
            #   q = (sig_g - 0.5)*sig_i;  c = 2*q + sig_f*c
            q = tmp_pool.tile([P, 4], F32, name=f"q{d}", tag=f"q{d}")
            cf = tmp_pool.tile([P, 4], F32, name=f"cf{d}", tag=f"cf{d}")
            nc.vector.scalar_tensor_tensor(
                out=q[0:P, 0:4], in0=sg[0:P, 8:12], scalar=0.5,
                in1=sg[0:P, 0:4], op0=OP.subtract, op1=OP.mult)
            nc.vector.tensor_tensor(
                out=cf[0:P, 0:4], in0=sg[0:P, 4:8],
                in1=c[0:P, 0:4], op=OP.mult)
            nc.vector.scalar_tensor_tensor(
                out=c[0:P, 0:4], in0=q[0:P, 0:4], scalar=2.0,
                in1=cf[0:P, 0:4], op0=OP.mult, op1=OP.add)
            th = tmp_pool.tile([P, 4], F32, name=f"th{d}", tag=f"th{d}")
            th_t[d] = th
            nc.scalar.activation(th[0:P, 0:4], c[0:P, 0:4], AF.Tanh)
        for d in dorder:
            tdx = t if d == 0 else (STEPS - 1 - t)
            nc.vector.tensor_tensor(
                out=H_out[d][0:P, 4 * tdx: 4 * tdx + 2],
                in0=sg_t[d][0:P, 12:14], in1=th_t[d][0:P, 0:2], op=OP.mult)
            nc.vector.tensor_tensor(
                out=H_out[d][0:P, 4 * tdx + 2: 4 * tdx + 4],
                in0=sg_t[d][0:P, 14:NG], in1=th_t[d][0:P, 2:4], op=OP.mult)
        if extra is not None:
            extra(t)


def build_nc():
    nc = bacc.Bacc("TRN2", target_bir_lowering=False, debug=False,
                   num_devices=NC)
    wemb = nc.dram_tensor("wemb", [50000, 300], F32, kind="ExternalInput").ap()
    pemb = nc.dram_tensor("pemb", [50, 100], F32, kind="ExternalInput").ap()
    widx = nc.dram_tensor("widx", [N, 1], I32, kind="ExternalInput").ap()
    pidx = nc.dram_tensor("pidx", [N, 1], I32, kind="ExternalInput").ap()
    whhTd = nc.dram_tensor("whhT", [4, 100, 6400], BF16, kind="ExternalInput").ap()
    wih0Td = nc.dram_tensor("wih0T", [2, 100, 6400], BF16, kind="ExternalInput").ap()
    wih1Td = nc.dram_tensor("wih1T", [2, 100, 12800], BF16, kind="ExternalInput").ap()
    biasTd = nc.dram_tensor("biasT", [4, 100, 512], F32, kind="ExternalInput").ap()
    id100d = nc.dram_tensor("id100", [100, 100], BF16, kind="ExternalInput").ap()
    uhTd = nc.dram_tensor("uhT", [100, 800], BF16, kind="ExternalInput").ap()
    umTd = nc.dram_tensor("umT", [100, 800], BF16, kind="ExternalInput").ap()
    b1rowd = nc.dram_tensor("b1row", [1, 100], F32, kind="ExternalInput").ap()
    w2d = nc.dram_tensor("w2", [100, 1], BF16, kind="ExternalInput").ap()
    b2d = nc.dram_tensor("b2", [128, 1], F32, kind="ExternalInput").ap()
    selTd = nc.dram_tensor("selT", [2, 128, 32], F32, kind="ExternalInput").ap()
    grid = nc.dram_tensor("grid", [32, N], F32, kind="ExternalOutput").ap()

    from contextlib import ExitStack
    with TileContext(nc) as tc, ExitStack() as ctx:
        top = ctx.enter_context(tc.tile_pool(name="top", bufs=1))
        # ---- persistent SBUF tiles (DMAs emitted in priority order) ----
        idn = top.tile([128, 128], F32, name="idn", tag="idn")
        make_identity(nc, idn[:, :])
        ones_sb = top.tile([1, 256], F32, name="ones", tag="ones")
        nc.gpsimd.memset(ones_sb[:, :], 1.0)
        whh_sb = [top.tile([100, 6400], BF16, name=f"whh{dl}", tag=f"whh{dl}")
                  for dl in range(4)]
        bias_sb = [top.tile([100, 512], F32, name=f"bias{dl}", tag=f"bias{dl}")
                   for dl in range(4)]
        id_sb = top.tile([100, 100], BF16, name="id100", tag="id100")
        wih1_sb = [top.tile([100, 12800], BF16, name=f"wih1{d}", tag=f"wih1{d}")
                   for d in range(2)]
        xg_sbs = [top.tile([100, 4096], BF16, name=f"xg{d}", tag=f"xg{d}")
                  for d in range(2)]
        H = [[top.tile([100, 4 * N], BF16, name=f"H{l}{d}", tag=f"H{l}{d}")
              for d in range(2)] for l in range(2)]
        xT = top.tile([100, 4 * N], BF16, name="xT", tag="xT")
        wih0_sb = [top.tile([100, 6400], BF16, name=f"wih0{d}",
                            tag=f"wih0{d}") for d in range(2)]
        if STEPS < N:
            for l in range(2):
                for d in range(2):
                    nc.gpsimd.memset(H[l][d][:, :], 0.0)

        # ========= embedding gather (first DMAs in the queue) =========
        with tc.tile_pool(name="wih0p", bufs=1) as w0p, \
             tc.tile_pool(name="embps", bufs=2, space="PSUM") as eps:
            idx_sb = w0p.tile([128, 4], I32, name="idx", tag="idx")
            nc.sync.dma_start(out=idx_sb[0:128, 0:1], in_=widx[0:128, 0:1])
            nc.sync.dma_start(out=idx_sb[0:128, 1:2], in_=widx[128:256, 0:1])
            nc.sync.dma_start(out=idx_sb[0:128, 2:3], in_=pidx[0:128, 0:1])
            nc.sync.dma_start(out=idx_sb[0:128, 3:4], in_=pidx[128:256, 0:1])
            x_sb = w0p.tile([128, 800], F32, name="xsb", tag="xsb")
            for cch in range(2):
                nc.gpsimd.indirect_dma_start(
                    out=x_sb[0:128, 400 * cch: 400 * cch + 300],
                    out_offset=None,
                    in_=wemb[:, :],
                    in_offset=IndirectOffsetOnAxis(
                        ap=idx_sb[0:128, cch:cch + 1], axis=0))
                nc.gpsimd.indirect_dma_start(
                    out=x_sb[0:128, 400 * cch + 300: 400 * cch + 400],
                    out_offset=None,
                    in_=pemb[:, :],
                    in_offset=IndirectOffsetOnAxis(
                        ap=idx_sb[0:128, 2 + cch:3 + cch], axis=0))
            # layer-0 weights + rec0 needs, in DMA-queue priority order
            nc.sync.dma_start(out=bias_sb[0][:, :], in_=biasTd[0])
            nc.sync.dma_start(out=id_sb[:, :], in_=id100d[:, :])
            nc.sync.dma_start(out=wih0_sb[0][:, :], in_=wih0Td[0])
            nc.sync.dma_start(out=whh_sb[0][:, :], in_=whhTd[0])
            nc.sync.dma_start(out=bias_sb[1][:, :], in_=biasTd[1])
            nc.sync.dma_start(out=wih0_sb[1][:, :], in_=wih0Td[1])
            nc.sync.dma_start(out=whh_sb[1][:, :], in_=whhTd[1])

            # x -> xT transpose
            for cch in range(2):
                for uc in range(4):
                    ptr = eps.tile([128, 128], F32, name="ptr", tag="ptr")
                    nc.tensor.transpose(
                        out=ptr[0:100, 0:128],
                        in_=x_sb[0:128, 400 * cch + 100 * uc:
                                 400 * cch + 100 * uc + 100],
                        identity=idn[:, :])
                    nc.vector.tensor_copy(
                        out=xT[0:100, 256 * uc + 128 * cch:
                               256 * uc + 128 * cch + 128],
                        in_=ptr[0:100, 0:128])

            # ========= layer 0 xg: only the chunks needed at rec0 start
            # (rest are interleaved into rec0's idle PE time) =========
            def rhs_l0(d, uc, tc):
                return xT[0:P, 256 * uc + 32 * tc: 256 * uc + 32 * tc + 32]

            with tc.tile_pool(name="xg0ps", bufs=2, space="PSUM") as xg_ps:
                _emit_xg_group(nc, 4, [0, 1, 2, 3], wih0_sb[0], rhs_l0,
                               0, 0, xg_sbs[0], bias_sb[0], xg_ps, "xgps")
                _emit_xg_group(nc, 4, [0, 1, 2, 3], wih0_sb[1], rhs_l0,
                               1, 7, xg_sbs[1], bias_sb[1], xg_ps, "xgps")

        # remaining big DMAs: execute during rec0
        for d in range(2):
            nc.sync.dma_start(out=wih1_sb[d][:, :], in_=wih1Td[d])
        for dl in range(2, 4):
            nc.sync.dma_start(out=whh_sb[dl][:, :], in_=whhTd[dl])
            nc.sync.dma_start(out=bias_sb[dl][:, :], in_=biasTd[dl])

        # ========= rec0 with layer-1 xg interleaved =========
        def rhs_l1(d, uc, tc):
            src = H[0][uc // 4]
            j = uc % 4
            return src[0:P, 128 * tc + j: 128 * tc + 128: 4]

        with tc.tile_pool(name="xg1buf", bufs=1) as xgbuf, \
             tc.tile_pool(name="xg1ps", bufs=2, space="PSUM") as xg1_ps:
            xgf_sbs = [xgbuf.tile([100, 4096], BF16, name=f"xgf{d}",
                                  tag=f"xgf{d}") for d in range(2)]
            xgb_sbs = [xgbuf.tile([100, 4096], BF16, name=f"xgb{d}",
                                  tag=f"xgb{d}") for d in range(2)]

            def mk_group(d, tc_, half):
                def emit():
                    _emit_xg_group(
                        nc, 8, list(range(4 * half, 4 * half + 4)),
                        wih1_sb[d], rhs_l1, d, tc_,
                        xgf_sbs[d] if half == 0 else xgb_sbs[d],
                        bias_sb[2 + d] if half == 0 else None,
                        xg1_ps, "xg1ps")
                return emit

            def mk_group0(d, tc_):
                def emit():
                    _emit_xg_group(nc, 4, [0, 1, 2, 3], wih0_sb[d], rhs_l0,
                                   d, tc_, xg_sbs[d], bias_sb[d], xg1_ps,
                                   "xg1ps")
                return emit

            # xg0 leftovers first (avail immediately), ordered by deadline:
            # fwd chunk tc needed by wall step 32*tc, bwd chunk by 224-32*tc
            pend = []
            for tc_ in range(1, 8):
                pend.append((0, mk_group0(0, tc_)))        # deadline 32*tc_
                pend.append((0, mk_group0(1, 7 - tc_)))    # same deadline
            for d in range(2):
                for tc_ in range(8):
                    pend.append((32 * tc_ + 32, mk_group(d, tc_, 0)))
                    pend.append((N - 32 * tc_, mk_group(d, tc_, 1)))
            pend.sort(key=lambda x: x[0])
            # end-gated groups: rec1 needs xgb[0]c0 / xgf[1]c7 at its step 0,
            # but xgf[0]c7 / xgb[1]c0 only by step ~224 -- emit those two
            # inside rec1's idle time instead
            pend = [e for e in pend if e[0] < N]
            urgent = [mk_group(1, 7, 0), mk_group(0, 0, 1)]
            late = [mk_group(0, 7, 0), mk_group(1, 0, 1)]
            st = {"i": 0, "last": -10}

            def extra(t):
                n_emit = 0
                while (st["i"] < len(pend) and pend[st["i"]][0] <= t
                       and t > st["last"] and n_emit < 2):
                    pend[st["i"]][1]()
                    st["i"] += 1
                    n_emit += 1
                if n_emit:
                    st["last"] = t

            with tc.tile_pool(name="rst0", bufs=1) as state_pool, \
                 tc.tile_pool(name="sg0", bufs=4) as sg_pool, \
                 tc.tile_pool(name="tmp0", bufs=4) as tmp_pool, \
                 tc.tile_pool(name="rec0ps", bufs=3, space="PSUM") as rec_ps:
                _emit_rec(nc, 0, whh_sb[0:2], xg_sbs, id_sb, H[0],
                          (state_pool, sg_pool, tmp_pool, rec_ps),
                          extra=extra)
            # leftover in-rec0 groups, then the two urgently needed ones
            while st["i"] < len(pend):
                pend[st["i"]][1]()
                st["i"] += 1
            for fn in urgent:
                fn()

            st1 = {"i": 0, "last": -10}

            def extra1(t):
                if st1["i"] < len(late) and t - st1["last"] >= 2:
                    late[st1["i"]]()
                    st1["i"] += 1
                    st1["last"] = t

            # ========= rec1 (dual injection: fwd + bwd halves) =========
            with tc.tile_pool(name="rst1", bufs=1) as state_pool, \
                 tc.tile_pool(name="sg1", bufs=4) as sg_pool, \
                 tc.tile_pool(name="tmp1", bufs=4) as tmp_pool, \
                 tc.tile_pool(name="rec1ps", bufs=3, space="PSUM") as rec_ps:
                _emit_rec(nc, 1, whh_sb[2:4], xgf_sbs, id_sb, H[1],
                          (state_pool, sg_pool, tmp_pool, rec_ps),
                          xg2_sbs=xgb_sbs, extra=extra1)
            while st1["i"] < len(late):
                late[st1["i"]]()
                st1["i"] += 1

        # ========= edge scorer =========
        with tc.tile_pool(name="edge", bufs=1) as ep, \
             tc.tile_pool(name="edgeth", bufs=2) as thp, \
             tc.tile_pool(name="edgeps", bufs=1, space="PSUM") as epps, \
             tc.tile_pool(name="edgepsS", bufs=1, space="PSUM") as spps:
            uh_sb = ep.tile([100, 800], BF16, name="uhT", tag="uhT")
            nc.sync.dma_start(out=uh_sb[:, :], in_=uhTd[:, :])
            um_sb = ep.tile([100, 800], BF16, name="umT", tag="umT")
            nc.sync.dma_start(out=um_sb[:, :], in_=umTd[:, :])
            b1_sb = ep.tile([1, 100], F32, name="b1row", tag="b1row")
            nc.sync.dma_start(out=b1_sb[:, :], in_=b1rowd[:, :])
            w2_sb = ep.tile([100, 1], BF16, name="w2", tag="w2")
            nc.sync.dma_start(out=w2_sb[:, :], in_=w2d[:, :])
            b2_sb = ep.tile([128, 1], F32, name="b2", tag="b2")
            nc.sync.dma_start(out=b2_sb[:, :], in_=b2d[:, :])
            selT_sb = ep.tile([128, 64], F32, name="selT", tag="selT")
            nc.sync.dma_start(out=selT_sb[0:128, 0:32], in_=selTd[0])
            nc.sync.dma_start(out=selT_sb[0:128, 32:64], in_=selTd[1])

            def h1_rhs(uc):
                return H[1][uc // 4][0:P, uc % 4: 4 * N: 4]

            # A^T [100, 256] (head half of fc1)
            pA = epps.tile([128, 512], F32, name="e1", tag="e1")
            for uc in range(8):
                nc.tensor.matmul(
                    pA[0:P, 0:256],
                    lhsT=uh_sb[0:P, 100 * uc: 100 * uc + 100],
                    rhs=h1_rhs(uc),
                    start=(uc == 0), stop=(uc == 7), skip_group_check=True)
            A_sb = ep.tile([100, 256], F32, name="A", tag="A")
            nc.vector.tensor_copy(out=A_sb[0:P, 0:256], in_=pA[0:P, 0:256])
            # B^T [100, 256] + b1 (modifier half)
            pB = epps.tile([128, 512], F32, name="e3", tag="e3")
            for uc in range(8):
                nc.tensor.matmul(
                    pB[0:P, 0:256],
                    lhsT=um_sb[0:P, 100 * uc: 100 * uc + 100],
                    rhs=h1_rhs(uc),
                    start=(uc == 0), stop=False, skip_group_check=True)
            nc.tensor.matmul(
                pB[0:P, 0:256],
                lhsT=b1_sb[0:1, 0:100],
                rhs=ones_sb[0:1, 0:256],
                start=False, stop=True, skip_group_check=True)
            B_sb = ep.tile([100, 256], BF16, name="Bsb", tag="Bsb")
            nc.vector.tensor_copy(out=B_sb[0:P, 0:256], in_=pB[0:P, 0:256])
            # A -> token-major via transpose, then per-core 32-head select
            A_tok = ep.tile([128, 256], F32, name="Atok", tag="Atok")
            for m in range(2):
                pT = epps.tile([128, 512], F32, name="e2", tag="e2")
                nc.tensor.transpose(
                    out=pT[0:128, 0:100],
                    in_=A_sb[0:100, 128 * m: 128 * m + 128],
                    identity=idn[0:100, 0:100])
                nc.vector.tensor_copy(
                    out=A_tok[0:128, 128 * m: 128 * m + 100],
                    in_=pT[0:128, 0:100])
            pS = epps.tile([128, 512], F32, name="e1", tag="e1")
            for m in range(2):
                nc.tensor.matmul(
                    pS[0:32, 0:100],
                    lhsT=selT_sb[0:128, 32 * m: 32 * m + 32],
                    rhs=A_tok[0:128, 128 * m: 128 * m + 100],
                    start=(m == 0), stop=(m == 1), skip_group_check=True)
            AselS = ep.tile([128, 128], F32, name="AselS", tag="AselS")
            nc.gpsimd.memset(AselS[:, :], 0.0)
            nc.vector.tensor_copy(out=AselS[0:32, 0:100], in_=pS[0:32, 0:100])
            pAT = epps.tile([128, 512], F32, name="e2", tag="e2")
            nc.tensor.transpose(out=pAT[0:128, 0:128],
                                in_=AselS[0:128, 0:128], identity=idn[:, :])
            AT_sb = ep.tile([128, 32], F32, name="AT", tag="AT")
            nc.vector.tensor_copy(out=AT_sb[0:128, 0:32], in_=pAT[0:128, 0:32])

            # per-head tanh + w2 dot
            psS_tiles = [spps.tile([128, 512], F32, name=f"psS{q}", tag=f"psS{q}")
                         for q in range(4)]
            for q in range(4):
                nc.vector.memset(psS_tiles[q][:, :], 0.0)
            gsb_tiles = [ep.tile([128, 512], F32, name=f"gsb{q}", tag=f"gsb{q}")
                         for q in range(4)]
            # 2 heads per tanh op: DVE pre-adds the per-head bias so the
            # Act op count (each spaced at busy+drain) is halved
            for g in range(16):
                z = thp.tile([100, 512], F32, name=f"z{g % 3}",
                             tag=f"z{g % 3}")
                for j in range(2):
                    nc.vector.tensor_scalar(
                        out=z[0:100, 256 * j: 256 * j + 256],
                        in0=B_sb[0:100, 0:256],
                        scalar1=AT_sb[0:100, 2 * g + j: 2 * g + j + 1],
                        scalar2=None, op0=OP.add)
                th_t = thp.tile([100, 512], BF16, name=f"th{g % 3}",
                                tag=f"th{g % 3}")
                nc.scalar.activation(
                    th_t[0:100, 0:512], z[0:100, 0:512], AF.Tanh)
                for j in range(2):
                    r = 2 * g + j
                    q, half = divmod(r // 4, 2)
                    nc.tensor.matmul(
                        psS_tiles[q][32 * (r % 4): 32 * (r % 4) + 1,
                                     256 * half: 256 * half + 256],
                        lhsT=w2_sb[0:100, 0:1],
                        rhs=th_t[0:100, 256 * j: 256 * j + 256],
                        start=True, stop=True,
                        skip_group_check=True,
                        tile_position=(0, 32 * (r % 4)))
            for q in range(4):
                nc.vector.tensor_scalar(
                    out=gsb_tiles[q][0:128, 0:512],
                    in0=psS_tiles[q][0:128, 0:512],
                    scalar1=b2_sb[0:128, 0:1], scalar2=None, op0=OP.add)
                for half in range(2):
                    rb = 4 * (2 * q + half)
                    nc.sync.dma_start(
                        out=grid[rb:rb + 4, 0:256],
                        in_=gsb_tiles[q][0:128:32, 256 * half: 256 * half + 256])

    nc.compile()
    return nc


_NC_CACHE = None


def _get_nc():
    global _NC_CACHE
    if _NC_CACHE is None:
        _NC_CACHE = build_nc()
    return _NC_CACHE


def kernel(**inputs) -> np.ndarray:
    from concourse.bass_utils import run_bass_kernel_spmd

    arr = _prep_inputs(**inputs)
    nc = _get_nc()
    in_maps = []
    for k in range(NC):
        m = dict(arr)
        m["selT"] = _make_selT(k)
        in_maps.append(m)
    res = run_bass_kernel_spmd(nc, in_maps, core_ids=list(range(NC)))
    grid = np.concatenate([res.results[k]["grid"] for k in range(NC)], axis=0)
    mask = np.ones((N, N), dtype=bool)
    np.fill_diagonal(mask, False)
    mask[:, 0] = False
    return grid[mask].reshape(-1, 1).astype(np.float32)


# revision 47
# speedup vs baseline: 3.3066x; 1.0009x over previous
"""Trainium2 Bass kernel: BiLSTM dependency-parser edge scorer (v2).

Self-contained. Accepts FULL inputs (as produced by setup_inputs()), returns
the FULL [65280, 1] float32 score tensor.

Key idea vs v1: all recurrence matmuls are WEIGHTS-STATIONARY (weights in
lhsT, the tiny h vector streams as rhs), so each step's 64 gate matmuls have
output free-size 1 instead of streaming 6400 PSUM rows.

Layouts (per direction d, layer l):
  gates PSUM tile [100, 16]: partition p, col n = 4*j + g where the LSTM
    unit is u = 100*j + p (j in 0..4) and g in {0:i, 1:f, 2:g, 3:o}.
  h storage H[l][d] [100, 4*256] bf16: h_t for unit (j, p) at col 4*t + j.
    Column 4*t+j is directly the rhs [100, 1] for K-chunk j of the next
    step's matmul -- no transpose inside the loop.
  c state [100, 4] f32.
  xg_sb[d] [100, 16*256] bf16: precomputed input projections + bias,
    injected into the PSUM accumulation via an identity-weight matmul.
g-gate rows are pre-scaled by 2 on host: tanh(x) = 2*sigmoid(2x) - 1.
"""

import os
import sys

sys.path.insert(0, "/opt/trn_rl_repo")

import numpy as np

import concourse.bass as bass
import concourse.mybir as mybir
from concourse import bacc
from concourse.bass import IndirectOffsetOnAxis
from concourse.masks import make_identity
from concourse.tile import TileContext

N = 256          # sequence length
HID = 400        # hidden per direction
NC = 8           # cores
P = 100          # partitions used for unit math
NG = 16          # gate cols per step
F32 = mybir.dt.float32
BF16 = mybir.dt.float16
I32 = mybir.dt.int32
AF = mybir.ActivationFunctionType
OP = mybir.AluOpType

STEPS = int(os.environ.get("DP_STEPS", str(N)))


# ---------------------------------------------------------------------------
# host-side weight layout prep
# ---------------------------------------------------------------------------

def _bf(a):
    return np.ascontiguousarray(np.asarray(a).astype(np.float16))


# R[p, n] = original torch gate-row for (partition p, col n)
_PP, _NN = np.meshgrid(np.arange(P), np.arange(NG), indexing="ij")
_R = 400 * (_NN // 4) + 100 * (_NN % 4) + _PP      # [100, 16]


def _scale_g(W):
    """Scale g-gate rows (orig rows 800:1200) by 2."""
    Ws = np.array(W, dtype=np.float64)
    Ws[800:1200] *= 2.0
    return Ws


def _wblocks(W, nuc):
    """W: [1600, U] scaled gate-major weights, U = 100*nuc.
    Returns [100, 16*nuc*100]: block (n, uc) at cols (n*nuc+uc)*100 holds
    lhsT[k, m] = W[R[m, n], 100*uc + k]."""
    arr = W[_R]                                    # [100p, 16n, U]
    A4 = arr.reshape(P, NG, nuc, 100)              # [p, n, uc, k]
    return A4.transpose(3, 1, 2, 0).reshape(100, NG * nuc * 100)


def _prep_inputs(word_idx, pos_idx, word_emb, pos_emb,
                 Wih0, Whh0, bih0, bhh0, Wih1, Whh1, bih1, bhh1,
                 fc1_W, fc1_b, fc2_W, fc2_b):
    arr = {}
    arr["widx"] = np.ascontiguousarray(
        np.asarray(word_idx).reshape(N, 1).astype(np.int32))
    arr["pidx"] = np.ascontiguousarray(
        np.asarray(pos_idx).reshape(N, 1).astype(np.int32))
    arr["wemb"] = np.ascontiguousarray(np.asarray(word_emb, dtype=np.float32))
    arr["pemb"] = np.ascontiguousarray(np.asarray(pos_emb, dtype=np.float32))

    Wih = [np.asarray(Wih0, np.float64), np.asarray(Wih1, np.float64)]
    Whh = [np.asarray(Whh0, np.float64), np.asarray(Whh1, np.float64)]
    bih = [np.asarray(bih0, np.float64), np.asarray(bih1, np.float64)]
    bhh = [np.asarray(bhh0, np.float64), np.asarray(bhh1, np.float64)]

    whhT = np.zeros((4, 100, NG * 4 * 100), np.float32)
    biasT = np.zeros((4, 100, 512), np.float32)
    wih0T = np.zeros((2, 100, NG * 4 * 100), np.float32)
    wih1T = np.zeros((2, 100, NG * 8 * 100), np.float32)
    for l in range(2):
        for d in range(2):
            dl = 2 * l + d
            whhT[dl] = _wblocks(_scale_g(Whh[l][d]), 4)
            b = _scale_g(bih[l][d] + bhh[l][d])[_R]          # [100, 16]
            biasT[dl] = np.tile(b, (1, 32)).astype(np.float32)
    for d in range(2):
        wih0T[d] = _wblocks(_scale_g(Wih[0][d]), 4)
        wih1T[d] = _wblocks(_scale_g(Wih[1][d]), 8)
    arr["whhT"] = _bf(whhT)
    arr["biasT"] = np.ascontiguousarray(biasT)
    arr["wih0T"] = _bf(wih0T)
    arr["wih1T"] = _bf(wih1T)

    # identity for the xg injection matmul
    arr["id100"] = _bf(np.eye(P, dtype=np.float32))

    # edge MLP: uhT/umT [100, 800]: block uc at cols 100*uc holds
    # lhsT[k, a] = fc1_W[a, 100*uc + k]
    f1 = np.asarray(fc1_W, np.float64)               # [100, 1600]
    arr["uhT"] = _bf(np.concatenate(
        [f1[:, 100 * u:100 * u + 100].T for u in range(8)], axis=1))
    arr["umT"] = _bf(np.concatenate(
        [f1[:, 800 + 100 * u:800 + 100 * u + 100].T for u in range(8)],
        axis=1))
    arr["b1row"] = np.ascontiguousarray(
        np.asarray(fc1_b, np.float32).reshape(1, 100))
    arr["w2"] = _bf(np.asarray(fc2_W, np.float32).reshape(100, 1))
    arr["b2"] = np.ascontiguousarray(
        np.full((128, 1), np.float32(np.asarray(fc2_b).reshape(())),
                dtype=np.float32))
    return arr


def _make_selT(core):
    s = np.zeros((2, 128, 32), np.float32)
    for r in range(32):
        t = 32 * core + r
        s[t // 128, t % 128, r] = 1.0
    return np.ascontiguousarray(s)


# ---------------------------------------------------------------------------
# device kernel build
# ---------------------------------------------------------------------------

def _emit_xg_group(nc, nuc, ucs, wih_sb_d, rhs_chunk, d, tc, dst_sb,
                   bias_sb, ps_pool, tag):
    """One t-chunk (32 tokens) of an input-projection GEMM: 16*len(ucs)
    weights-stationary matmuls accumulating into a PSUM bank, then one
    PSUM->SBUF copy (adding bias if given)."""
    ps = ps_pool.tile([128, 512], F32, name=tag, tag=tag)
    for n in range(NG):
        for i, uc in enumerate(ucs):
            nc.tensor.matmul(
                ps[0:P, n:512:16],
                lhsT=wih_sb_d[0:P, (n * nuc + uc) * 100:
                              (n * nuc + uc) * 100 + 100],
                rhs=rhs_chunk(d, uc, tc),
                start=(i == 0), stop=(i == len(ucs) - 1),
                skip_group_check=True)
    if bias_sb is not None:
        nc.vector.tensor_tensor(
            out=dst_sb[0:P, 512 * tc: 512 * tc + 512],
            in0=ps[0:P, 0:512], in1=bias_sb[0:P, 0:512], op=OP.add)
    else:
        nc.vector.tensor_copy(
            out=dst_sb[0:P, 512 * tc: 512 * tc + 512],
            in_=ps[0:P, 0:512])


def _emit_xg(nc, l, wih_sb, rhs_chunk, xg_sbs, bias_sbs, ps_pool):
    """Full xg for layer l: xg[d][p, 16*t + n] = sum_u W[r(p,n), u]*in[t,u]+b."""
    nuc = 4 if l == 0 else 8
    for d in range(2):
        for tc in range(8):
            _emit_xg_group(nc, nuc, list(range(nuc)), wih_sb[d], rhs_chunk,
                           d, tc, xg_sbs[d], bias_sbs[d], ps_pool, "xgps")


def _emit_rec(nc, l, whh_sb, xg_sbs, id_sb, H_out, pools, xg2_sbs=None,
              extra=None):
    """STEPS wall-steps, both directions interleaved. xg2_sbs: optional
    second injection source (bwd-half input projections for layer 1).
    extra(t): called after each wall-step to emit overlapped work."""
    state_pool, sg_pool, tmp_pool, ps_pool = pools
    cs = []
    for d in range(2):
        c = state_pool.tile([P, 4], F32, name=f"c{d}", tag=f"c{d}")
        nc.gpsimd.memset(c[:, :], 0.0)
        cs.append(c)

    for t in range(STEPS):
        ps_t, sg_t, th_t = {}, {}, {}
        dorder = (0, 1)
        # --- PE: injection + 64 weight matmuls per direction ---
        for d in dorder:
            tdx = t if d == 0 else (STEPS - 1 - t)
            ps = ps_pool.tile([128, 512], F32, name=f"ps{d}", tag=f"ps{d}")
            ps_t[d] = ps
            first = (t == 0)
            nc.tensor.matmul(
                ps[0:P, 0:NG],
                lhsT=id_sb[0:P, 0:P],
                rhs=xg_sbs[d][0:P, NG * tdx: NG * tdx + NG],
                start=True, stop=(first and xg2_sbs is None),
                skip_group_check=True)
            if xg2_sbs is not None:
                nc.tensor.matmul(
                    ps[0:P, 0:NG],
                    lhsT=id_sb[0:P, 0:P],
                    rhs=xg2_sbs[d][0:P, NG * tdx: NG * tdx + NG],
                    start=False, stop=first, skip_group_check=True)
            if not first:
                pdx = tdx - 1 if d == 0 else tdx + 1
                # uc-major: uc 0/1 depend only on the first h half-write, so
                # their issue overlaps the second half-write
                for uc in range(4):
                    for n in range(NG):
                        nc.tensor.matmul(
                            ps[0:P, n:n + 1],
                            lhsT=whh_sb[d][0:P, (4 * n + uc) * 100:
                                           (4 * n + uc) * 100 + 100],
                            rhs=H_out[d][0:P, 4 * pdx + uc: 4 * pdx + uc + 1],
                            start=False, stop=(uc == 3),
                            skip_group_check=True)
        # --- Act: sigmoid over all gates (g pre-scaled by 2) ---
        for d in dorder:
            sg = sg_pool.tile([P, NG], F32, name=f"sg{d}", tag=f"sg{d}")
            sg_t[d] = sg
            nc.scalar.activation(sg[0:P, 0:NG], ps_t[d][0:P, 0:NG], AF.Sigmoid)
        # --- DVE per direction: c = sig(f)*c + sig(i)*(2*sig(2g) - 1),
        #     then tanh(c) via Pade [3/2] (|c| < 0.5 here, err < 1e-6):
        #     tanh(c) ~= c*(15 + c^2) / (15 + 6*c^2);  h = sig(o)*tanh ---
        for d in dorder:
            tdx = t if d == 0 else (STEPS - 1 - t)
            sg, c = sg_t[d], cs[d]
            # c = sig(f)*c + sig(i)*(2*sig(2g)-1), fused as:
            #   q = (sig_g - 0.5)*sig_i;  c = 2*q + sig_f*c
            q = tmp_pool.tile([P, 4], F32, name=f"q{d}", tag=f"q{d}")
            cf = tmp_pool.tile([P, 4], F32, name=f"cf{d}", tag=f"cf{d}")
            nc.vector.scalar_tensor_tensor(
                out=q[0:P, 0:4], in0=sg[0:P, 8:12], scalar=0.5,
                in1=sg[0:P, 0:4], op0=OP.subtract, op1=OP.mult)
            nc.vector.tensor_tensor(
                out=cf[0:P, 0:4], in0=sg[0:P, 4:8],
                in1=c[0:P, 0:4], op=OP.mult)
            nc.vector.scalar_tensor_tensor(
                out=c[0:P, 0:4], in0=q[0:P, 0:4], scalar=2.0,
                in1=cf[0:P, 0:4], op0=OP.mult, op1=OP.add)
            th = tmp_pool.tile([P, 4], F32, name=f"th{d}", tag=f"th{d}")
            th_t[d] = th
            nc.scalar.activation(th[0:P, 0:4], c[0:P, 0:4], AF.Tanh)
        for d in dorder:
            tdx = t if d == 0 else (STEPS - 1 - t)
            nc.vector.tensor_tensor(
                out=H_out[d][0:P, 4 * tdx: 4 * tdx + 2],
                in0=sg_t[d][0:P, 12:14], in1=th_t[d][0:P, 0:2], op=OP.mult)
            nc.vector.tensor_tensor(
                out=H_out[d][0:P, 4 * tdx + 2: 4 * tdx + 4],
                in0=sg_t[d][0:P, 14:NG], in1=th_t[d][0:P, 2:4], op=OP.mult)
        if extra is not None:
            extra(t)


def build_nc():
    nc = bacc.Bacc("TRN2", target_bir_lowering=False, debug=False,
                   num_devices=NC)
    wemb = nc.dram_tensor("wemb", [50000, 300], F32, kind="ExternalInput").ap()
    pemb = nc.dram_tensor("pemb", [50, 100], F32, kind="ExternalInput").ap()
    widx = nc.dram_tensor("widx", [N, 1], I32, kind="ExternalInput").ap()
    pidx = nc.dram_tensor("pidx", [N, 1], I32, kind="ExternalInput").ap()
    whhTd = nc.dram_tensor("whhT", [4, 100, 6400], BF16, kind="ExternalInput").ap()
    wih0Td = nc.dram_tensor("wih0T", [2, 100, 6400], BF16, kind="ExternalInput").ap()
    wih1Td = nc.dram_tensor("wih1T", [2, 100, 12800], BF16, kind="ExternalInput").ap()
    biasTd = nc.dram_tensor("biasT", [4, 100, 512], F32, kind="ExternalInput").ap()
    id100d = nc.dram_tensor("id100", [100, 100], BF16, kind="ExternalInput").ap()
    uhTd = nc.dram_tensor("uhT", [100, 800], BF16, kind="ExternalInput").ap()
    umTd = nc.dram_tensor("umT", [100, 800], BF16, kind="ExternalInput").ap()
    b1rowd = nc.dram_tensor("b1row", [1, 100], F32, kind="ExternalInput").ap()
    w2d = nc.dram_tensor("w2", [100, 1], BF16, kind="ExternalInput").ap()
    b2d = nc.dram_tensor("b2", [128, 1], F32, kind="ExternalInput").ap()
    selTd = nc.dram_tensor("selT", [2, 128, 32], F32, kind="ExternalInput").ap()
    grid = nc.dram_tensor("grid", [32, N], F32, kind="ExternalOutput").ap()

    from contextlib import ExitStack
    with TileContext(nc) as tc, ExitStack() as ctx:
        top = ctx.enter_context(tc.tile_pool(name="top", bufs=1))
        # ---- persistent SBUF tiles (DMAs emitted in priority order) ----
        idn = top.tile([128, 128], F32, name="idn", tag="idn")
        make_identity(nc, idn[:, :])
        warm = top.tile([1, 2], F32, name="warm", tag="warm")
        nc.gpsimd.memset(warm[:, :], 0.0)
        nc.scalar.activation(warm[0:1, 0:1], warm[0:1, 1:2], AF.Sigmoid)
        ones_sb = top.tile([1, 256], F32, name="ones", tag="ones")
        nc.gpsimd.memset(ones_sb[:, :], 1.0)
        whh_sb = [top.tile([100, 6400], BF16, name=f"whh{dl}", tag=f"whh{dl}")
                  for dl in range(4)]
        bias_sb = [top.tile([100, 512], F32, name=f"bias{dl}", tag=f"bias{dl}")
                   for dl in range(4)]
        id_sb = top.tile([100, 100], BF16, name="id100", tag="id100")
        wih1_sb = [top.tile([100, 12800], BF16, name=f"wih1{d}", tag=f"wih1{d}")
                   for d in range(2)]
        xg_sbs = [top.tile([100, 4096], BF16, name=f"xg{d}", tag=f"xg{d}")
                  for d in range(2)]
        H = [[top.tile([100, 4 * N], BF16, name=f"H{l}{d}", tag=f"H{l}{d}")
              for d in range(2)] for l in range(2)]
        xT = top.tile([100, 4 * N], BF16, name="xT", tag="xT")
        wih0_sb = [top.tile([100, 6400], BF16, name=f"wih0{d}",
                            tag=f"wih0{d}") for d in range(2)]
        if STEPS < N:
            for l in range(2):
                for d in range(2):
                    nc.gpsimd.memset(H[l][d][:, :], 0.0)

        # ========= embedding gather (first DMAs in the queue) =========
        with tc.tile_pool(name="wih0p", bufs=1) as w0p, \
             tc.tile_pool(name="embps", bufs=2, space="PSUM") as eps:
            idx_sb = w0p.tile([128, 4], I32, name="idx", tag="idx")
            nc.sync.dma_start(out=idx_sb[0:128, 0:1], in_=widx[0:128, 0:1])
            nc.sync.dma_start(out=idx_sb[0:128, 1:2], in_=widx[128:256, 0:1])
            nc.sync.dma_start(out=idx_sb[0:128, 2:3], in_=pidx[0:128, 0:1])
            nc.sync.dma_start(out=idx_sb[0:128, 3:4], in_=pidx[128:256, 0:1])
            x_sb = w0p.tile([128, 800], F32, name="xsb", tag="xsb")
            for cch in range(2):
                nc.gpsimd.indirect_dma_start(
                    out=x_sb[0:128, 400 * cch: 400 * cch + 300],
                    out_offset=None,
                    in_=wemb[:, :],
                    in_offset=IndirectOffsetOnAxis(
                        ap=idx_sb[0:128, cch:cch + 1], axis=0))
                nc.gpsimd.indirect_dma_start(
                    out=x_sb[0:128, 400 * cch + 300: 400 * cch + 400],
                    out_offset=None,
                    in_=pemb[:, :],
                    in_offset=IndirectOffsetOnAxis(
                        ap=idx_sb[0:128, 2 + cch:3 + cch], axis=0))
            # layer-0 weights + rec0 needs, in DMA-queue priority order
            nc.sync.dma_start(out=bias_sb[0][:, :], in_=biasTd[0])
            nc.sync.dma_start(out=bias_sb[1][:, :], in_=biasTd[1])
            nc.sync.dma_start(out=id_sb[:, :], in_=id100d[:, :])
            nc.sync.dma_start(out=wih0_sb[0][:, :], in_=wih0Td[0])
            nc.sync.dma_start(out=wih0_sb[1][:, :], in_=wih0Td[1])
            nc.sync.dma_start(out=whh_sb[0][:, :], in_=whhTd[0])
            nc.sync.dma_start(out=whh_sb[1][:, :], in_=whhTd[1])

            # x -> xT transpose
            for cch in range(2):
                for uc in range(4):
                    ptr = eps.tile([128, 128], F32, name="ptr", tag="ptr")
                    nc.tensor.transpose(
                        out=ptr[0:100, 0:128],
                        in_=x_sb[0:128, 400 * cch + 100 * uc:
                                 400 * cch + 100 * uc + 100],
                        identity=idn[:, :])
                    nc.vector.tensor_copy(
                        out=xT[0:100, 256 * uc + 128 * cch:
                               256 * uc + 128 * cch + 128],
                        in_=ptr[0:100, 0:128])

            # ========= layer 0 xg: only the chunks needed at rec0 start
            # (rest are interleaved into rec0's idle PE time) =========
            def rhs_l0(d, uc, tc):
                return xT[0:P, 256 * uc + 32 * tc: 256 * uc + 32 * tc + 32]

            with tc.tile_pool(name="xg0ps", bufs=2, space="PSUM") as xg_ps:
                _emit_xg_group(nc, 4, [0, 1, 2, 3], wih0_sb[0], rhs_l0,
                               0, 0, xg_sbs[0], bias_sb[0], xg_ps, "xgps")
                _emit_xg_group(nc, 4, [0, 1, 2, 3], wih0_sb[1], rhs_l0,
                               1, 7, xg_sbs[1], bias_sb[1], xg_ps, "xgps")

        # remaining big DMAs: execute during rec0
        for d in range(2):
            nc.sync.dma_start(out=wih1_sb[d][:, :], in_=wih1Td[d])
        for dl in range(2, 4):
            nc.sync.dma_start(out=whh_sb[dl][:, :], in_=whhTd[dl])
            nc.sync.dma_start(out=bias_sb[dl][:, :], in_=biasTd[dl])

        # ========= rec0 with layer-1 xg interleaved =========
        def rhs_l1(d, uc, tc):
            src = H[0][uc // 4]
            j = uc % 4
            return src[0:P, 128 * tc + j: 128 * tc + 128: 4]

        with tc.tile_pool(name="xg1buf", bufs=1) as xgbuf, \
             tc.tile_pool(name="xg1ps", bufs=2, space="PSUM") as xg1_ps:
            xgf_sbs = [xgbuf.tile([100, 4096], BF16, name=f"xgf{d}",
                                  tag=f"xgf{d}") for d in range(2)]
            xgb_sbs = [xgbuf.tile([100, 4096], BF16, name=f"xgb{d}",
                                  tag=f"xgb{d}") for d in range(2)]

            def mk_group(d, tc_, half):
                def emit():
                    _emit_xg_group(
                        nc, 8, list(range(4 * half, 4 * half + 4)),
                        wih1_sb[d], rhs_l1, d, tc_,
                        xgf_sbs[d] if half == 0 else xgb_sbs[d],
                        bias_sb[2 + d] if half == 0 else None,
                        xg1_ps, "xg1ps")
                return emit

            def mk_group0(d, tc_):
                def emit():
                    _emit_xg_group(nc, 4, [0, 1, 2, 3], wih0_sb[d], rhs_l0,
                                   d, tc_, xg_sbs[d], bias_sb[d], xg1_ps,
                                   "xg1ps")
                return emit

            # xg0 leftovers first (avail immediately), ordered by deadline:
            # fwd chunk tc needed by wall step 32*tc, bwd chunk by 224-32*tc
            pend = []
            for tc_ in range(1, 8):
                pend.append((0, mk_group0(0, tc_)))        # deadline 32*tc_
                pend.append((0, mk_group0(1, 7 - tc_)))    # same deadline
            for d in range(2):
                for tc_ in range(8):
                    pend.append((32 * tc_ + 32, mk_group(d, tc_, 0)))
                    pend.append((N - 32 * tc_, mk_group(d, tc_, 1)))
            pend.sort(key=lambda x: x[0])
            # end-gated groups: rec1 needs xgb[0]c0 / xgf[1]c7 at its step 0,
            # but xgf[0]c7 / xgb[1]c0 only by step ~224 -- emit those two
            # inside rec1's idle time instead
            pend = [e for e in pend if e[0] < N]
            urgent = [mk_group(1, 7, 0), mk_group(0, 0, 1)]
            late = [mk_group(0, 7, 0), mk_group(1, 0, 1)]
            st = {"i": 0, "last": -10}

            def extra(t):
                n_emit = 0
                while (st["i"] < len(pend) and pend[st["i"]][0] <= t
                       and t > st["last"] and n_emit < 2):
                    pend[st["i"]][1]()
                    st["i"] += 1
                    n_emit += 1
                if n_emit:
                    st["last"] = t

            with tc.tile_pool(name="rst0", bufs=1) as state_pool, \
                 tc.tile_pool(name="sg0", bufs=4) as sg_pool, \
                 tc.tile_pool(name="tmp0", bufs=4) as tmp_pool, \
                 tc.tile_pool(name="rec0ps", bufs=3, space="PSUM") as rec_ps:
                _emit_rec(nc, 0, whh_sb[0:2], xg_sbs, id_sb, H[0],
                          (state_pool, sg_pool, tmp_pool, rec_ps),
                          extra=extra)
            # leftover in-rec0 groups, then the two urgently needed ones
            while st["i"] < len(pend):
                pend[st["i"]][1]()
                st["i"] += 1
            for fn in urgent:
                fn()

            st1 = {"i": 0, "last": -10}

            def extra1(t):
                if st1["i"] < len(late) and t - st1["last"] >= 2:
                    late[st1["i"]]()
                    st1["i"] += 1
                    st1["last"] = t

            # ========= rec1 (dual injection: fwd + bwd halves) =========
            with tc.tile_pool(name="rst1", bufs=1) as state_pool, \
                 tc.tile_pool(name="sg1", bufs=4) as sg_pool, \
                 tc.tile_pool(name="tmp1", bufs=4) as tmp_pool, \
                 tc.tile_pool(name="rec1ps", bufs=3, space="PSUM") as rec_ps:
                _emit_rec(nc, 1, whh_sb[2:4], xgf_sbs, id_sb, H[1],
                          (state_pool, sg_pool, tmp_pool, rec_ps),
                          xg2_sbs=xgb_sbs, extra=extra1)
            while st1["i"] < len(late):
                late[st1["i"]]()
                st1["i"] += 1

        # ========= edge scorer =========
        with tc.tile_pool(name="edge", bufs=1) as ep, \
             tc.tile_pool(name="edgeth", bufs=2) as thp, \
             tc.tile_pool(name="edgeps", bufs=1, space="PSUM") as epps, \
             tc.tile_pool(name="edgepsS", bufs=1, space="PSUM") as spps:
            uh_sb = ep.tile([100, 800], BF16, name="uhT", tag="uhT")
            nc.sync.dma_start(out=uh_sb[:, :], in_=uhTd[:, :])
            um_sb = ep.tile([100, 800], BF16, name="umT", tag="umT")
            nc.sync.dma_start(out=um_sb[:, :], in_=umTd[:, :])
            b1_sb = ep.tile([1, 100], F32, name="b1row", tag="b1row")
            nc.sync.dma_start(out=b1_sb[:, :], in_=b1rowd[:, :])
            w2_sb = ep.tile([100, 1], BF16, name="w2", tag="w2")
            nc.sync.dma_start(out=w2_sb[:, :], in_=w2d[:, :])
            b2_sb = ep.tile([128, 1], F32, name="b2", tag="b2")
            nc.sync.dma_start(out=b2_sb[:, :], in_=b2d[:, :])
            selT_sb = ep.tile([128, 64], F32, name="selT", tag="selT")
            nc.sync.dma_start(out=selT_sb[0:128, 0:32], in_=selTd[0])
            nc.sync.dma_start(out=selT_sb[0:128, 32:64], in_=selTd[1])

            def h1_rhs(uc):
                return H[1][uc // 4][0:P, uc % 4: 4 * N: 4]

            # A^T [100, 256] (head half of fc1)
            pA = epps.tile([128, 512], F32, name="e1", tag="e1")
            for uc in range(8):
                nc.tensor.matmul(
                    pA[0:P, 0:256],
                    lhsT=uh_sb[0:P, 100 * uc: 100 * uc + 100],
                    rhs=h1_rhs(uc),
                    start=(uc == 0), stop=(uc == 7), skip_group_check=True)
            A_sb = ep.tile([100, 256], F32, name="A", tag="A")
            nc.vector.tensor_copy(out=A_sb[0:P, 0:256], in_=pA[0:P, 0:256])
            # B^T [100, 256] + b1 (modifier half)
            pB = epps.tile([128, 512], F32, name="e3", tag="e3")
            for uc in range(8):
                nc.tensor.matmul(
                    pB[0:P, 0:256],
                    lhsT=um_sb[0:P, 100 * uc: 100 * uc + 100],
                    rhs=h1_rhs(uc),
                    start=(uc == 0), stop=False, skip_group_check=True)
            nc.tensor.matmul(
                pB[0:P, 0:256],
                lhsT=b1_sb[0:1, 0:100],
                rhs=ones_sb[0:1, 0:256],
                start=False, stop=True, skip_group_check=True)
            B_sb = ep.tile([100, 256], BF16, name="Bsb", tag="Bsb")
            nc.vector.tensor_copy(out=B_sb[0:P, 0:256], in_=pB[0:P, 0:256])
            # A -> token-major via transpose, then per-core 32-head select
            A_tok = ep.tile([128, 256], F32, name="Atok", tag="Atok")
            for m in range(2):
                pT = epps.tile([128, 512], F32, name="e2", tag="e2")
                nc.tensor.transpose(
                    out=pT[0:128, 0:100],
                    in_=A_sb[0:100, 128 * m: 128 * m + 128],
                    identity=idn[0:100, 0:100])
                nc.vector.tensor_copy(
                    out=A_tok[0:128, 128 * m: 128 * m + 100],
                    in_=pT[0:128, 0:100])
            pS = epps.tile([128, 512], F32, name="e1", tag="e1")
            for m in range(2):
                nc.tensor.matmul(
                    pS[0:32, 0:100],
                    lhsT=selT_sb[0:128, 32 * m: 32 * m + 32],
                    rhs=A_tok[0:128, 128 * m: 128 * m + 100],
                    start=(m == 0), stop=(m == 1), skip_group_check=True)
            AselS = ep.tile([128, 128], F32, name="AselS", tag="AselS")
            nc.gpsimd.memset(AselS[:, :], 0.0)
            nc.vector.tensor_copy(out=AselS[0:32, 0:100], in_=pS[0:32, 0:100])
            pAT = epps.tile([128, 512], F32, name="e2", tag="e2")
            nc.tensor.transpose(out=pAT[0:128, 0:128],
                                in_=AselS[0:128, 0:128], identity=idn[:, :])
            AT_sb = ep.tile([128, 32], F32, name="AT", tag="AT")
            nc.vector.tensor_copy(out=AT_sb[0:128, 0:32], in_=pAT[0:128, 0:32])

            # per-head tanh + w2 dot
            psS_tiles = [spps.tile([128, 512], F32, name=f"psS{q}", tag=f"psS{q}")
                         for q in range(4)]
            for q in range(4):
                nc.vector.memset(psS_tiles[q][:, :], 0.0)
            gsb_tiles = [ep.tile([128, 512], F32, name=f"gsb{q}", tag=f"gsb{q}")
                         for q in range(4)]
            # 2 heads per tanh op: DVE pre-adds the per-head bias so the
            # Act op count (each spaced at busy+drain) is halved
            for g in range(16):
                z = thp.tile([100, 512], F32, name=f"z{g % 3}",
                             tag=f"z{g % 3}")
                for j in range(2):
                    nc.vector.tensor_scalar(
                        out=z[0:100, 256 * j: 256 * j + 256],
                        in0=B_sb[0:100, 0:256],
                        scalar1=AT_sb[0:100, 2 * g + j: 2 * g + j + 1],
                        scalar2=None, op0=OP.add)
                th_t = thp.tile([100, 512], BF16, name=f"th{g % 3}",
                                tag=f"th{g % 3}")
                nc.scalar.activation(
                    th_t[0:100, 0:512], z[0:100, 0:512], AF.Tanh)
                for j in range(2):
                    r = 2 * g + j
                    q, half = divmod(r // 4, 2)
                    nc.tensor.matmul(
                        psS_tiles[q][32 * (r % 4): 32 * (r % 4) + 1,
                                     256 * half: 256 * half + 256],
                        lhsT=w2_sb[0:100, 0:1],
                        rhs=th_t[0:100, 256 * j: 256 * j + 256],
                        start=True, stop=True,
                        skip_group_check=True,
                        tile_position=(0, 32 * (r % 4)))
            for q in range(4):
                nc.vector.tensor_scalar(
                    out=gsb_tiles[q][0:128, 0:512],
                    in0=psS_tiles[q][0:128, 0:512],
                    scalar1=b2_sb[0:128, 0:1], scalar2=None, op0=OP.add)
                for half in range(2):
                    rb = 4 * (2 * q + half)
                    nc.sync.dma_start(
                        out=grid[rb:rb + 4, 0:256],
                        in_=gsb_tiles[q][0:128:32, 256 * half: 256 * half + 256])

    nc.compile()
    return nc


_NC_CACHE = None


def _get_nc():
    global _NC_CACHE
    if _NC_CACHE is None:
        _NC_CACHE = build_nc()
    return _NC_CACHE


def kernel(**inputs) -> np.ndarray:
    from concourse.bass_utils import run_bass_kernel_spmd

    arr = _prep_inputs(**inputs)
    nc = _get_nc()
    in_maps = []
    for k in range(NC):
        m = dict(arr)
        m["selT"] = _make_selT(k)
        in_maps.append(m)
    res = run_bass_kernel_spmd(nc, in_maps, core_ids=list(range(NC)))
    grid = np.concatenate([res.results[k]["grid"] for k in range(NC)], axis=0)
    mask = np.ones((N, N), dtype=bool)
    np.fill_diagonal(mask, False)
    mask[:, 0] = False
    return grid[mask].reshape(-1, 1).astype(np.float32)


# revision 48
# speedup vs baseline: 3.3089x; 1.0007x over previous
"""Trainium2 Bass kernel: BiLSTM dependency-parser edge scorer (v2).

Self-contained. Accepts FULL inputs (as produced by setup_inputs()), returns
the FULL [65280, 1] float32 score tensor.

Key idea vs v1: all recurrence matmuls are WEIGHTS-STATIONARY (weights in
lhsT, the tiny h vector streams as rhs), so each step's 64 gate matmuls have
output free-size 1 instead of streaming 6400 PSUM rows.

Layouts (per direction d, layer l):
  gates PSUM tile [100, 16]: partition p, col n = 4*j + g where the LSTM
    unit is u = 100*j + p (j in 0..4) and g in {0:i, 1:f, 2:g, 3:o}.
  h storage H[l][d] [100, 4*256] bf16: h_t for unit (j, p) at col 4*t + j.
    Column 4*t+j is directly the rhs [100, 1] for K-chunk j of the next
    step's matmul -- no transpose inside the loop.
  c state [100, 4] f32.
  xg_sb[d] [100, 16*256] bf16: precomputed input projections + bias,
    injected into the PSUM accumulation via an identity-weight matmul.
g-gate rows are pre-scaled by 2 on host: tanh(x) = 2*sigmoid(2x) - 1.
"""

import os
import sys

sys.path.insert(0, "/opt/trn_rl_repo")

import numpy as np

import concourse.bass as bass
import concourse.mybir as mybir
from concourse import bacc
from concourse.bass import IndirectOffsetOnAxis
from concourse.masks import make_identity
from concourse.tile import TileContext

N = 256          # sequence length
HID = 400        # hidden per direction
NC = 8           # cores
P = 100          # partitions used for unit math
NG = 16          # gate cols per step
F32 = mybir.dt.float32
BF16 = mybir.dt.float16
I32 = mybir.dt.int32
AF = mybir.ActivationFunctionType
OP = mybir.AluOpType

STEPS = int(os.environ.get("DP_STEPS", str(N)))


# ---------------------------------------------------------------------------
# host-side weight layout prep
# ---------------------------------------------------------------------------

def _bf(a):
    return np.ascontiguousarray(np.asarray(a).astype(np.float16))


# R[p, n] = original torch gate-row for (partition p, col n)
_PP, _NN = np.meshgrid(np.arange(P), np.arange(NG), indexing="ij")
_R = 400 * (_NN // 4) + 100 * (_NN % 4) + _PP      # [100, 16]


def _scale_g(W):
    """Scale g-gate rows (orig rows 800:1200) by 2."""
    Ws = np.array(W, dtype=np.float64)
    Ws[800:1200] *= 2.0
    return Ws


def _wblocks(W, nuc):
    """W: [1600, U] scaled gate-major weights, U = 100*nuc.
    Returns [100, 16*nuc*100]: block (n, uc) at cols (n*nuc+uc)*100 holds
    lhsT[k, m] = W[R[m, n], 100*uc + k]."""
    arr = W[_R]                                    # [100p, 16n, U]
    A4 = arr.reshape(P, NG, nuc, 100)              # [p, n, uc, k]
    return A4.transpose(3, 1, 2, 0).reshape(100, NG * nuc * 100)


def _prep_inputs(word_idx, pos_idx, word_emb, pos_emb,
                 Wih0, Whh0, bih0, bhh0, Wih1, Whh1, bih1, bhh1,
                 fc1_W, fc1_b, fc2_W, fc2_b):
    arr = {}
    arr["widx"] = np.ascontiguousarray(
        np.asarray(word_idx).reshape(N, 1).astype(np.int32))
    arr["pidx"] = np.ascontiguousarray(
        np.asarray(pos_idx).reshape(N, 1).astype(np.int32))
    arr["wemb"] = np.ascontiguousarray(np.asarray(word_emb, dtype=np.float32))
    arr["pemb"] = np.ascontiguousarray(np.asarray(pos_emb, dtype=np.float32))

    Wih = [np.asarray(Wih0, np.float64), np.asarray(Wih1, np.float64)]
    Whh = [np.asarray(Whh0, np.float64), np.asarray(Whh1, np.float64)]
    bih = [np.asarray(bih0, np.float64), np.asarray(bih1, np.float64)]
    bhh = [np.asarray(bhh0, np.float64), np.asarray(bhh1, np.float64)]

    whhT = np.zeros((4, 100, NG * 4 * 100), np.float32)
    biasT = np.zeros((4, 100, 512), np.float32)
    wih0T = np.zeros((2, 100, NG * 4 * 100), np.float32)
    wih1T = np.zeros((2, 100, NG * 8 * 100), np.float32)
    for l in range(2):
        for d in range(2):
            dl = 2 * l + d
            whhT[dl] = _wblocks(_scale_g(Whh[l][d]), 4)
            b = _scale_g(bih[l][d] + bhh[l][d])[_R]          # [100, 16]
            biasT[dl] = np.tile(b, (1, 32)).astype(np.float32)
    for d in range(2):
        wih0T[d] = _wblocks(_scale_g(Wih[0][d]), 4)
        wih1T[d] = _wblocks(_scale_g(Wih[1][d]), 8)
    arr["whhT"] = _bf(whhT)
    arr["biasT"] = np.ascontiguousarray(biasT)
    arr["wih0T"] = _bf(wih0T)
    arr["wih1T"] = _bf(wih1T)

    # identity for the xg injection matmul
    arr["id100"] = _bf(np.eye(P, dtype=np.float32))

    # edge MLP: uhT/umT [100, 800]: block uc at cols 100*uc holds
    # lhsT[k, a] = fc1_W[a, 100*uc + k]
    f1 = np.asarray(fc1_W, np.float64)               # [100, 1600]
    arr["uhT"] = _bf(np.concatenate(
        [f1[:, 100 * u:100 * u + 100].T for u in range(8)], axis=1))
    arr["umT"] = _bf(np.concatenate(
        [f1[:, 800 + 100 * u:800 + 100 * u + 100].T for u in range(8)],
        axis=1))
    arr["b1row"] = np.ascontiguousarray(
        np.asarray(fc1_b, np.float32).reshape(1, 100))
    arr["w2"] = _bf(np.asarray(fc2_W, np.float32).reshape(100, 1))
    arr["b2"] = np.ascontiguousarray(
        np.full((128, 1), np.float32(np.asarray(fc2_b).reshape(())),
                dtype=np.float32))
    return arr


def _make_selT(core):
    s = np.zeros((2, 128, 32), np.float32)
    for r in range(32):
        t = 32 * core + r
        s[t // 128, t % 128, r] = 1.0
    return np.ascontiguousarray(s)


# ---------------------------------------------------------------------------
# device kernel build
# ---------------------------------------------------------------------------

def _emit_xg_group(nc, nuc, ucs, wih_sb_d, rhs_chunk, d, tc, dst_sb,
                   bias_sb, ps_pool, tag):
    """One t-chunk (32 tokens) of an input-projection GEMM: 16*len(ucs)
    weights-stationary matmuls accumulating into a PSUM bank, then one
    PSUM->SBUF copy (adding bias if given)."""
    ps = ps_pool.tile([128, 512], F32, name=tag, tag=tag)
    for n in range(NG):
        for i, uc in enumerate(ucs):
            nc.tensor.matmul(
                ps[0:P, n:512:16],
                lhsT=wih_sb_d[0:P, (n * nuc + uc) * 100:
                              (n * nuc + uc) * 100 + 100],
                rhs=rhs_chunk(d, uc, tc),
                start=(i == 0), stop=(i == len(ucs) - 1),
                skip_group_check=True)
    if bias_sb is not None:
        nc.vector.tensor_tensor(
            out=dst_sb[0:P, 512 * tc: 512 * tc + 512],
            in0=ps[0:P, 0:512], in1=bias_sb[0:P, 0:512], op=OP.add)
    else:
        nc.vector.tensor_copy(
            out=dst_sb[0:P, 512 * tc: 512 * tc + 512],
            in_=ps[0:P, 0:512])


def _emit_xg_part(nc, wih_sb_d, H0, d, tc, lo, hi, half, dst_sb,
                  bias_sb, ps_pool):
    """Tokens [32*tc+lo, 32*tc+hi) of a layer-1 xg half-group."""
    ps = ps_pool.tile([128, 512], F32, name="xg1ps", tag="xg1ps")
    for n in range(NG):
        for i, uc in enumerate(range(4 * half, 4 * half + 4)):
            src = H0[uc // 4]
            j = uc % 4
            nc.tensor.matmul(
                ps[0:P, 16 * lo + n: 16 * hi: 16],
                lhsT=wih_sb_d[0:P, (n * 8 + uc) * 100: (n * 8 + uc) * 100 + 100],
                rhs=src[0:P, 128 * tc + 4 * lo + j: 128 * tc + 4 * hi: 4],
                start=(i == 0), stop=(i == 3), skip_group_check=True)
    if bias_sb is not None:
        nc.vector.tensor_tensor(
            out=dst_sb[0:P, 512 * tc + 16 * lo: 512 * tc + 16 * hi],
            in0=ps[0:P, 16 * lo: 16 * hi],
            in1=bias_sb[0:P, 16 * lo: 16 * hi], op=OP.add)
    else:
        nc.vector.tensor_copy(
            out=dst_sb[0:P, 512 * tc + 16 * lo: 512 * tc + 16 * hi],
            in_=ps[0:P, 16 * lo: 16 * hi])


def _emit_xg(nc, l, wih_sb, rhs_chunk, xg_sbs, bias_sbs, ps_pool):
    """Full xg for layer l: xg[d][p, 16*t + n] = sum_u W[r(p,n), u]*in[t,u]+b."""
    nuc = 4 if l == 0 else 8
    for d in range(2):
        for tc in range(8):
            _emit_xg_group(nc, nuc, list(range(nuc)), wih_sb[d], rhs_chunk,
                           d, tc, xg_sbs[d], bias_sbs[d], ps_pool, "xgps")


def _emit_rec(nc, l, whh_sb, xg_sbs, id_sb, H_out, pools, xg2_sbs=None,
              extra=None):
    """STEPS wall-steps, both directions interleaved. xg2_sbs: optional
    second injection source (bwd-half input projections for layer 1).
    extra(t): called after each wall-step to emit overlapped work."""
    state_pool, sg_pool, tmp_pool, ps_pool = pools
    cs = []
    for d in range(2):
        c = state_pool.tile([P, 4], F32, name=f"c{d}", tag=f"c{d}")
        nc.gpsimd.memset(c[:, :], 0.0)
        cs.append(c)

    for t in range(STEPS):
        ps_t, sg_t, th_t = {}, {}, {}
        dorder = (0, 1)
        # --- PE: injection + 64 weight matmuls per direction ---
        for d in dorder:
            tdx = t if d == 0 else (STEPS - 1 - t)
            ps = ps_pool.tile([128, 512], F32, name=f"ps{d}", tag=f"ps{d}")
            ps_t[d] = ps
            first = (t == 0)
            nc.tensor.matmul(
                ps[0:P, 0:NG],
                lhsT=id_sb[0:P, 0:P],
                rhs=xg_sbs[d][0:P, NG * tdx: NG * tdx + NG],
                start=True, stop=(first and xg2_sbs is None),
                skip_group_check=True)
            if xg2_sbs is not None:
                nc.tensor.matmul(
                    ps[0:P, 0:NG],
                    lhsT=id_sb[0:P, 0:P],
                    rhs=xg2_sbs[d][0:P, NG * tdx: NG * tdx + NG],
                    start=False, stop=first, skip_group_check=True)
            if not first:
                pdx = tdx - 1 if d == 0 else tdx + 1
                # uc-major: uc 0/1 depend only on the first h half-write, so
                # their issue overlaps the second half-write
                for uc in range(4):
                    for n in range(NG):
                        nc.tensor.matmul(
                            ps[0:P, n:n + 1],
                            lhsT=whh_sb[d][0:P, (4 * n + uc) * 100:
                                           (4 * n + uc) * 100 + 100],
                            rhs=H_out[d][0:P, 4 * pdx + uc: 4 * pdx + uc + 1],
                            start=False, stop=(uc == 3),
                            skip_group_check=True)
        # --- Act: sigmoid over all gates (g pre-scaled by 2) ---
        for d in dorder:
            sg = sg_pool.tile([P, NG], F32, name=f"sg{d}", tag=f"sg{d}")
            sg_t[d] = sg
            nc.scalar.activation(sg[0:P, 0:NG], ps_t[d][0:P, 0:NG], AF.Sigmoid)
        # --- DVE per direction: c = sig(f)*c + sig(i)*(2*sig(2g) - 1),
        #     then tanh(c) via Pade [3/2] (|c| < 0.5 here, err < 1e-6):
        #     tanh(c) ~= c*(15 + c^2) / (15 + 6*c^2);  h = sig(o)*tanh ---
        for d in dorder:
            tdx = t if d == 0 else (STEPS - 1 - t)
            sg, c = sg_t[d], cs[d]
            # c = sig(f)*c + sig(i)*(2*sig(2g)-1), fused as:
            #   q = (sig_g - 0.5)*sig_i;  c = 2*q + sig_f*c
            q = tmp_pool.tile([P, 4], F32, name=f"q{d}", tag=f"q{d}")
            cf = tmp_pool.tile([P, 4], F32, name=f"cf{d}", tag=f"cf{d}")
            nc.vector.scalar_tensor_tensor(
                out=q[0:P, 0:4], in0=sg[0:P, 8:12], scalar=0.5,
                in1=sg[0:P, 0:4], op0=OP.subtract, op1=OP.mult)
            nc.vector.tensor_tensor(
                out=cf[0:P, 0:4], in0=sg[0:P, 4:8],
                in1=c[0:P, 0:4], op=OP.mult)
            nc.vector.scalar_tensor_tensor(
                out=c[0:P, 0:4], in0=q[0:P, 0:4], scalar=2.0,
                in1=cf[0:P, 0:4], op0=OP.mult, op1=OP.add)
            th = tmp_pool.tile([P, 4], F32, name=f"th{d}", tag=f"th{d}")
            th_t[d] = th
            nc.scalar.activation(th[0:P, 0:4], c[0:P, 0:4], AF.Tanh)
        for d in dorder:
            tdx = t if d == 0 else (STEPS - 1 - t)
            nc.vector.tensor_tensor(
                out=H_out[d][0:P, 4 * tdx: 4 * tdx + 2],
                in0=sg_t[d][0:P, 12:14], in1=th_t[d][0:P, 0:2], op=OP.mult)
            nc.vector.tensor_tensor(
                out=H_out[d][0:P, 4 * tdx + 2: 4 * tdx + 4],
                in0=sg_t[d][0:P, 14:NG], in1=th_t[d][0:P, 2:4], op=OP.mult)
        if extra is not None:
            extra(t)


def build_nc():
    nc = bacc.Bacc("TRN2", target_bir_lowering=False, debug=False,
                   num_devices=NC)
    wemb = nc.dram_tensor("wemb", [50000, 300], F32, kind="ExternalInput").ap()
    pemb = nc.dram_tensor("pemb", [50, 100], F32, kind="ExternalInput").ap()
    widx = nc.dram_tensor("widx", [N, 1], I32, kind="ExternalInput").ap()
    pidx = nc.dram_tensor("pidx", [N, 1], I32, kind="ExternalInput").ap()
    whhTd = nc.dram_tensor("whhT", [4, 100, 6400], BF16, kind="ExternalInput").ap()
    wih0Td = nc.dram_tensor("wih0T", [2, 100, 6400], BF16, kind="ExternalInput").ap()
    wih1Td = nc.dram_tensor("wih1T", [2, 100, 12800], BF16, kind="ExternalInput").ap()
    biasTd = nc.dram_tensor("biasT", [4, 100, 512], F32, kind="ExternalInput").ap()
    id100d = nc.dram_tensor("id100", [100, 100], BF16, kind="ExternalInput").ap()
    uhTd = nc.dram_tensor("uhT", [100, 800], BF16, kind="ExternalInput").ap()
    umTd = nc.dram_tensor("umT", [100, 800], BF16, kind="ExternalInput").ap()
    b1rowd = nc.dram_tensor("b1row", [1, 100], F32, kind="ExternalInput").ap()
    w2d = nc.dram_tensor("w2", [100, 1], BF16, kind="ExternalInput").ap()
    b2d = nc.dram_tensor("b2", [128, 1], F32, kind="ExternalInput").ap()
    selTd = nc.dram_tensor("selT", [2, 128, 32], F32, kind="ExternalInput").ap()
    grid = nc.dram_tensor("grid", [32, N], F32, kind="ExternalOutput").ap()

    from contextlib import ExitStack
    with TileContext(nc) as tc, ExitStack() as ctx:
        top = ctx.enter_context(tc.tile_pool(name="top", bufs=1))
        # ---- persistent SBUF tiles (DMAs emitted in priority order) ----
        idn = top.tile([128, 128], F32, name="idn", tag="idn")
        make_identity(nc, idn[:, :])
        warm = top.tile([1, 2], F32, name="warm", tag="warm")
        nc.gpsimd.memset(warm[:, :], 0.0)
        nc.scalar.activation(warm[0:1, 0:1], warm[0:1, 1:2], AF.Sigmoid)
        ones_sb = top.tile([1, 256], F32, name="ones", tag="ones")
        nc.gpsimd.memset(ones_sb[:, :], 1.0)
        whh_sb = [top.tile([100, 6400], BF16, name=f"whh{dl}", tag=f"whh{dl}")
                  for dl in range(4)]
        bias_sb = [top.tile([100, 512], F32, name=f"bias{dl}", tag=f"bias{dl}")
                   for dl in range(4)]
        id_sb = top.tile([100, 100], BF16, name="id100", tag="id100")
        wih1_sb = [top.tile([100, 12800], BF16, name=f"wih1{d}", tag=f"wih1{d}")
                   for d in range(2)]
        xg_sbs = [top.tile([100, 4096], BF16, name=f"xg{d}", tag=f"xg{d}")
                  for d in range(2)]
        H = [[top.tile([100, 4 * N], BF16, name=f"H{l}{d}", tag=f"H{l}{d}")
              for d in range(2)] for l in range(2)]
        xT = top.tile([100, 4 * N], BF16, name="xT", tag="xT")
        wih0_sb = [top.tile([100, 6400], BF16, name=f"wih0{d}",
                            tag=f"wih0{d}") for d in range(2)]
        if STEPS < N:
            for l in range(2):
                for d in range(2):
                    nc.gpsimd.memset(H[l][d][:, :], 0.0)

        # ========= embedding gather (first DMAs in the queue) =========
        with tc.tile_pool(name="wih0p", bufs=1) as w0p, \
             tc.tile_pool(name="embps", bufs=2, space="PSUM") as eps:
            idx_sb = w0p.tile([128, 4], I32, name="idx", tag="idx")
            nc.sync.dma_start(out=idx_sb[0:128, 0:1], in_=widx[0:128, 0:1])
            nc.sync.dma_start(out=idx_sb[0:128, 1:2], in_=widx[128:256, 0:1])
            nc.sync.dma_start(out=idx_sb[0:128, 2:3], in_=pidx[0:128, 0:1])
            nc.sync.dma_start(out=idx_sb[0:128, 3:4], in_=pidx[128:256, 0:1])
            x_sb = w0p.tile([128, 800], F32, name="xsb", tag="xsb")
            for cch in range(2):
                nc.gpsimd.indirect_dma_start(
                    out=x_sb[0:128, 400 * cch: 400 * cch + 300],
                    out_offset=None,
                    in_=wemb[:, :],
                    in_offset=IndirectOffsetOnAxis(
                        ap=idx_sb[0:128, cch:cch + 1], axis=0))
                nc.gpsimd.indirect_dma_start(
                    out=x_sb[0:128, 400 * cch + 300: 400 * cch + 400],
                    out_offset=None,
                    in_=pemb[:, :],
                    in_offset=IndirectOffsetOnAxis(
                        ap=idx_sb[0:128, 2 + cch:3 + cch], axis=0))
            # layer-0 weights + rec0 needs, in DMA-queue priority order
            nc.sync.dma_start(out=bias_sb[0][:, :], in_=biasTd[0])
            nc.sync.dma_start(out=bias_sb[1][:, :], in_=biasTd[1])
            nc.sync.dma_start(out=id_sb[:, :], in_=id100d[:, :])
            nc.sync.dma_start(out=wih0_sb[0][:, :], in_=wih0Td[0])
            nc.sync.dma_start(out=wih0_sb[1][:, :], in_=wih0Td[1])
            nc.sync.dma_start(out=whh_sb[0][:, :], in_=whhTd[0])
            nc.sync.dma_start(out=whh_sb[1][:, :], in_=whhTd[1])

            # x -> xT transpose
            for cch in range(2):
                for uc in range(4):
                    ptr = eps.tile([128, 128], F32, name="ptr", tag="ptr")
                    nc.tensor.transpose(
                        out=ptr[0:100, 0:128],
                        in_=x_sb[0:128, 400 * cch + 100 * uc:
                                 400 * cch + 100 * uc + 100],
                        identity=idn[:, :])
                    nc.vector.tensor_copy(
                        out=xT[0:100, 256 * uc + 128 * cch:
                               256 * uc + 128 * cch + 128],
                        in_=ptr[0:100, 0:128])

            # ========= layer 0 xg: only the chunks needed at rec0 start
            # (rest are interleaved into rec0's idle PE time) =========
            def rhs_l0(d, uc, tc):
                return xT[0:P, 256 * uc + 32 * tc: 256 * uc + 32 * tc + 32]

            with tc.tile_pool(name="xg0ps", bufs=2, space="PSUM") as xg_ps:
                _emit_xg_group(nc, 4, [0, 1, 2, 3], wih0_sb[0], rhs_l0,
                               0, 0, xg_sbs[0], bias_sb[0], xg_ps, "xgps")
                _emit_xg_group(nc, 4, [0, 1, 2, 3], wih0_sb[1], rhs_l0,
                               1, 7, xg_sbs[1], bias_sb[1], xg_ps, "xgps")

        # remaining big DMAs: execute during rec0
        for d in range(2):
            nc.sync.dma_start(out=wih1_sb[d][:, :], in_=wih1Td[d])
        for dl in range(2, 4):
            nc.sync.dma_start(out=whh_sb[dl][:, :], in_=whhTd[dl])
            nc.sync.dma_start(out=bias_sb[dl][:, :], in_=biasTd[dl])

        # ========= rec0 with layer-1 xg interleaved =========
        def rhs_l1(d, uc, tc):
            src = H[0][uc // 4]
            j = uc % 4
            return src[0:P, 128 * tc + j: 128 * tc + 128: 4]

        with tc.tile_pool(name="xg1buf", bufs=1) as xgbuf, \
             tc.tile_pool(name="xg1ps", bufs=2, space="PSUM") as xg1_ps:
            xgf_sbs = [xgbuf.tile([100, 4096], BF16, name=f"xgf{d}",
                                  tag=f"xgf{d}") for d in range(2)]
            xgb_sbs = [xgbuf.tile([100, 4096], BF16, name=f"xgb{d}",
                                  tag=f"xgb{d}") for d in range(2)]

            def mk_group(d, tc_, half):
                def emit():
                    _emit_xg_group(
                        nc, 8, list(range(4 * half, 4 * half + 4)),
                        wih1_sb[d], rhs_l1, d, tc_,
                        xgf_sbs[d] if half == 0 else xgb_sbs[d],
                        bias_sb[2 + d] if half == 0 else None,
                        xg1_ps, "xg1ps")
                return emit

            def mk_group0(d, tc_):
                def emit():
                    _emit_xg_group(nc, 4, [0, 1, 2, 3], wih0_sb[d], rhs_l0,
                                   d, tc_, xg_sbs[d], bias_sb[d], xg1_ps,
                                   "xg1ps")
                return emit

            # xg0 leftovers first (avail immediately), ordered by deadline:
            # fwd chunk tc needed by wall step 32*tc, bwd chunk by 224-32*tc
            pend = []
            for tc_ in range(1, 8):
                pend.append((0, mk_group0(0, tc_)))        # deadline 32*tc_
                pend.append((0, mk_group0(1, 7 - tc_)))    # same deadline
            for d in range(2):
                for tc_ in range(8):
                    pend.append((32 * tc_ + 32, mk_group(d, tc_, 0)))
                    pend.append((N - 32 * tc_, mk_group(d, tc_, 1)))
            pend.sort(key=lambda x: x[0])
            # end-gated groups: rec1 needs xgb[0]c0 / xgf[1]c7 at its step 0,
            # but xgf[0]c7 / xgb[1]c0 only by step ~224 -- emit those two
            # inside rec1's idle time instead
            pend = [e for e in pend if e[0] < N]
            # rec1-bwd(d=1) starts at token 255 (xgf chunk 7); rec1-fwd(d=0)
            # at token 0 (xgb chunk 0): pre-emit only 8 tokens of each
            urgent = [
                lambda: _emit_xg_part(nc, wih1_sb[1], H[0], 1, 7, 24, 32, 0,
                                      xgf_sbs[1], bias_sb[3], xg1_ps),
                lambda: _emit_xg_part(nc, wih1_sb[0], H[0], 0, 0, 0, 8, 1,
                                      xgb_sbs[0], None, xg1_ps),
            ]
            late = [
                lambda: _emit_xg_part(nc, wih1_sb[1], H[0], 1, 7, 0, 24, 0,
                                      xgf_sbs[1], bias_sb[3], xg1_ps),
                lambda: _emit_xg_part(nc, wih1_sb[0], H[0], 0, 0, 8, 32, 1,
                                      xgb_sbs[0], None, xg1_ps),
                mk_group(0, 7, 0), mk_group(1, 0, 1)]
            st = {"i": 0, "last": -10}

            def extra(t):
                n_emit = 0
                while (st["i"] < len(pend) and pend[st["i"]][0] <= t
                       and t > st["last"] and n_emit < 2):
                    pend[st["i"]][1]()
                    st["i"] += 1
                    n_emit += 1
                if n_emit:
                    st["last"] = t

            with tc.tile_pool(name="rst0", bufs=1) as state_pool, \
                 tc.tile_pool(name="sg0", bufs=4) as sg_pool, \
                 tc.tile_pool(name="tmp0", bufs=4) as tmp_pool, \
                 tc.tile_pool(name="rec0ps", bufs=3, space="PSUM") as rec_ps:
                _emit_rec(nc, 0, whh_sb[0:2], xg_sbs, id_sb, H[0],
                          (state_pool, sg_pool, tmp_pool, rec_ps),
                          extra=extra)
            # leftover in-rec0 groups, then the two urgently needed ones
            while st["i"] < len(pend):
                pend[st["i"]][1]()
                st["i"] += 1
            for fn in urgent:
                fn()

            st1 = {"i": 0, "last": -10}

            def extra1(t):
                if st1["i"] < len(late) and t > st1["last"]:
                    late[st1["i"]]()
                    st1["i"] += 1
                    st1["last"] = t

            # ========= rec1 (dual injection: fwd + bwd halves) =========
            with tc.tile_pool(name="rst1", bufs=1) as state_pool, \
                 tc.tile_pool(name="sg1", bufs=4) as sg_pool, \
                 tc.tile_pool(name="tmp1", bufs=4) as tmp_pool, \
                 tc.tile_pool(name="rec1ps", bufs=3, space="PSUM") as rec_ps:
                _emit_rec(nc, 1, whh_sb[2:4], xgf_sbs, id_sb, H[1],
                          (state_pool, sg_pool, tmp_pool, rec_ps),
                          xg2_sbs=xgb_sbs, extra=extra1)
            while st1["i"] < len(late):
                late[st1["i"]]()
                st1["i"] += 1

        # ========= edge scorer =========
        with tc.tile_pool(name="edge", bufs=1) as ep, \
             tc.tile_pool(name="edgeth", bufs=2) as thp, \
             tc.tile_pool(name="edgeps", bufs=1, space="PSUM") as epps, \
             tc.tile_pool(name="edgepsS", bufs=1, space="PSUM") as spps:
            uh_sb = ep.tile([100, 800], BF16, name="uhT", tag="uhT")
            nc.sync.dma_start(out=uh_sb[:, :], in_=uhTd[:, :])
            um_sb = ep.tile([100, 800], BF16, name="umT", tag="umT")
            nc.sync.dma_start(out=um_sb[:, :], in_=umTd[:, :])
            b1_sb = ep.tile([1, 100], F32, name="b1row", tag="b1row")
            nc.sync.dma_start(out=b1_sb[:, :], in_=b1rowd[:, :])
            w2_sb = ep.tile([100, 1], BF16, name="w2", tag="w2")
            nc.sync.dma_start(out=w2_sb[:, :], in_=w2d[:, :])
            b2_sb = ep.tile([128, 1], F32, name="b2", tag="b2")
            nc.sync.dma_start(out=b2_sb[:, :], in_=b2d[:, :])
            selT_sb = ep.tile([128, 64], F32, name="selT", tag="selT")
            nc.sync.dma_start(out=selT_sb[0:128, 0:32], in_=selTd[0])
            nc.sync.dma_start(out=selT_sb[0:128, 32:64], in_=selTd[1])

            def h1_rhs(uc):
                return H[1][uc // 4][0:P, uc % 4: 4 * N: 4]

            # A^T [100, 256] (head half of fc1)
            pA = epps.tile([128, 512], F32, name="e1", tag="e1")
            for uc in range(8):
                nc.tensor.matmul(
                    pA[0:P, 0:256],
                    lhsT=uh_sb[0:P, 100 * uc: 100 * uc + 100],
                    rhs=h1_rhs(uc),
                    start=(uc == 0), stop=(uc == 7), skip_group_check=True)
            A_sb = ep.tile([100, 256], F32, name="A", tag="A")
            nc.vector.tensor_copy(out=A_sb[0:P, 0:256], in_=pA[0:P, 0:256])
            # B^T [100, 256] + b1 (modifier half)
            pB = epps.tile([128, 512], F32, name="e3", tag="e3")
            for uc in range(8):
                nc.tensor.matmul(
                    pB[0:P, 0:256],
                    lhsT=um_sb[0:P, 100 * uc: 100 * uc + 100],
                    rhs=h1_rhs(uc),
                    start=(uc == 0), stop=False, skip_group_check=True)
            nc.tensor.matmul(
                pB[0:P, 0:256],
                lhsT=b1_sb[0:1, 0:100],
                rhs=ones_sb[0:1, 0:256],
                start=False, stop=True, skip_group_check=True)
            B_sb = ep.tile([100, 256], BF16, name="Bsb", tag="Bsb")
            nc.vector.tensor_copy(out=B_sb[0:P, 0:256], in_=pB[0:P, 0:256])
            # A -> token-major via transpose, then per-core 32-head select
            A_tok = ep.tile([128, 256], F32, name="Atok", tag="Atok")
            for m in range(2):
                pT = epps.tile([128, 512], F32, name="e2", tag="e2")
                nc.tensor.transpose(
                    out=pT[0:128, 0:100],
                    in_=A_sb[0:100, 128 * m: 128 * m + 128],
                    identity=idn[0:100, 0:100])
                nc.vector.tensor_copy(
                    out=A_tok[0:128, 128 * m: 128 * m + 100],
                    in_=pT[0:128, 0:100])
            pS = epps.tile([128, 512], F32, name="e1", tag="e1")
            for m in range(2):
                nc.tensor.matmul(
                    pS[0:32, 0:100],
                    lhsT=selT_sb[0:128, 32 * m: 32 * m + 32],
                    rhs=A_tok[0:128, 128 * m: 128 * m + 100],
                    start=(m == 0), stop=(m == 1), skip_group_check=True)
            AselS = ep.tile([128, 128], F32, name="AselS", tag="AselS")
            nc.gpsimd.memset(AselS[:, :], 0.0)
            nc.vector.tensor_copy(out=AselS[0:32, 0:100], in_=pS[0:32, 0:100])
            pAT = epps.tile([128, 512], F32, name="e2", tag="e2")
            nc.tensor.transpose(out=pAT[0:128, 0:128],
                                in_=AselS[0:128, 0:128], identity=idn[:, :])
            AT_sb = ep.tile([128, 32], F32, name="AT", tag="AT")
            nc.vector.tensor_copy(out=AT_sb[0:128, 0:32], in_=pAT[0:128, 0:32])

            # per-head tanh + w2 dot
            psS_tiles = [spps.tile([128, 512], F32, name=f"psS{q}", tag=f"psS{q}")
                         for q in range(4)]
            for q in range(4):
                nc.vector.memset(psS_tiles[q][:, :], 0.0)
            gsb_tiles = [ep.tile([128, 512], F32, name=f"gsb{q}", tag=f"gsb{q}")
                         for q in range(4)]
            # 2 heads per tanh op: DVE pre-adds the per-head bias so the
            # Act op count (each spaced at busy+drain) is halved
            for g in range(16):
                z = thp.tile([100, 512], F32, name=f"z{g % 3}",
                             tag=f"z{g % 3}")
                for j in range(2):
                    nc.vector.tensor_scalar(
                        out=z[0:100, 256 * j: 256 * j + 256],
                        in0=B_sb[0:100, 0:256],
                        scalar1=AT_sb[0:100, 2 * g + j: 2 * g + j + 1],
                        scalar2=None, op0=OP.add)
                th_t = thp.tile([100, 512], BF16, name=f"th{g % 3}",
                                tag=f"th{g % 3}")
                nc.scalar.activation(
                    th_t[0:100, 0:512], z[0:100, 0:512], AF.Tanh)
                for j in range(2):
                    r = 2 * g + j
                    q, half = divmod(r // 4, 2)
                    nc.tensor.matmul(
                        psS_tiles[q][32 * (r % 4): 32 * (r % 4) + 1,
                                     256 * half: 256 * half + 256],
                        lhsT=w2_sb[0:100, 0:1],
                        rhs=th_t[0:100, 256 * j: 256 * j + 256],
                        start=True, stop=True,
                        skip_group_check=True,
                        tile_position=(0, 32 * (r % 4)))
            for q in range(4):
                nc.vector.tensor_scalar(
                    out=gsb_tiles[q][0:128, 0:512],
                    in0=psS_tiles[q][0:128, 0:512],
                    scalar1=b2_sb[0:128, 0:1], scalar2=None, op0=OP.add)
                for half in range(2):
                    rb = 4 * (2 * q + half)
                    nc.sync.dma_start(
                        out=grid[rb:rb + 4, 0:256],
                        in_=gsb_tiles[q][0:128:32, 256 * half: 256 * half + 256])

    nc.compile()
    return nc


_NC_CACHE = None


def _get_nc():
    global _NC_CACHE
    if _NC_CACHE is None:
        _NC_CACHE = build_nc()
    return _NC_CACHE


def kernel(**inputs) -> np.ndarray:
    from concourse.bass_utils import run_bass_kernel_spmd

    arr = _prep_inputs(**inputs)
    nc = _get_nc()
    in_maps = []
    for k in range(NC):
        m = dict(arr)
        m["selT"] = _make_selT(k)
        in_maps.append(m)
    res = run_bass_kernel_spmd(nc, in_maps, core_ids=list(range(NC)))
    grid = np.concatenate([res.results[k]["grid"] for k in range(NC)], axis=0)
    mask = np.ones((N, N), dtype=bool)
    np.fill_diagonal(mask, False)
    mask[:, 0] = False
    return grid[mask].reshape(-1, 1).astype(np.float32)
